# revision 47
# baseline (speedup 1.0000x reference)
"""Trainium2 Bass kernel for nn_Attention_10917806866815.

Multi-head attention forward (B=8, S=32x32=1024, C=768, 12 heads, hd=64),
data-parallel across 8 NeuronCores: core b computes batch element b.
No collectives needed.

Host side (sharding-time prep in kernel()): inputs are pre-transposed to
channel-major and cast to fp16, so the device kernel is pure matmul work:
  xT [768,1024], w_qkvT [768,2304], w_projT [768,768] -- all fp16.

Pipeline (v2.1). The Tile scheduler is dependency-driven (emission order
is only a priority tie-break), and PSUM write-after-read hazards are
tracked per *tile*, so the slot structure is built around two separate
score tiles:

  st_A [128,1024] = c0 of both heads   (q columns 0-511)
  st_B [128,1024] = c1 of both heads   (q columns 512-1023)

Per slot: paired score matmuls (partition bases 0/64 -> disjoint PE row
groups, run concurrently) fill st_A then st_B; ScalarE exps the two
halves separately (exp_A, exp_B) into one merged pt tile [128, 2048].
The next slot's c0 matmuls only WAR-wait on exp_A and c1 only on exp_B,
so the slot cadence is the ScalarE back-to-back rate (~2.3us), not the
previous serialized scores->exp_h0->exp_h1->scores loop (~2.8us).

DMA: a dma_start costs ~1.3us of sequencer issue time, so the inputs
move as 8 large multi-dim-AP transfers (split between the SP and
Activation HWDGE queues), and the mid-kernel normalization bounce plus
the output stores issue from the otherwise-idle GpSimd software-DGE
queue.  Output is stored fp16 (halves traffic; fp16 rounding is far
inside the error budget).

QKV "extras" (projection chunks), PV chunk-0 (1-slot lag), the deferred
chunk-1 bursts, and evacuations are all emitted at low scheduler
priority so a ready score matmul always pops first.

Output projection in the epilogue ping-pongs the freed score tiles
(head = pairs 0-4, tail = pair 5 after the last normalization).

Precision: fp16 operands with fp32 PSUM accumulation.
"""

import numpy as np

import concourse.bass as bass
import concourse.mybir as mybir
import concourse.tile as tile
from concourse import bacc
from concourse.bass_utils import run_bass_kernel_spmd

DIM = 768
S = 1024
NH = 12
HD = 64
SCALE = HD ** -0.5

F32 = mybir.dt.float32
FP16 = mybir.dt.float16

NC_T = S // 128          # 8 token tiles
NC_C = DIM // 128        # 6 channel tiles
NPAIR = NH // 2          # 6 head pairs
VW = HD + 1              # 65: v columns per head incl. ones column

LOWPRI = -1_000_000      # deprioritize non-score work in the ready heap


def build_bass():
    nc = bacc.Bacc(None, target_bir_lowering=False)

    xT_ext = nc.declare_dram_parameter("xT", [DIM, S], FP16, isOutput=False)
    wqkvT_ext = nc.declare_dram_parameter(
        "w_qkvT", [DIM, 3 * DIM], FP16, isOutput=False
    )
    wprojT_ext = nc.declare_dram_parameter(
        "w_projT", [DIM, DIM], FP16, isOutput=False
    )
    out_ext = nc.declare_dram_parameter("out", [S, DIM], FP16, isOutput=True)

    with tile.TileContext(nc) as tc:
        from contextlib import ExitStack

        with ExitStack() as ctx:
            consts = ctx.enter_context(tc.tile_pool(name="consts", bufs=1))
            persist = ctx.enter_context(tc.tile_pool(name="persist", bufs=1))

            # c-major operands: [:, j, :] is channel-tile j.
            xT = persist.tile([128, NC_C, S], FP16, tag="xT", name="xT")
            wqkvT = persist.tile(
                [128, NC_C, 3 * DIM], FP16, tag="wqkvT", name="wqkvT"
            )
            wprojT = persist.tile([128, NC_C, DIM], FP16, tag="wprojT", name="wprojT")

            # ---- bulk input DMA: 8 large transfers, ordered by need ----
            # srcs as [p, k, ...] views of the DRAM tensors
            x_src = xT_ext[:].rearrange("(k p) s -> p k s", k=NC_C)
            w_src = wqkvT_ext[:].rearrange(
                "(k p) (g c) -> p k g c", k=NC_C, g=3
            )
            wp_src = wprojT_ext[:].rearrange("(k p) c -> p k c", k=NC_C)
            w_dst = wqkvT[:].rearrange("p k (g c) -> p k g c", g=3)

            # x split by k-tiles across the SP and Activation HWDGE queues
            # so both column halves land ~12us; q0/k0 ride the Vector
            # queue (small, land early); q1/k1 + wproj on the GpSimd
            # SWDGE which is otherwise idle until the first norm bounce.
            nc.sync.dma_start(out=xT[:, 0:2, :], in_=x_src[:, 0:2, :])
            nc.scalar.dma_start(out=xT[:, 2:4, :], in_=x_src[:, 2:4, :])
            nc.gpsimd.dma_start(                                 # q0
                out=w_dst[:, :, 0, 0:128], in_=w_src[:, :, 0, 0:128]
            )
            nc.gpsimd.dma_start(                                 # k0
                out=w_dst[:, :, 1, 0:128], in_=w_src[:, :, 1, 0:128]
            )
            nc.gpsimd.dma_start(out=xT[:, 4:6, :], in_=x_src[:, 4:6, :])
            nc.gpsimd.dma_start(                                 # q1
                out=w_dst[:, :, 0, 128:256], in_=w_src[:, :, 0, 128:256]
            )
            nc.gpsimd.dma_start(                                 # k1
                out=w_dst[:, :, 1, 128:256], in_=w_src[:, :, 1, 128:256]
            )
            # v heads 0-7 (pair-0 extras need them early)
            nc.sync.dma_start(
                out=w_dst[:, :, 2, 0:512], in_=w_src[:, :, 2, 0:512]
            )
            # q2-5, k2-5
            nc.sync.dma_start(
                out=w_dst[:, :, 0, 256:768], in_=w_src[:, :, 0, 256:768]
            )
            nc.sync.dma_start(
                out=w_dst[:, :, 1, 256:768], in_=w_src[:, :, 1, 256:768]
            )
            # v heads 8-11
            nc.scalar.dma_start(
                out=w_dst[:, :, 2, 512:768], in_=w_src[:, :, 2, 512:768]
            )
            nc.gpsimd.dma_start(out=wprojT[:], in_=wp_src[:])

            qkT = [
                persist.tile([128, S], FP16, tag=f"qkT{ot}", name=f"qkT{ot}")
                for ot in range(2 * NPAIR)
            ]
            # v_ext rows padded to NH*VW+63 so every per-head stationary
            # slice can be 128 columns wide (NumWeights==128 -> the LDW
            # uses fast-weight-load and hides behind in-flight matmuls);
            # PV out rows 65-127 are garbage and never read.
            v_ext = [
                persist.tile([128, NH * VW + 63], FP16, tag=f"vext{tt}",
                             name=f"vext{tt}")
                for tt in range(NC_T)
            ]
            # attnT as column-half tiles: projection q-tiles 0-3 only
            # depend on the c0 half, so pair-5's epilogue normalize can
            # release them early
            attnTa = [
                persist.tile([128, 512], FP16, tag=f"attnTa{p}",
                             name=f"attnTa{p}")
                for p in range(NPAIR)
            ]
            attnTb = [
                persist.tile([128, 512], FP16, tag=f"attnTb{p}",
                             name=f"attnTb{p}")
                for p in range(NPAIR)
            ]

            def attn_q(p, tt):
                # [128, 128] slice of pair p's attnT at q-tile tt
                return (attnTa, attnTb)[tt // 4][p][
                    :, (tt % 4) * 128:(tt % 4 + 1) * 128
                ]
            ones64 = consts.tile([1, 64], FP16, tag="ones64", name="ones64")
            nc.vector.memset(ones64[:], 1.0)
            for tt in range(NC_T):
                nc.gpsimd.memset(v_ext[tt][:], 1.0)

            with (
                tc.tile_pool(name="stps", bufs=1, space="PSUM") as stps,
                tc.tile_pool(name="pvps", bufs=1, space="PSUM") as pvps,
                tc.tile_pool(name="bgps", bufs=1, space="PSUM") as bgps,
                tc.tile_pool(name="ptpool", bufs=1) as ptpool,
                tc.tile_pool(name="normp", bufs=2) as normp,
                tc.tile_pool(name="outp", bufs=3) as outp,
                tc.tile_pool(name="rdram", bufs=2, space="DRAM") as rdram,
            ):
                # 2 shared background PSUM banks: QKV-projection extras,
                # chunk-1 PV bursts, warm-up, norm broadcasts.  Each
                # logical use occupies its tag contiguously in emission
                # order.
                bg_flip = [0]

                def bg_tile(name, shape=(128, 512)):
                    t = bgps.tile(list(shape), F32, tag=f"bg{bg_flip[0]}",
                                  name=name, bufs=1)
                    bg_flip[0] ^= 1
                    return t

                # ---- QKV building blocks ----
                def emit_qk_chunk(ot, c):
                    ps = bg_tile("qkvp")
                    for k in range(NC_C):
                        nc.tensor.matmul(
                            ps[:],
                            wqkvT[:, k, ot * 128:(ot + 1) * 128],
                            xT[:, k, c * 512:(c + 1) * 512],
                            start=(k == 0),
                            stop=(k == NC_C - 1),
                        )
                    nc.vector.tensor_copy(qkT[ot][:, c * 512:(c + 1) * 512], ps[:])

                def emit_v_chunk(tt, c):
                    o0, ow, h0, nh = [
                        (2 * DIM, 512, 0, 8), (2 * DIM + 512, 256, 8, 4)
                    ][c]
                    ps = bg_tile("vp")
                    for k in range(NC_C):
                        nc.tensor.matmul(
                            ps[:, :ow],
                            xT[:, k, tt * 128:(tt + 1) * 128],
                            wqkvT[:, k, o0:o0 + ow],
                            start=(k == 0),
                            stop=(k == NC_C - 1),
                        )
                    dst = (
                        v_ext[tt][:, 0:NH * VW]
                        .rearrange("p (h e) -> p h e", e=VW)[:, h0:h0 + nh, 0:HD]
                    )
                    nc.vector.tensor_copy(
                        dst, ps[:, :ow].rearrange("p (h e) -> p h e", e=HD)
                    )

                # extras[p][T]: QKV work dependencies only require:
                #   v chunk-0 tile T ready before pair-0 PV consumes it at
                #   slot T+1; pair p+1's q/k ready before pair p+1.
                # The dep-driven scheduler fills PE idle time with these
                # (they run at low priority).
                extras = [[[] for _ in range(NC_T)] for _ in range(NPAIR)]

                def TH(f, *a):
                    return lambda: f(*a)

                for tt in range(NC_T):
                    extras[0][tt].append(TH(emit_v_chunk, tt, 0))
                for p in range(1, NPAIR - 1):
                    extras[p][2].append(TH(emit_qk_chunk, p + 1, 0))
                    extras[p][3].append(TH(emit_qk_chunk, NPAIR + p + 1, 0))
                    extras[p][5].append(TH(emit_qk_chunk, p + 1, 1))
                    extras[p][6].append(TH(emit_qk_chunk, NPAIR + p + 1, 1))
                for i in range(NC_T):  # v chunk-1 (needed by pair 4's PV)
                    extras[1 + i // 3][[1, 4, 7][i % 3]].append(
                        TH(emit_v_chunk, i, 1)
                    )

                # ---- HAM warm-up: keep the PE busy through the DMA
                # lead-in so the first real matmuls run at full clock ----
                # 28 matmuls (~7-8us): long enough to cover the input-DMA
                # wait so the PE never sees a >3.4us idle window (which
                # would re-throttle HAM and run the prologue at 1.2 GHz)
                wu = consts.tile([128, 512], FP16, tag="wu", name="wu")
                nc.vector.memset(wu[:], 0.0)
                wups = bg_tile("wups")
                NWU = 12
                for i in range(NWU):
                    nc.tensor.matmul(
                        wups[:], wu[:, 0:128], wu[:],
                        start=(i == 0), stop=(i == NWU - 1),
                    )

                # ---- prologue: q/k for pairs 0 and 1 up front (low
                # priority so pair-0 score matmuls preempt as soon as
                # their chunks land) ----
                with tc.high_priority(offset=LOWPRI):
                    emit_qk_chunk(0, 0)
                    emit_qk_chunk(NPAIR, 0)
                    emit_qk_chunk(0, 1)
                    emit_qk_chunk(NPAIR, 1)
                    emit_qk_chunk(1, 0)
                    emit_qk_chunk(NPAIR + 1, 0)
                    emit_qk_chunk(1, 1)
                    emit_qk_chunk(NPAIR + 1, 1)

                # ---- attention: software-pipelined slot stream ----
                pts_of = {}     # (p, T) -> pt tile [128, 2048] h-major
                pv0_of = {}     # p -> [pv0_h0, pv0_h1]  (chunk-0 accums)
                pv1_of = {}     # last pair only: incremental chunk-1 accums
                sums_sb_of = {}

                def sc_mm(st, p, T, c):
                    kT_t = qkT[NPAIR + p]
                    qT_t = qkT[p]
                    for h in range(2):
                        r0 = h * 64
                        nc.tensor.matmul(
                            st[:, h * 512:(h + 1) * 512],
                            kT_t[r0:r0 + 64, T * 128:(T + 1) * 128],
                            qT_t[r0:r0 + 64, c * 512:(c + 1) * 512],
                            start=True,
                            stop=True,
                        )

                def slot(p, T):
                    if T == 0:
                        sums_sb_of[p] = normp.tile(
                            [1, 2 * S], F32, tag="sums", name="sums", bufs=2
                        )
                        pv0_of[p] = [
                            pvps.tile([128, 512], F32, tag=f"pva{h}",
                                      name=f"pva{h}", bufs=1)
                            for h in range(2)
                        ]
                        if p == NPAIR - 1:
                            # last pair: chunk-1 accumulates incrementally
                            # in the (now extras-free) background banks so
                            # the epilogue isn't serialized behind a burst;
                            # sums go to per-half tiles so each half's
                            # reciprocal fires as soon as its rows land
                            pv1_of[p] = [bg_tile(f"pvL{h}") for h in range(2)]
                            sums5[0] = normp.tile([1, 2, 512], F32,
                                                  tag="s5c0", name="s5c0",
                                                  bufs=1)
                            sums5[1] = normp.tile([1, 2, 512], F32,
                                                  tag="s5c1", name="s5c1",
                                                  bufs=1)
                    st_a = stps.tile([128, S], F32, tag="stA", name="stA",
                                     bufs=1)
                    st_b = stps.tile([128, S], F32, tag="stB", name="stB",
                                     bufs=1)
                    pt = ptpool.tile([128, 2 * S], FP16, tag=f"pt{T}",
                                     name=f"pt{T}", bufs=2)
                    pts_of[(p, T)] = pt
                    pt_r = pt[:].rearrange("p (h c q) -> p h c q", h=2, q=512)

                    sc_mm(st_a, p, T, 0)
                    nc.scalar.activation(
                        out=pt_r[:, :, 0, :],
                        in_=st_a[:].rearrange("p (h q) -> p h q", h=2),
                        func=mybir.ActivationFunctionType.Exp,
                        scale=float(SCALE),
                    )
                    with tc.high_priority(offset=LOWPRI):
                        if T > 0:
                            for h in range(2):
                                nc.tensor.matmul(
                                    pv0_of[p][h][:],
                                    v_ext[T - 1][
                                        :, (2 * p + h) * VW:(2 * p + h) * VW + 128
                                    ],
                                    pts_of[(p, T - 1)][:, h * S:h * S + 512],
                                    start=(T == 1),
                                    stop=(T == NC_T - 1),
                                )
                        if p == NPAIR - 1 and T > 0:
                            for h in range(2):
                                nc.tensor.matmul(
                                    pv1_of[p][h][:],
                                    v_ext[T - 1][
                                        :, (2 * p + h) * VW:(2 * p + h) * VW + 128
                                    ],
                                    pts_of[(p, T - 1)][:, h * S + 512:(h + 1) * S],
                                    start=(T == 1),
                                    stop=(T == NC_T - 1),
                                )
                        for th in extras[p][T]:
                            th()
                    sc_mm(st_b, p, T, 1)
                    nc.scalar.activation(
                        out=pt_r[:, :, 1, :],
                        in_=st_b[:].rearrange("p (h q) -> p h q", h=2),
                        func=mybir.ActivationFunctionType.Exp,
                        scale=float(SCALE),
                    )

                def scalar_recip(dst, src):
                    nc.scalar.add_instruction(
                        mybir.InstActivation(
                            name=nc.get_next_instruction_name(),
                            ins=[
                                nc.scalar.lower_ap(src),
                                mybir.ImmediateValue(
                                    dtype=mybir.dt.float32, value=0.0
                                ),
                                mybir.ImmediateValue(
                                    dtype=mybir.dt.float32, value=1.0
                                ),
                                mybir.ImmediateValue(
                                    dtype=mybir.dt.float32, value=0.0
                                ),
                            ],
                            outs=[nc.scalar.lower_ap(dst)],
                            func=mybir.ActivationFunctionType.Reciprocal,
                        )
                    )

                recip5 = {}
                sums5 = {}

                def finish_c0(p):
                    for h in range(2):
                        nc.tensor.matmul(
                            pv0_of[p][h][:],
                            v_ext[NC_T - 1][
                                :, (2 * p + h) * VW:(2 * p + h) * VW + 128
                            ],
                            pts_of[(p, NC_T - 1)][:, h * S:h * S + 512],
                            start=False,
                            stop=True,
                        )
                    for h in range(2):
                        nc.vector.tensor_copy(
                            sums5[0][0:1, h, :] if p == NPAIR - 1
                            else sums_sb_of[p][0:1, h * S:h * S + 512],
                            pv0_of[p][h][HD:HD + 1, :],
                        )
                    if p == NPAIR - 1:
                        # preload the reciprocal ACT table set (the real
                        # reciprocals would otherwise pay the ~2.7us table
                        # switch on the critical tail), then the c0-half
                        # reciprocal as soon as its sums rows land
                        scalar_recip(
                            normp.tile([1, 1], F32, tag="rscr", name="rscr",
                                       bufs=1)[:],
                            ones64[0:1, 0:1],
                        )
                        recip5[0] = normp.tile([1, 2, 512], FP16, tag="rc0",
                                               name="rc0", bufs=1)
                        scalar_recip(recip5[0][:], sums5[0][:])
                    for h in range(2):
                        nc.vector.tensor_copy(
                            attnTa[p][h * 64:(h + 1) * 64, :],
                            pv0_of[p][h][0:HD, :],
                        )
                    del pv0_of[p]

                def finish_c1_last(p):
                    for h in range(2):
                        nc.tensor.matmul(
                            pv1_of[p][h][:],
                            v_ext[NC_T - 1][
                                :, (2 * p + h) * VW:(2 * p + h) * VW + 128
                            ],
                            pts_of[(p, NC_T - 1)][:, h * S + 512:(h + 1) * S],
                            start=False,
                            stop=True,
                        )
                    for h in range(2):
                        nc.vector.tensor_copy(
                            sums5[1][0:1, h, :],
                            pv1_of[p][h][HD:HD + 1, :],
                        )
                    recip5[1] = normp.tile([1, 2, 512], FP16, tag="rc1",
                                           name="rc1", bufs=1)
                    scalar_recip(recip5[1][:], sums5[1][:])
                    # attnTb copies are deferred to norm5_half(1) so the
                    # c0-half normalize multiplies run first on the DVE
                    for Tq in range(NC_T):
                        del pts_of[(p, Tq)]

                def burst_c1(p):
                    pv1 = [bg_tile(f"pvb{h}") for h in range(2)]
                    for Tq in range(NC_T):
                        for h in range(2):
                            nc.tensor.matmul(
                                pv1[h][:],
                                v_ext[Tq][
                                    :, (2 * p + h) * VW:(2 * p + h) * VW + 128
                                ],
                                pts_of[(p, Tq)][:, h * S + 512:(h + 1) * S],
                                start=(Tq == 0),
                                stop=(Tq == NC_T - 1),
                            )
                    for h in range(2):
                        nc.vector.tensor_copy(
                            sums_sb_of[p][0:1, h * S + 512:h * S + 1024],
                            pv1[h][HD:HD + 1, :],
                        )
                        nc.vector.tensor_copy(
                            attnTb[p][h * 64:(h + 1) * 64, :],
                            pv1[h][0:HD, :],
                        )
                    for Tq in range(NC_T):
                        del pts_of[(p, Tq)]
                    norm(p)

                def norm(p):
                    # reciprocal of the 2048 sums: repartition [1,2048] ->
                    # [128,16] via a DRAM bounce (issued on the idle GpSimd
                    # SWDGE queue) so the 8-cycle/element DVE divide runs
                    # on 128 lanes (pairs 0-4; the last pair is handled by
                    # norm5_half on the epilogue path)
                    sums_sb = sums_sb_of[p]
                    rd = rdram.tile([1, 2 * S], F32, tag="rd", name="rd")
                    sd = rdram.tile([1, 2 * S], F32, tag="sd", name="sd")
                    nc.gpsimd.dma_start(out=sd[:], in_=sums_sb[:])
                    sr = normp.tile([128, 16], F32, tag="sr", name="sr")
                    nc.gpsimd.dma_start(
                        out=sr[:],
                        in_=bass.AP(
                            tensor=sd.tensor,
                            offset=sd.offset,
                            ap=[[16, 128], [1, 16]],
                        ),
                    )
                    rr = normp.tile([128, 16], F32, tag="rr", name="rr")
                    nc.vector.reciprocal(rr[:], sr[:])
                    nc.gpsimd.dma_start(
                        out=bass.AP(
                            tensor=rd.tensor,
                            offset=rd.offset,
                            ap=[[16, 128], [1, 16]],
                        ),
                        in_=rr[:],
                    )
                    rb = normp.tile([128, S], F32, tag="rb", name="rb")
                    for h in range(2):
                        row = rd[0:1, h * S:(h + 1) * S]
                        row_bc = bass.AP(
                            tensor=row.tensor,
                            offset=row.offset,
                            ap=[[0, 64]] + list(row.ap[1:]),
                        )
                        nc.gpsimd.dma_start(
                            out=rb[h * 64:(h + 1) * 64, :], in_=row_bc
                        )
                    rb_r = rb[:].rearrange("d (c q) -> d c q", q=512)
                    nc.vector.tensor_mul(
                        attnTa[p][:], attnTa[p][:], rb_r[:, 0, :]
                    )
                    nc.vector.tensor_mul(
                        attnTb[p][:], attnTb[p][:], rb_r[:, 1, :]
                    )

                def norm5_half(c):
                    # last pair, one column half: broadcast 1/sums via two
                    # matmuls into a freed pva bank (bg banks still hold
                    # the unread chunk-1 accumulators), then normalize
                    p = NPAIR - 1
                    at = (attnTa, attnTb)[c][p]
                    bc = pvps.tile([128, 512], F32, tag=f"pva{c}",
                                   name=f"bc{c}", bufs=1)
                    for h in range(2):
                        nc.tensor.matmul(
                            bc[h * 64:(h + 1) * 64, :],
                            ones64[0:1, :],
                            recip5[c][0:1, h, :],
                            start=True,
                            stop=True,
                        )
                    if c == 1:
                        for h in range(2):
                            nc.vector.tensor_copy(
                                at[h * 64:(h + 1) * 64, :],
                                pv1_of[p][h][0:HD, :],
                            )
                        del pv1_of[p]
                    for h in range(2):
                        nc.vector.tensor_mul(
                            at[h * 64:(h + 1) * 64, :],
                            at[h * 64:(h + 1) * 64, :],
                            bc[h * 64:(h + 1) * 64, :],
                        )

                # emission order: chunk-1 burst of pair p-1 deferred past
                # the next pair's first two slots (low priority keeps it
                # out of the scores' way).  Pair 4's burst is un-deferred
                # (the bg banks belong to pair 5's incremental chunk-1
                # during pair 5), and pair 5 finishes both chunks inline.
                for p in range(NPAIR):
                    slot(p, 0)
                    slot(p, 1)
                    if 0 < p < NPAIR - 1:
                        with tc.high_priority(offset=LOWPRI):
                            burst_c1(p - 1)
                    for T in range(2, NC_T):
                        slot(p, T)
                    with tc.high_priority(offset=LOWPRI):
                        finish_c0(p)
                        if p == NPAIR - 2:
                            burst_c1(p)
                        elif p == NPAIR - 1:
                            finish_c1_last(p)
                # pair-5 norm is emitted between the first two projection
                # heads: the PE instruction stream is static, so the bc
                # matmuls must sit AFTER ~4us of head matmuls to cover the
                # ScalarE reciprocal (+table load) latency without a stall

                # ---------------- output projection ----------------
                # PSUM ping-pongs the freed score tiles (tags stA/stB).
                # Depth-2 pipeline: each tile's pair-5 matmul (gated by the
                # last normalization) is deferred past the next tile's
                # early matmuls.  Output stores go out fp16 on the GpSimd
                # queue.
                def proj_head(tt):
                    ps = stps.tile([128, S], F32,
                                   tag=("stA", "stB")[tt % 2], name=f"prj{tt}",
                                   bufs=1)
                    for o0, ow in [(0, 512), (512, 256)]:
                        for p in range(NPAIR - 1):
                            nc.tensor.matmul(
                                ps[:, o0:o0 + ow],
                                attn_q(p, tt),
                                wprojT[:, p, o0:o0 + ow],
                                start=(p == 0),
                                stop=False,
                            )
                    return ps

                def proj_tail(tt, ps):
                    for o0, ow in [(0, 512), (512, 256)]:
                        nc.tensor.matmul(
                            ps[:, o0:o0 + ow],
                            attn_q(NPAIR - 1, tt),
                            wprojT[:, NPAIR - 1, o0:o0 + ow],
                            start=False,
                            stop=True,
                        )
                    ob = outp.tile([128, DIM], FP16, tag="ob", name="ob")
                    nc.vector.tensor_copy(ob[:, 0:512], ps[:, 0:512])
                    nc.scalar.copy(out=ob[:, 512:768], in_=ps[:, 512:768])
                    nc.gpsimd.dma_start(
                        out=out_ext[tt * 128:(tt + 1) * 128, :], in_=ob[:]
                    )

                # [head0, bc_c0, head1, bc_c1, tail0, head2, tail1, ...]:
                # each norm half sits behind a head's worth of PE work so
                # the split reciprocals are ready when the PE reaches the
                # bc matmuls, and tails 0-3 only need the c0 half
                pending = None
                for tt in range(NC_T):
                    ps = proj_head(tt)
                    if tt <= 1:
                        with tc.high_priority(offset=LOWPRI):
                            norm5_half(tt)
                    if pending is not None:
                        proj_tail(*pending)
                    pending = (tt, ps)
                proj_tail(*pending)

    nc.finalize()
    return nc


_NC_CACHE = None


def kernel(**inputs) -> np.ndarray:
    global _NC_CACHE
    x = np.asarray(inputs["x"], dtype=np.float32)
    w_qkv = np.asarray(inputs["w_qkv"], dtype=np.float32)
    w_proj = np.asarray(inputs["w_proj"], dtype=np.float32)
    b_proj = np.asarray(inputs["b_proj"], dtype=np.float32)
    B, H, W, C = x.shape
    assert (B, H * W, C) == (8, S, DIM)

    # host-side sharding + layout prep: channel-major fp16 operands
    wqkvT = np.ascontiguousarray(w_qkv.T).astype(np.float16)       # [768, 2304]
    wprojT = np.ascontiguousarray(w_proj.T).astype(np.float16)     # [768, 768]
    xTs = [
        np.ascontiguousarray(x[b].reshape(S, DIM).T).astype(np.float16)
        for b in range(B)
    ]

    if _NC_CACHE is None:
        _NC_CACHE = build_bass()
    nc = _NC_CACHE

    in_maps = [
        {"xT": xTs[b], "w_qkvT": wqkvT, "w_projT": wprojT}
        for b in range(B)
    ]
    res = run_bass_kernel_spmd(nc, in_maps, list(range(B)))
    out = np.stack(
        [
            np.asarray(res.results[b]["out"]).astype(np.float32).reshape(H, W, C)
            for b in range(B)
        ]
    )
    return (out + b_proj.reshape(1, 1, 1, C)).astype(np.float32)


if __name__ == "__main__":
    rng = np.random.default_rng(0)
    ins = {
        "x": rng.standard_normal((8, 32, 32, DIM), dtype=np.float32),
        "w_qkv": rng.standard_normal((3 * DIM, DIM), dtype=np.float32)
        * DIM ** -0.5,
        "w_proj": rng.standard_normal((DIM, DIM), dtype=np.float32) * DIM ** -0.5,
        "b_proj": np.zeros(DIM, dtype=np.float32),
    }
    o = kernel(**ins)
    print(o.shape, o.dtype)



# revision 58
# speedup vs baseline: 1.0057x; 1.0057x over previous
"""Trainium2 Bass kernel for nn_Attention_10917806866815.

Multi-head attention forward (B=8, S=32x32=1024, C=768, 12 heads, hd=64),
data-parallel across 8 NeuronCores: core b computes batch element b.
No collectives needed.

Host side (sharding-time prep in kernel()): inputs are pre-transposed to
channel-major and cast to fp16, so the device kernel is pure matmul work:
  xT [768,1024], w_qkvT [768,2304], w_projT [768,768] -- all fp16.

Pipeline (v2.1). The Tile scheduler is dependency-driven (emission order
is only a priority tie-break), and PSUM write-after-read hazards are
tracked per *tile*, so the slot structure is built around two separate
score tiles:

  st_A [128,1024] = c0 of both heads   (q columns 0-511)
  st_B [128,1024] = c1 of both heads   (q columns 512-1023)

Per slot: paired score matmuls (partition bases 0/64 -> disjoint PE row
groups, run concurrently) fill st_A then st_B; ScalarE exps the two
halves separately (exp_A, exp_B) into one merged pt tile [128, 2048].
The next slot's c0 matmuls only WAR-wait on exp_A and c1 only on exp_B,
so the slot cadence is the ScalarE back-to-back rate (~2.3us), not the
previous serialized scores->exp_h0->exp_h1->scores loop (~2.8us).

DMA: a dma_start costs ~1.3us of sequencer issue time, so the inputs
move as 8 large multi-dim-AP transfers (split between the SP and
Activation HWDGE queues), and the mid-kernel normalization bounce plus
the output stores issue from the otherwise-idle GpSimd software-DGE
queue.  Output is stored fp16 (halves traffic; fp16 rounding is far
inside the error budget).

QKV "extras" (projection chunks), PV chunk-0 (1-slot lag), the deferred
chunk-1 bursts, and evacuations are all emitted at low scheduler
priority so a ready score matmul always pops first.

Output projection in the epilogue ping-pongs the freed score tiles
(head = pairs 0-4, tail = pair 5 after the last normalization).

Precision: fp16 operands with fp32 PSUM accumulation.
"""

import numpy as np

import concourse.bass as bass
import concourse.mybir as mybir
import concourse.tile as tile
from concourse import bacc
from concourse.bass_utils import run_bass_kernel_spmd

DIM = 768
S = 1024
NH = 12
HD = 64
SCALE = HD ** -0.5

F32 = mybir.dt.float32
FP16 = mybir.dt.float16

NC_T = S // 128          # 8 token tiles
NC_C = DIM // 128        # 6 channel tiles
NPAIR = NH // 2          # 6 head pairs
VW = HD + 1              # 65: v columns per head incl. ones column

LOWPRI = -1_000_000      # deprioritize non-score work in the ready heap


def build_bass():
    nc = bacc.Bacc(None, target_bir_lowering=False)

    xT_ext = nc.declare_dram_parameter("xT", [DIM, S], FP16, isOutput=False)
    wqkvT_ext = nc.declare_dram_parameter(
        "w_qkvT", [DIM, 3 * DIM], FP16, isOutput=False
    )
    wprojT_ext = nc.declare_dram_parameter(
        "w_projT", [DIM, DIM], FP16, isOutput=False
    )
    out_ext = nc.declare_dram_parameter("out", [S, DIM], FP16, isOutput=True)

    with tile.TileContext(nc) as tc:
        from contextlib import ExitStack

        with ExitStack() as ctx:
            consts = ctx.enter_context(tc.tile_pool(name="consts", bufs=1))
            persist = ctx.enter_context(tc.tile_pool(name="persist", bufs=1))

            # c-major operands: [:, j, :] is channel-tile j.
            xT = persist.tile([128, NC_C, S], FP16, tag="xT", name="xT")
            wqkvT = persist.tile(
                [128, NC_C, 3 * DIM], FP16, tag="wqkvT", name="wqkvT"
            )
            wprojT = persist.tile([128, NC_C, DIM], FP16, tag="wprojT", name="wprojT")

            # ---- bulk input DMA: 8 large transfers, ordered by need ----
            # srcs as [p, k, ...] views of the DRAM tensors
            x_src = xT_ext[:].rearrange("(k p) s -> p k s", k=NC_C)
            w_src = wqkvT_ext[:].rearrange(
                "(k p) (g c) -> p k g c", k=NC_C, g=3
            )
            wp_src = wprojT_ext[:].rearrange("(k p) c -> p k c", k=NC_C)
            w_dst = wqkvT[:].rearrange("p k (g c) -> p k g c", g=3)

            # x split by k-tiles across the SP and Activation HWDGE queues
            # so both column halves land ~12us; q0/k0 ride the Vector
            # queue (small, land early); q1/k1 + wproj on the GpSimd
            # SWDGE which is otherwise idle until the first norm bounce.
            nc.sync.dma_start(out=xT[:, 0:3, :], in_=x_src[:, 0:3, :])
            nc.scalar.dma_start(out=xT[:, 3:6, :], in_=x_src[:, 3:6, :])
            nc.gpsimd.dma_start(                                 # q0
                out=w_dst[:, :, 0, 0:128], in_=w_src[:, :, 0, 0:128]
            )
            nc.gpsimd.dma_start(                                 # k0
                out=w_dst[:, :, 1, 0:128], in_=w_src[:, :, 1, 0:128]
            )
            nc.gpsimd.dma_start(                                 # q1
                out=w_dst[:, :, 0, 128:256], in_=w_src[:, :, 0, 128:256]
            )
            nc.gpsimd.dma_start(                                 # k1
                out=w_dst[:, :, 1, 128:256], in_=w_src[:, :, 1, 128:256]
            )
            # v heads 0-7 (pair-0 extras need them early)
            nc.sync.dma_start(
                out=w_dst[:, :, 2, 0:512], in_=w_src[:, :, 2, 0:512]
            )
            # q2-5, k2-5
            nc.sync.dma_start(
                out=w_dst[:, :, 0, 256:768], in_=w_src[:, :, 0, 256:768]
            )
            nc.sync.dma_start(
                out=w_dst[:, :, 1, 256:768], in_=w_src[:, :, 1, 256:768]
            )
            # v heads 8-11
            nc.scalar.dma_start(
                out=w_dst[:, :, 2, 512:768], in_=w_src[:, :, 2, 512:768]
            )
            nc.gpsimd.dma_start(out=wprojT[:], in_=wp_src[:])

            qkT = [
                persist.tile([128, S], FP16, tag=f"qkT{ot}", name=f"qkT{ot}")
                for ot in range(2 * NPAIR)
            ]
            # v_ext rows padded to NH*VW+63 so every per-head stationary
            # slice can be 128 columns wide (NumWeights==128 -> the LDW
            # uses fast-weight-load and hides behind in-flight matmuls);
            # PV out rows 65-127 are garbage and never read.
            v_ext = [
                persist.tile([128, NH * VW + 63], FP16, tag=f"vext{tt}",
                             name=f"vext{tt}")
                for tt in range(NC_T)
            ]
            # attnT as column-half tiles: projection q-tiles 0-3 only
            # depend on the c0 half, so pair-5's epilogue normalize can
            # release them early
            attnTa = [
                persist.tile([128, 512], FP16, tag=f"attnTa{p}",
                             name=f"attnTa{p}")
                for p in range(NPAIR)
            ]
            attnTb = [
                persist.tile([128, 512], FP16, tag=f"attnTb{p}",
                             name=f"attnTb{p}")
                for p in range(NPAIR)
            ]

            def attn_q(p, tt):
                # [128, 128] slice of pair p's attnT at q-tile tt
                return (attnTa, attnTb)[tt // 4][p][
                    :, (tt % 4) * 128:(tt % 4 + 1) * 128
                ]
            ones64 = consts.tile([1, 64], FP16, tag="ones64", name="ones64")
            nc.vector.memset(ones64[:], 1.0)
            for tt in range(NC_T):
                nc.gpsimd.memset(v_ext[tt][:], 1.0)

            with (
                tc.tile_pool(name="stps", bufs=1, space="PSUM") as stps,
                tc.tile_pool(name="pvps", bufs=1, space="PSUM") as pvps,
                tc.tile_pool(name="bgps", bufs=1, space="PSUM") as bgps,
                tc.tile_pool(name="ptpool", bufs=1) as ptpool,
                tc.tile_pool(name="normp", bufs=2) as normp,
                tc.tile_pool(name="outp", bufs=3) as outp,
                tc.tile_pool(name="rdram", bufs=2, space="DRAM") as rdram,
            ):
                # 2 shared background PSUM banks: QKV-projection extras,
                # chunk-1 PV bursts, warm-up, norm broadcasts.  Each
                # logical use occupies its tag contiguously in emission
                # order.
                bg_flip = [0]

                def bg_tile(name, shape=(128, 512)):
                    t = bgps.tile(list(shape), F32, tag=f"bg{bg_flip[0]}",
                                  name=name, bufs=1)
                    bg_flip[0] ^= 1
                    return t

                # ---- QKV building blocks ----
                def emit_qk_chunk(ot, c):
                    ps = bg_tile("qkvp")
                    for k in range(NC_C):
                        nc.tensor.matmul(
                            ps[:],
                            wqkvT[:, k, ot * 128:(ot + 1) * 128],
                            xT[:, k, c * 512:(c + 1) * 512],
                            start=(k == 0),
                            stop=(k == NC_C - 1),
                        )
                    nc.vector.tensor_copy(qkT[ot][:, c * 512:(c + 1) * 512], ps[:])

                def emit_v_chunk(tt, c):
                    o0, ow, h0, nh = [
                        (2 * DIM, 512, 0, 8), (2 * DIM + 512, 256, 8, 4)
                    ][c]
                    ps = bg_tile("vp")
                    for k in range(NC_C):
                        nc.tensor.matmul(
                            ps[:, :ow],
                            xT[:, k, tt * 128:(tt + 1) * 128],
                            wqkvT[:, k, o0:o0 + ow],
                            start=(k == 0),
                            stop=(k == NC_C - 1),
                        )
                    dst = (
                        v_ext[tt][:, 0:NH * VW]
                        .rearrange("p (h e) -> p h e", e=VW)[:, h0:h0 + nh, 0:HD]
                    )
                    nc.vector.tensor_copy(
                        dst, ps[:, :ow].rearrange("p (h e) -> p h e", e=HD)
                    )

                # extras[p][T]: QKV work dependencies only require:
                #   v chunk-0 tile T ready before pair-0 PV consumes it at
                #   slot T+1; pair p+1's q/k ready before pair p+1.
                # The dep-driven scheduler fills PE idle time with these
                # (they run at low priority).
                extras = [[[] for _ in range(NC_T)] for _ in range(NPAIR)]

                def TH(f, *a):
                    return lambda: f(*a)

                for tt in range(NC_T):
                    extras[0][tt].append(TH(emit_v_chunk, tt, 0))
                for p in range(1, NPAIR - 1):
                    extras[p][2].append(TH(emit_qk_chunk, p + 1, 0))
                    extras[p][3].append(TH(emit_qk_chunk, NPAIR + p + 1, 0))
                    extras[p][5].append(TH(emit_qk_chunk, p + 1, 1))
                    extras[p][6].append(TH(emit_qk_chunk, NPAIR + p + 1, 1))
                for i in range(NC_T):  # v chunk-1 (needed by pair 4's PV)
                    extras[1 + i // 3][[1, 4, 7][i % 3]].append(
                        TH(emit_v_chunk, i, 1)
                    )

                # ---- HAM warm-up: keep the PE busy through the DMA
                # lead-in so the first real matmuls run at full clock ----
                # 28 matmuls (~7-8us): long enough to cover the input-DMA
                # wait so the PE never sees a >3.4us idle window (which
                # would re-throttle HAM and run the prologue at 1.2 GHz)
                wu = consts.tile([128, 512], FP16, tag="wu", name="wu")
                nc.vector.memset(wu[:], 0.0)
                wups = bg_tile("wups")
                NWU = 18
                for i in range(NWU):
                    nc.tensor.matmul(
                        wups[:], wu[:, 0:128], wu[:],
                        start=(i == 0), stop=(i == NWU - 1),
                    )

                # ---- prologue: q/k for pairs 0 and 1 up front (low
                # priority so pair-0 score matmuls preempt as soon as
                # their chunks land) ----
                with tc.high_priority(offset=LOWPRI):
                    emit_qk_chunk(0, 0)
                    emit_qk_chunk(NPAIR, 0)
                    emit_qk_chunk(0, 1)
                    emit_qk_chunk(NPAIR, 1)
                    emit_qk_chunk(1, 0)
                    emit_qk_chunk(NPAIR + 1, 0)
                    emit_qk_chunk(1, 1)
                    emit_qk_chunk(NPAIR + 1, 1)

                # ---- attention: software-pipelined slot stream ----
                pts_of = {}     # (p, T) -> pt tile [128, 2048] h-major
                pv0_of = {}     # p -> [pv0_h0, pv0_h1]  (chunk-0 accums)
                pv1_of = {}     # last pair only: incremental chunk-1 accums
                sums_sb_of = {}

                def sc_mm(st, p, T, c):
                    kT_t = qkT[NPAIR + p]
                    qT_t = qkT[p]
                    for h in range(2):
                        r0 = h * 64
                        nc.tensor.matmul(
                            st[:, h * 512:(h + 1) * 512],
                            kT_t[r0:r0 + 64, T * 128:(T + 1) * 128],
                            qT_t[r0:r0 + 64, c * 512:(c + 1) * 512],
                            start=True,
                            stop=True,
                        )

                def slot(p, T):
                    if T == 0:
                        sums_sb_of[p] = normp.tile(
                            [1, 2 * S], F32, tag="sums", name="sums", bufs=2
                        )
                        pv0_of[p] = [
                            pvps.tile([128, 512], F32, tag=f"pva{h}",
                                      name=f"pva{h}", bufs=1)
                            for h in range(2)
                        ]
                        if p == NPAIR - 1:
                            # last pair: chunk-1 accumulates incrementally
                            # in the (now extras-free) background banks so
                            # the epilogue isn't serialized behind a burst;
                            # sums go to per-half tiles so each half's
                            # reciprocal fires as soon as its rows land
                            pv1_of[p] = [bg_tile(f"pvL{h}") for h in range(2)]
                            sums5[0] = normp.tile([1, 2, 512], F32,
                                                  tag="s5c0", name="s5c0",
                                                  bufs=1)
                            sums5[1] = normp.tile([1, 2, 512], F32,
                                                  tag="s5c1", name="s5c1",
                                                  bufs=1)
                    st_a = stps.tile([128, S], F32, tag="stA", name="stA",
                                     bufs=1)
                    st_b = stps.tile([128, S], F32, tag="stB", name="stB",
                                     bufs=1)
                    # per-half pt tiles: chunk-0 consumers only RAW-wait on
                    # exp_A, chunk-1 only on exp_B
                    ptA = ptpool.tile([128, S], FP16, tag=f"ptA{T}",
                                      name=f"ptA{T}", bufs=2)
                    ptB = ptpool.tile([128, S], FP16, tag=f"ptB{T}",
                                      name=f"ptB{T}", bufs=2)
                    pts_of[(p, T)] = (ptA, ptB)

                    sc_mm(st_a, p, T, 0)
                    nc.scalar.activation(
                        out=ptA[:].rearrange("p (h q) -> p h q", h=2),
                        in_=st_a[:].rearrange("p (h q) -> p h q", h=2),
                        func=mybir.ActivationFunctionType.Exp,
                        scale=float(SCALE),
                    )
                    with tc.high_priority(offset=LOWPRI):
                        if T > 0:
                            for h in range(2):
                                nc.tensor.matmul(
                                    pv0_of[p][h][:],
                                    v_ext[T - 1][
                                        :, (2 * p + h) * VW:(2 * p + h) * VW + 128
                                    ],
                                    pts_of[(p, T - 1)][0][:, h * 512:(h + 1) * 512],
                                    start=(T == 1),
                                    stop=(T == NC_T - 1),
                                )
                        if p == NPAIR - 1 and T > 0:
                            for h in range(2):
                                nc.tensor.matmul(
                                    pv1_of[p][h][:],
                                    v_ext[T - 1][
                                        :, (2 * p + h) * VW:(2 * p + h) * VW + 128
                                    ],
                                    pts_of[(p, T - 1)][1][:, h * 512:(h + 1) * 512],
                                    start=(T == 1),
                                    stop=(T == NC_T - 1),
                                )
                        for th in extras[p][T]:
                            th()
                    sc_mm(st_b, p, T, 1)
                    nc.scalar.activation(
                        out=ptB[:].rearrange("p (h q) -> p h q", h=2),
                        in_=st_b[:].rearrange("p (h q) -> p h q", h=2),
                        func=mybir.ActivationFunctionType.Exp,
                        scale=float(SCALE),
                    )

                def scalar_recip(dst, src):
                    nc.scalar.add_instruction(
                        mybir.InstActivation(
                            name=nc.get_next_instruction_name(),
                            ins=[
                                nc.scalar.lower_ap(src),
                                mybir.ImmediateValue(
                                    dtype=mybir.dt.float32, value=0.0
                                ),
                                mybir.ImmediateValue(
                                    dtype=mybir.dt.float32, value=1.0
                                ),
                                mybir.ImmediateValue(
                                    dtype=mybir.dt.float32, value=0.0
                                ),
                            ],
                            outs=[nc.scalar.lower_ap(dst)],
                            func=mybir.ActivationFunctionType.Reciprocal,
                        )
                    )

                recip5 = {}
                sums5 = {}

                def finish_c0(p):
                    for h in range(2):
                        nc.tensor.matmul(
                            pv0_of[p][h][:],
                            v_ext[NC_T - 1][
                                :, (2 * p + h) * VW:(2 * p + h) * VW + 128
                            ],
                            pts_of[(p, NC_T - 1)][0][:, h * 512:(h + 1) * 512],
                            start=False,
                            stop=True,
                        )
                    for h in range(2):
                        nc.vector.tensor_copy(
                            sums5[0][0:1, h, :] if p == NPAIR - 1
                            else sums_sb_of[p][0:1, h * S:h * S + 512],
                            pv0_of[p][h][HD:HD + 1, :],
                        )
                    if p == NPAIR - 1:
                        # preload the reciprocal ACT table set (the real
                        # reciprocals would otherwise pay the ~2.7us table
                        # switch on the critical tail), then the c0-half
                        # reciprocal as soon as its sums rows land
                        scalar_recip(
                            normp.tile([1, 1], F32, tag="rscr", name="rscr",
                                       bufs=1)[:],
                            ones64[0:1, 0:1],
                        )
                        recip5[0] = normp.tile([1, 2, 512], FP16, tag="rc0",
                                               name="rc0", bufs=1)
                        scalar_recip(recip5[0][:], sums5[0][:])
                    for h in range(2):
                        nc.vector.tensor_copy(
                            attnTa[p][h * 64:(h + 1) * 64, :],
                            pv0_of[p][h][0:HD, :],
                        )
                    del pv0_of[p]

                def finish_c1_last(p):
                    for h in range(2):
                        nc.tensor.matmul(
                            pv1_of[p][h][:],
                            v_ext[NC_T - 1][
                                :, (2 * p + h) * VW:(2 * p + h) * VW + 128
                            ],
                            pts_of[(p, NC_T - 1)][1][:, h * 512:(h + 1) * 512],
                            start=False,
                            stop=True,
                        )
                    for h in range(2):
                        nc.vector.tensor_copy(
                            sums5[1][0:1, h, :],
                            pv1_of[p][h][HD:HD + 1, :],
                        )
                    recip5[1] = normp.tile([1, 2, 512], FP16, tag="rc1",
                                           name="rc1", bufs=1)
                    scalar_recip(recip5[1][:], sums5[1][:])
                    # attnTb copies are deferred to norm5_half(1) so the
                    # c0-half normalize multiplies run first on the DVE
                    for Tq in range(NC_T):
                        del pts_of[(p, Tq)]

                def burst_c1(p):
                    pv1 = [bg_tile(f"pvb{h}") for h in range(2)]
                    for Tq in range(NC_T):
                        for h in range(2):
                            nc.tensor.matmul(
                                pv1[h][:],
                                v_ext[Tq][
                                    :, (2 * p + h) * VW:(2 * p + h) * VW + 128
                                ],
                                pts_of[(p, Tq)][1][:, h * 512:(h + 1) * 512],
                                start=(Tq == 0),
                                stop=(Tq == NC_T - 1),
                            )
                    for h in range(2):
                        nc.vector.tensor_copy(
                            sums_sb_of[p][0:1, h * S + 512:h * S + 1024],
                            pv1[h][HD:HD + 1, :],
                        )
                        nc.vector.tensor_copy(
                            attnTb[p][h * 64:(h + 1) * 64, :],
                            pv1[h][0:HD, :],
                        )
                    for Tq in range(NC_T):
                        del pts_of[(p, Tq)]
                    norm(p)

                def norm(p):
                    # reciprocal of the 2048 sums: repartition [1,2048] ->
                    # [128,16] via a DRAM bounce (issued on the idle GpSimd
                    # SWDGE queue) so the 8-cycle/element DVE divide runs
                    # on 128 lanes (pairs 0-4; the last pair is handled by
                    # norm5_half on the epilogue path)
                    sums_sb = sums_sb_of[p]
                    rd = rdram.tile([1, 2 * S], F32, tag="rd", name="rd")
                    sd = rdram.tile([1, 2 * S], F32, tag="sd", name="sd")
                    nc.gpsimd.dma_start(out=sd[:], in_=sums_sb[:])
                    sr = normp.tile([128, 16], F32, tag="sr", name="sr")
                    nc.gpsimd.dma_start(
                        out=sr[:],
                        in_=bass.AP(
                            tensor=sd.tensor,
                            offset=sd.offset,
                            ap=[[16, 128], [1, 16]],
                        ),
                    )
                    rr = normp.tile([128, 16], F32, tag="rr", name="rr")
                    nc.vector.reciprocal(rr[:], sr[:])
                    nc.gpsimd.dma_start(
                        out=bass.AP(
                            tensor=rd.tensor,
                            offset=rd.offset,
                            ap=[[16, 128], [1, 16]],
                        ),
                        in_=rr[:],
                    )
                    rb = normp.tile([128, S], F32, tag="rb", name="rb")
                    for h in range(2):
                        row = rd[0:1, h * S:(h + 1) * S]
                        row_bc = bass.AP(
                            tensor=row.tensor,
                            offset=row.offset,
                            ap=[[0, 64]] + list(row.ap[1:]),
                        )
                        nc.gpsimd.dma_start(
                            out=rb[h * 64:(h + 1) * 64, :], in_=row_bc
                        )
                    rb_r = rb[:].rearrange("d (c q) -> d c q", q=512)
                    nc.vector.tensor_mul(
                        attnTa[p][:], attnTa[p][:], rb_r[:, 0, :]
                    )
                    nc.vector.tensor_mul(
                        attnTb[p][:], attnTb[p][:], rb_r[:, 1, :]
                    )

                def norm5_half(c):
                    # last pair, one column half: broadcast 1/sums via two
                    # matmuls into a freed pva bank (bg banks still hold
                    # the unread chunk-1 accumulators), then normalize
                    p = NPAIR - 1
                    at = (attnTa, attnTb)[c][p]
                    bc = pvps.tile([128, 512], F32, tag=f"pva{c}",
                                   name=f"bc{c}", bufs=1)
                    for h in range(2):
                        nc.tensor.matmul(
                            bc[h * 64:(h + 1) * 64, :],
                            ones64[0:1, :],
                            recip5[c][0:1, h, :],
                            start=True,
                            stop=True,
                        )
                    if c == 1:
                        for h in range(2):
                            nc.vector.tensor_copy(
                                at[h * 64:(h + 1) * 64, :],
                                pv1_of[p][h][0:HD, :],
                            )
                        del pv1_of[p]
                    for h in range(2):
                        nc.vector.tensor_mul(
                            at[h * 64:(h + 1) * 64, :],
                            at[h * 64:(h + 1) * 64, :],
                            bc[h * 64:(h + 1) * 64, :],
                        )

                # emission order: chunk-1 burst of pair p-1 deferred past
                # the next pair's first two slots (low priority keeps it
                # out of the scores' way).  Pair 4's burst is un-deferred
                # (the bg banks belong to pair 5's incremental chunk-1
                # during pair 5), and pair 5 finishes both chunks inline.
                for p in range(NPAIR):
                    slot(p, 0)
                    slot(p, 1)
                    if 0 < p < NPAIR - 1:
                        with tc.high_priority(offset=LOWPRI):
                            burst_c1(p - 1)
                    for T in range(2, NC_T):
                        slot(p, T)
                    if p == NPAIR - 1:
                        # the last pair's finish/normalize chain is the
                        # epilogue critical path: it must outrank the
                        # projection-head filler matmuls
                        with tc.high_priority(offset=1000):
                            finish_c0(p)
                            finish_c1_last(p)
                    else:
                        with tc.high_priority(offset=LOWPRI):
                            finish_c0(p)
                            if p == NPAIR - 2:
                                burst_c1(p)
                # pair-5 norm is emitted between the first two projection
                # heads: the PE instruction stream is static, so the bc
                # matmuls must sit AFTER ~4us of head matmuls to cover the
                # ScalarE reciprocal (+table load) latency without a stall

                # ---------------- output projection ----------------
                # PSUM ping-pongs the freed score tiles (tags stA/stB).
                # Depth-2 pipeline: each tile's pair-5 matmul (gated by the
                # last normalization) is deferred past the next tile's
                # early matmuls.  Output stores go out fp16 on the GpSimd
                # queue.
                def proj_head(tt):
                    ps = stps.tile([128, S], F32,
                                   tag=("stA", "stB")[tt % 2], name=f"prj{tt}",
                                   bufs=1)
                    for o0, ow in [(0, 512), (512, 256)]:
                        for p in range(NPAIR - 1):
                            nc.tensor.matmul(
                                ps[:, o0:o0 + ow],
                                attn_q(p, tt),
                                wprojT[:, p, o0:o0 + ow],
                                start=(p == 0),
                                stop=False,
                            )
                    return ps

                def proj_tail(tt, ps):
                    for o0, ow in [(0, 512), (512, 256)]:
                        nc.tensor.matmul(
                            ps[:, o0:o0 + ow],
                            attn_q(NPAIR - 1, tt),
                            wprojT[:, NPAIR - 1, o0:o0 + ow],
                            start=False,
                            stop=True,
                        )
                    ob = outp.tile([128, DIM], FP16, tag="ob", name="ob")
                    nc.vector.tensor_copy(ob[:, 0:512], ps[:, 0:512])
                    nc.scalar.copy(out=ob[:, 512:768], in_=ps[:, 512:768])
                    nc.gpsimd.dma_start(
                        out=out_ext[tt * 128:(tt + 1) * 128, :], in_=ob[:]
                    )

                # [head0, bc_c0, head1, bc_c1, tail0, head2, tail1, ...]:
                # each norm half sits behind a head's worth of PE work so
                # the split reciprocals are ready when the PE reaches the
                # bc matmuls, and tails 0-3 only need the c0 half
                pending = None
                for tt in range(NC_T):
                    ps = proj_head(tt)
                    if tt <= 1:
                        with tc.high_priority(offset=1000):
                            norm5_half(tt)
                    if pending is not None:
                        proj_tail(*pending)
                    pending = (tt, ps)
                proj_tail(*pending)

    nc.finalize()
    return nc


_NC_CACHE = None


def kernel(**inputs) -> np.ndarray:
    global _NC_CACHE
    x = np.asarray(inputs["x"], dtype=np.float32)
    w_qkv = np.asarray(inputs["w_qkv"], dtype=np.float32)
    w_proj = np.asarray(inputs["w_proj"], dtype=np.float32)
    b_proj = np.asarray(inputs["b_proj"], dtype=np.float32)
    B, H, W, C = x.shape
    assert (B, H * W, C) == (8, S, DIM)

    # host-side sharding + layout prep: channel-major fp16 operands
    wqkvT = np.ascontiguousarray(w_qkv.T).astype(np.float16)       # [768, 2304]
    wprojT = np.ascontiguousarray(w_proj.T).astype(np.float16)     # [768, 768]
    xTs = [
        np.ascontiguousarray(x[b].reshape(S, DIM).T).astype(np.float16)
        for b in range(B)
    ]

    if _NC_CACHE is None:
        _NC_CACHE = build_bass()
    nc = _NC_CACHE

    in_maps = [
        {"xT": xTs[b], "w_qkvT": wqkvT, "w_projT": wprojT}
        for b in range(B)
    ]
    res = run_bass_kernel_spmd(nc, in_maps, list(range(B)))
    out = np.stack(
        [
            np.asarray(res.results[b]["out"]).astype(np.float32).reshape(H, W, C)
            for b in range(B)
        ]
    )
    return (out + b_proj.reshape(1, 1, 1, C)).astype(np.float32)


if __name__ == "__main__":
    rng = np.random.default_rng(0)
    ins = {
        "x": rng.standard_normal((8, 32, 32, DIM), dtype=np.float32),
        "w_qkv": rng.standard_normal((3 * DIM, DIM), dtype=np.float32)
        * DIM ** -0.5,
        "w_proj": rng.standard_normal((DIM, DIM), dtype=np.float32) * DIM ** -0.5,
        "b_proj": np.zeros(DIM, dtype=np.float32),
    }
    o = kernel(**ins)
    print(o.shape, o.dtype)



# revision 61
# speedup vs baseline: 1.0255x; 1.0197x over previous
"""Trainium2 Bass kernel for nn_Attention_10917806866815.

Multi-head attention forward (B=8, S=32x32=1024, C=768, 12 heads, hd=64),
data-parallel across 8 NeuronCores: core b computes batch element b.
No collectives needed.

Host side (sharding-time prep in kernel()): inputs are pre-transposed to
channel-major and cast to fp16, so the device kernel is pure matmul work:
  xT [768,1024], w_qkvT [768,2304], w_projT [768,768] -- all fp16.

Pipeline (v2.1). The Tile scheduler is dependency-driven (emission order
is only a priority tie-break), and PSUM write-after-read hazards are
tracked per *tile*, so the slot structure is built around two separate
score tiles:

  st_A [128,1024] = c0 of both heads   (q columns 0-511)
  st_B [128,1024] = c1 of both heads   (q columns 512-1023)

Per slot: paired score matmuls (partition bases 0/64 -> disjoint PE row
groups, run concurrently) fill st_A then st_B; ScalarE exps the two
halves separately (exp_A, exp_B) into one merged pt tile [128, 2048].
The next slot's c0 matmuls only WAR-wait on exp_A and c1 only on exp_B,
so the slot cadence is the ScalarE back-to-back rate (~2.3us), not the
previous serialized scores->exp_h0->exp_h1->scores loop (~2.8us).

DMA: a dma_start costs ~1.3us of sequencer issue time, so the inputs
move as 8 large multi-dim-AP transfers (split between the SP and
Activation HWDGE queues), and the mid-kernel normalization bounce plus
the output stores issue from the otherwise-idle GpSimd software-DGE
queue.  Output is stored fp16 (halves traffic; fp16 rounding is far
inside the error budget).

QKV "extras" (projection chunks), PV chunk-0 (1-slot lag), the deferred
chunk-1 bursts, and evacuations are all emitted at low scheduler
priority so a ready score matmul always pops first.

Output projection in the epilogue ping-pongs the freed score tiles
(head = pairs 0-4, tail = pair 5 after the last normalization).

Precision: fp16 operands with fp32 PSUM accumulation.
"""

import numpy as np

import concourse.bass as bass
import concourse.mybir as mybir
import concourse.tile as tile
from concourse import bacc
from concourse.bass_utils import run_bass_kernel_spmd

DIM = 768
S = 1024
NH = 12
HD = 64
SCALE = HD ** -0.5

F32 = mybir.dt.float32
FP16 = mybir.dt.float16

NC_T = S // 128          # 8 token tiles
NC_C = DIM // 128        # 6 channel tiles
NPAIR = NH // 2          # 6 head pairs
VW = HD + 1              # 65: v columns per head incl. ones column

LOWPRI = -1_000_000      # deprioritize non-score work in the ready heap


def build_bass():
    nc = bacc.Bacc(None, target_bir_lowering=False)

    xT_ext = nc.declare_dram_parameter("xT", [DIM, S], FP16, isOutput=False)
    wqkvT_ext = nc.declare_dram_parameter(
        "w_qkvT", [DIM, 3 * DIM], FP16, isOutput=False
    )
    wprojT_ext = nc.declare_dram_parameter(
        "w_projT", [DIM, DIM], FP16, isOutput=False
    )
    out_ext = nc.declare_dram_parameter("out", [S, DIM], FP16, isOutput=True)

    with tile.TileContext(nc) as tc:
        from contextlib import ExitStack

        with ExitStack() as ctx:
            consts = ctx.enter_context(tc.tile_pool(name="consts", bufs=1))
            persist = ctx.enter_context(tc.tile_pool(name="persist", bufs=1))

            # c-major operands: [:, j, :] is channel-tile j.
            xT = persist.tile([128, NC_C, S], FP16, tag="xT", name="xT")
            wqkvT = persist.tile(
                [128, NC_C, 3 * DIM], FP16, tag="wqkvT", name="wqkvT"
            )
            wprojT = persist.tile([128, NC_C, DIM], FP16, tag="wprojT", name="wprojT")

            # ---- bulk input DMA: 8 large transfers, ordered by need ----
            # srcs as [p, k, ...] views of the DRAM tensors
            x_src = xT_ext[:].rearrange("(k p) s -> p k s", k=NC_C)
            w_src = wqkvT_ext[:].rearrange(
                "(k p) (g c) -> p k g c", k=NC_C, g=3
            )
            wp_src = wprojT_ext[:].rearrange("(k p) c -> p k c", k=NC_C)
            w_dst = wqkvT[:].rearrange("p k (g c) -> p k g c", g=3)

            # x split by k-tiles across the SP and Activation HWDGE queues
            # so both column halves land ~12us; q0/k0 ride the Vector
            # queue (small, land early); q1/k1 + wproj on the GpSimd
            # SWDGE which is otherwise idle until the first norm bounce.
            nc.sync.dma_start(out=xT[:, 0:3, :], in_=x_src[:, 0:3, :])
            nc.scalar.dma_start(out=xT[:, 3:6, :], in_=x_src[:, 3:6, :])
            nc.gpsimd.dma_start(                                 # q0
                out=w_dst[:, :, 0, 0:128], in_=w_src[:, :, 0, 0:128]
            )
            nc.gpsimd.dma_start(                                 # k0
                out=w_dst[:, :, 1, 0:128], in_=w_src[:, :, 1, 0:128]
            )
            nc.gpsimd.dma_start(                                 # q1
                out=w_dst[:, :, 0, 128:256], in_=w_src[:, :, 0, 128:256]
            )
            nc.gpsimd.dma_start(                                 # k1
                out=w_dst[:, :, 1, 128:256], in_=w_src[:, :, 1, 128:256]
            )
            # v heads 0-7 (pair-0 extras need them early)
            nc.sync.dma_start(
                out=w_dst[:, :, 2, 0:512], in_=w_src[:, :, 2, 0:512]
            )
            # q2-5, k2-5
            nc.sync.dma_start(
                out=w_dst[:, :, 0, 256:768], in_=w_src[:, :, 0, 256:768]
            )
            nc.sync.dma_start(
                out=w_dst[:, :, 1, 256:768], in_=w_src[:, :, 1, 256:768]
            )
            # v heads 8-11
            nc.scalar.dma_start(
                out=w_dst[:, :, 2, 512:768], in_=w_src[:, :, 2, 512:768]
            )
            nc.gpsimd.dma_start(out=wprojT[:], in_=wp_src[:])

            qkT = [
                persist.tile([128, S], FP16, tag=f"qkT{ot}", name=f"qkT{ot}")
                for ot in range(2 * NPAIR)
            ]
            # v_ext rows padded to NH*VW+63 so every per-head stationary
            # slice can be 128 columns wide (NumWeights==128 -> the LDW
            # uses fast-weight-load and hides behind in-flight matmuls);
            # PV out rows 65-127 are garbage and never read.
            v_ext = [
                persist.tile([128, NH * VW + 63], FP16, tag=f"vext{tt}",
                             name=f"vext{tt}")
                for tt in range(NC_T)
            ]
            # attnT as column-half tiles: projection q-tiles 0-3 only
            # depend on the c0 half, so pair-5's epilogue normalize can
            # release them early
            attnTa = [
                persist.tile([128, 512], FP16, tag=f"attnTa{p}",
                             name=f"attnTa{p}")
                for p in range(NPAIR)
            ]
            attnTb = [
                persist.tile([128, 512], FP16, tag=f"attnTb{p}",
                             name=f"attnTb{p}")
                for p in range(NPAIR)
            ]

            def attn_q(p, tt):
                # [128, 128] slice of pair p's attnT at q-tile tt
                return (attnTa, attnTb)[tt // 4][p][
                    :, (tt % 4) * 128:(tt % 4 + 1) * 128
                ]
            ones64 = consts.tile([1, 64], FP16, tag="ones64", name="ones64")
            nc.vector.memset(ones64[:], 1.0)
            for tt in range(NC_T):
                nc.gpsimd.memset(v_ext[tt][:], 1.0)

            with (
                tc.tile_pool(name="stps", bufs=1, space="PSUM") as stps,
                tc.tile_pool(name="pvps", bufs=1, space="PSUM") as pvps,
                tc.tile_pool(name="bgps", bufs=1, space="PSUM") as bgps,
                tc.tile_pool(name="ptpool", bufs=1) as ptpool,
                tc.tile_pool(name="normp", bufs=2) as normp,
                tc.tile_pool(name="outp", bufs=3) as outp,
                tc.tile_pool(name="rdram", bufs=2, space="DRAM") as rdram,
            ):
                # 2 shared background PSUM banks: QKV-projection extras,
                # chunk-1 PV bursts, warm-up, norm broadcasts.  Each
                # logical use occupies its tag contiguously in emission
                # order.
                bg_flip = [0]

                def bg_tile(name, shape=(128, 512)):
                    t = bgps.tile(list(shape), F32, tag=f"bg{bg_flip[0]}",
                                  name=name, bufs=1)
                    bg_flip[0] ^= 1
                    return t

                # ---- QKV building blocks ----
                def emit_qk_chunk(ot, c):
                    ps = bg_tile("qkvp")
                    for k in range(NC_C):
                        nc.tensor.matmul(
                            ps[:],
                            wqkvT[:, k, ot * 128:(ot + 1) * 128],
                            xT[:, k, c * 512:(c + 1) * 512],
                            start=(k == 0),
                            stop=(k == NC_C - 1),
                        )
                    nc.vector.tensor_copy(qkT[ot][:, c * 512:(c + 1) * 512], ps[:])

                def emit_v_chunk(tt, c):
                    o0, ow, h0, nh = [
                        (2 * DIM, 512, 0, 8), (2 * DIM + 512, 256, 8, 4)
                    ][c]
                    ps = bg_tile("vp")
                    for k in range(NC_C):
                        nc.tensor.matmul(
                            ps[:, :ow],
                            xT[:, k, tt * 128:(tt + 1) * 128],
                            wqkvT[:, k, o0:o0 + ow],
                            start=(k == 0),
                            stop=(k == NC_C - 1),
                        )
                    dst = (
                        v_ext[tt][:, 0:NH * VW]
                        .rearrange("p (h e) -> p h e", e=VW)[:, h0:h0 + nh, 0:HD]
                    )
                    nc.vector.tensor_copy(
                        dst, ps[:, :ow].rearrange("p (h e) -> p h e", e=HD)
                    )

                # extras[p][T]: QKV work dependencies only require:
                #   v chunk-0 tile T ready before pair-0 PV consumes it at
                #   slot T+1; pair p+1's q/k ready before pair p+1.
                # The dep-driven scheduler fills PE idle time with these
                # (they run at low priority).
                extras = [[[] for _ in range(NC_T)] for _ in range(NPAIR)]

                def TH(f, *a):
                    return lambda: f(*a)

                for tt in range(NC_T):
                    extras[0][tt].append(TH(emit_v_chunk, tt, 0))
                for p in range(1, NPAIR - 1):
                    extras[p][2].append(TH(emit_qk_chunk, p + 1, 0))
                    extras[p][3].append(TH(emit_qk_chunk, NPAIR + p + 1, 0))
                    extras[p][5].append(TH(emit_qk_chunk, p + 1, 1))
                    extras[p][6].append(TH(emit_qk_chunk, NPAIR + p + 1, 1))
                for i in range(NC_T):  # v chunk-1 (needed by pair 4's PV)
                    extras[1 + i // 3][[1, 4, 7][i % 3]].append(
                        TH(emit_v_chunk, i, 1)
                    )

                # ---- HAM warm-up: keep the PE busy through the DMA
                # lead-in so the first real matmuls run at full clock ----
                # 28 matmuls (~7-8us): long enough to cover the input-DMA
                # wait so the PE never sees a >3.4us idle window (which
                # would re-throttle HAM and run the prologue at 1.2 GHz)
                wu = consts.tile([128, 512], FP16, tag="wu", name="wu")
                nc.vector.memset(wu[:], 0.0)
                wups = bg_tile("wups")
                NWU = 18
                for i in range(NWU):
                    nc.tensor.matmul(
                        wups[:], wu[:, 0:128], wu[:],
                        start=(i == 0), stop=(i == NWU - 1),
                    )

                # ---- prologue: q/k for pairs 0 and 1 up front (low
                # priority so pair-0 score matmuls preempt as soon as
                # their chunks land) ----
                with tc.high_priority(offset=LOWPRI):
                    emit_qk_chunk(0, 0)
                    emit_qk_chunk(NPAIR, 0)
                    emit_qk_chunk(0, 1)
                    emit_qk_chunk(NPAIR, 1)
                    emit_qk_chunk(1, 0)
                    emit_qk_chunk(NPAIR + 1, 0)
                    emit_qk_chunk(1, 1)
                    emit_qk_chunk(NPAIR + 1, 1)

                # ---- attention: software-pipelined slot stream ----
                pts_of = {}     # (p, T) -> pt tile [128, 2048] h-major
                pv0_of = {}     # p -> [pv0_h0, pv0_h1]  (chunk-0 accums)
                pv1_of = {}     # last pair only: incremental chunk-1 accums
                sums_sb_of = {}

                def sc_mm(st, p, T, c):
                    kT_t = qkT[NPAIR + p]
                    qT_t = qkT[p]
                    for h in range(2):
                        r0 = h * 64
                        nc.tensor.matmul(
                            st[:, h * 512:(h + 1) * 512],
                            kT_t[r0:r0 + 64, T * 128:(T + 1) * 128],
                            qT_t[r0:r0 + 64, c * 512:(c + 1) * 512],
                            start=True,
                            stop=True,
                        )

                def slot(p, T):
                    if T == 0:
                        sums_sb_of[p] = normp.tile(
                            [1, 2 * S], F32, tag="sums", name="sums", bufs=2
                        )
                        pv0_of[p] = [
                            pvps.tile([128, 512], F32, tag=f"pva{h}",
                                      name=f"pva{h}", bufs=1)
                            for h in range(2)
                        ]
                        if p == NPAIR - 1:
                            # last pair: chunk-1 accumulates incrementally
                            # in the (now extras-free) background banks so
                            # the epilogue isn't serialized behind a burst;
                            # sums go to per-half tiles so each half's
                            # reciprocal fires as soon as its rows land
                            pv1_of[p] = [bg_tile(f"pvL{h}") for h in range(2)]
                            sums5[0] = normp.tile([1, 2, 512], F32,
                                                  tag="s5c0", name="s5c0",
                                                  bufs=1)
                            sums5[1] = normp.tile([1, 2, 512], F32,
                                                  tag="s5c1", name="s5c1",
                                                  bufs=1)
                    st_a = stps.tile([128, S], F32, tag="stA", name="stA",
                                     bufs=1)
                    st_b = stps.tile([128, S], F32, tag="stB", name="stB",
                                     bufs=1)
                    # per-half pt tiles: chunk-0 consumers only RAW-wait on
                    # exp_A, chunk-1 only on exp_B
                    ptA = ptpool.tile([128, S], FP16, tag=f"ptA{T}",
                                      name=f"ptA{T}", bufs=2)
                    ptB = ptpool.tile([128, S], FP16, tag=f"ptB{T}",
                                      name=f"ptB{T}", bufs=2)
                    pts_of[(p, T)] = (ptA, ptB)

                    sc_mm(st_a, p, T, 0)
                    nc.scalar.activation(
                        out=ptA[:].rearrange("p (h q) -> p h q", h=2),
                        in_=st_a[:].rearrange("p (h q) -> p h q", h=2),
                        func=mybir.ActivationFunctionType.Exp,
                        scale=float(SCALE),
                    )
                    # the very last slot's PV feeds the epilogue critical
                    # path: normal priority so the finish chain isn't
                    # stuck behind projection-head filler
                    last_slot = p == NPAIR - 1 and T == NC_T - 1
                    with tc.high_priority(offset=0 if last_slot else LOWPRI):
                        if T > 0:
                            for h in range(2):
                                nc.tensor.matmul(
                                    pv0_of[p][h][:],
                                    v_ext[T - 1][
                                        :, (2 * p + h) * VW:(2 * p + h) * VW + 128
                                    ],
                                    pts_of[(p, T - 1)][0][:, h * 512:(h + 1) * 512],
                                    start=(T == 1),
                                    stop=(T == NC_T - 1),
                                )
                        if p == NPAIR - 1 and T > 0:
                            for h in range(2):
                                nc.tensor.matmul(
                                    pv1_of[p][h][:],
                                    v_ext[T - 1][
                                        :, (2 * p + h) * VW:(2 * p + h) * VW + 128
                                    ],
                                    pts_of[(p, T - 1)][1][:, h * 512:(h + 1) * 512],
                                    start=(T == 1),
                                    stop=(T == NC_T - 1),
                                )
                        for th in extras[p][T]:
                            th()
                    sc_mm(st_b, p, T, 1)
                    nc.scalar.activation(
                        out=ptB[:].rearrange("p (h q) -> p h q", h=2),
                        in_=st_b[:].rearrange("p (h q) -> p h q", h=2),
                        func=mybir.ActivationFunctionType.Exp,
                        scale=float(SCALE),
                    )

                def scalar_recip(dst, src):
                    nc.scalar.add_instruction(
                        mybir.InstActivation(
                            name=nc.get_next_instruction_name(),
                            ins=[
                                nc.scalar.lower_ap(src),
                                mybir.ImmediateValue(
                                    dtype=mybir.dt.float32, value=0.0
                                ),
                                mybir.ImmediateValue(
                                    dtype=mybir.dt.float32, value=1.0
                                ),
                                mybir.ImmediateValue(
                                    dtype=mybir.dt.float32, value=0.0
                                ),
                            ],
                            outs=[nc.scalar.lower_ap(dst)],
                            func=mybir.ActivationFunctionType.Reciprocal,
                        )
                    )

                recip5 = {}
                sums5 = {}

                def finish_c0(p):
                    for h in range(2):
                        nc.tensor.matmul(
                            pv0_of[p][h][:],
                            v_ext[NC_T - 1][
                                :, (2 * p + h) * VW:(2 * p + h) * VW + 128
                            ],
                            pts_of[(p, NC_T - 1)][0][:, h * 512:(h + 1) * 512],
                            start=False,
                            stop=True,
                        )
                    for h in range(2):
                        nc.vector.tensor_copy(
                            sums5[0][0:1, h, :] if p == NPAIR - 1
                            else sums_sb_of[p][0:1, h * S:h * S + 512],
                            pv0_of[p][h][HD:HD + 1, :],
                        )
                    if p == NPAIR - 1:
                        # preload the reciprocal ACT table set (the real
                        # reciprocals would otherwise pay the ~2.7us table
                        # switch on the critical tail), then the c0-half
                        # reciprocal as soon as its sums rows land
                        scalar_recip(
                            normp.tile([1, 1], F32, tag="rscr", name="rscr",
                                       bufs=1)[:],
                            ones64[0:1, 0:1],
                        )
                        recip5[0] = normp.tile([1, 2, 512], FP16, tag="rc0",
                                               name="rc0", bufs=1)
                        scalar_recip(recip5[0][:], sums5[0][:])
                    for h in range(2):
                        nc.vector.tensor_copy(
                            attnTa[p][h * 64:(h + 1) * 64, :],
                            pv0_of[p][h][0:HD, :],
                        )
                    del pv0_of[p]

                def finish_c1_last(p):
                    for h in range(2):
                        nc.tensor.matmul(
                            pv1_of[p][h][:],
                            v_ext[NC_T - 1][
                                :, (2 * p + h) * VW:(2 * p + h) * VW + 128
                            ],
                            pts_of[(p, NC_T - 1)][1][:, h * 512:(h + 1) * 512],
                            start=False,
                            stop=True,
                        )
                    for h in range(2):
                        nc.vector.tensor_copy(
                            sums5[1][0:1, h, :],
                            pv1_of[p][h][HD:HD + 1, :],
                        )
                    recip5[1] = normp.tile([1, 2, 512], FP16, tag="rc1",
                                           name="rc1", bufs=1)
                    scalar_recip(recip5[1][:], sums5[1][:])
                    # attnTb copies are deferred to norm5_half(1) so the
                    # c0-half normalize multiplies run first on the DVE
                    for Tq in range(NC_T):
                        del pts_of[(p, Tq)]

                def burst_c1(p):
                    pv1 = [bg_tile(f"pvb{h}") for h in range(2)]
                    for Tq in range(NC_T):
                        for h in range(2):
                            nc.tensor.matmul(
                                pv1[h][:],
                                v_ext[Tq][
                                    :, (2 * p + h) * VW:(2 * p + h) * VW + 128
                                ],
                                pts_of[(p, Tq)][1][:, h * 512:(h + 1) * 512],
                                start=(Tq == 0),
                                stop=(Tq == NC_T - 1),
                            )
                    for h in range(2):
                        nc.vector.tensor_copy(
                            sums_sb_of[p][0:1, h * S + 512:h * S + 1024],
                            pv1[h][HD:HD + 1, :],
                        )
                        nc.vector.tensor_copy(
                            attnTb[p][h * 64:(h + 1) * 64, :],
                            pv1[h][0:HD, :],
                        )
                    for Tq in range(NC_T):
                        del pts_of[(p, Tq)]
                    norm(p)

                def norm(p):
                    # reciprocal of the 2048 sums: repartition [1,2048] ->
                    # [128,16] via a DRAM bounce (issued on the idle GpSimd
                    # SWDGE queue) so the 8-cycle/element DVE divide runs
                    # on 128 lanes (pairs 0-4; the last pair is handled by
                    # norm5_half on the epilogue path)
                    sums_sb = sums_sb_of[p]
                    rd = rdram.tile([1, 2 * S], F32, tag="rd", name="rd")
                    sd = rdram.tile([1, 2 * S], F32, tag="sd", name="sd")
                    nc.gpsimd.dma_start(out=sd[:], in_=sums_sb[:])
                    sr = normp.tile([128, 16], F32, tag="sr", name="sr")
                    nc.gpsimd.dma_start(
                        out=sr[:],
                        in_=bass.AP(
                            tensor=sd.tensor,
                            offset=sd.offset,
                            ap=[[16, 128], [1, 16]],
                        ),
                    )
                    rr = normp.tile([128, 16], F32, tag="rr", name="rr")
                    nc.vector.reciprocal(rr[:], sr[:])
                    nc.gpsimd.dma_start(
                        out=bass.AP(
                            tensor=rd.tensor,
                            offset=rd.offset,
                            ap=[[16, 128], [1, 16]],
                        ),
                        in_=rr[:],
                    )
                    rb = normp.tile([128, S], F32, tag="rb", name="rb")
                    for h in range(2):
                        row = rd[0:1, h * S:(h + 1) * S]
                        row_bc = bass.AP(
                            tensor=row.tensor,
                            offset=row.offset,
                            ap=[[0, 64]] + list(row.ap[1:]),
                        )
                        nc.gpsimd.dma_start(
                            out=rb[h * 64:(h + 1) * 64, :], in_=row_bc
                        )
                    rb_r = rb[:].rearrange("d (c q) -> d c q", q=512)
                    nc.vector.tensor_mul(
                        attnTa[p][:], attnTa[p][:], rb_r[:, 0, :]
                    )
                    nc.vector.tensor_mul(
                        attnTb[p][:], attnTb[p][:], rb_r[:, 1, :]
                    )

                def norm5_half(c):
                    # last pair, one column half: broadcast 1/sums via two
                    # matmuls into a freed pva bank (bg banks still hold
                    # the unread chunk-1 accumulators), then normalize
                    p = NPAIR - 1
                    at = (attnTa, attnTb)[c][p]
                    bc = pvps.tile([128, 512], F32, tag=f"pva{c}",
                                   name=f"bc{c}", bufs=1)
                    for h in range(2):
                        nc.tensor.matmul(
                            bc[h * 64:(h + 1) * 64, :],
                            ones64[0:1, :],
                            recip5[c][0:1, h, :],
                            start=True,
                            stop=True,
                        )
                    if c == 1:
                        for h in range(2):
                            nc.vector.tensor_copy(
                                at[h * 64:(h + 1) * 64, :],
                                pv1_of[p][h][0:HD, :],
                            )
                        del pv1_of[p]
                    for h in range(2):
                        nc.vector.tensor_mul(
                            at[h * 64:(h + 1) * 64, :],
                            at[h * 64:(h + 1) * 64, :],
                            bc[h * 64:(h + 1) * 64, :],
                        )

                # emission order: chunk-1 burst of pair p-1 deferred past
                # the next pair's first two slots (low priority keeps it
                # out of the scores' way).  Pair 4's burst is un-deferred
                # (the bg banks belong to pair 5's incremental chunk-1
                # during pair 5), and pair 5 finishes both chunks inline.
                for p in range(NPAIR):
                    slot(p, 0)
                    slot(p, 1)
                    if 0 < p < NPAIR - 1:
                        with tc.high_priority(offset=LOWPRI):
                            burst_c1(p - 1)
                    for T in range(2, NC_T):
                        slot(p, T)
                    if p == NPAIR - 1:
                        # the last pair's finish/normalize chain is the
                        # epilogue critical path: absolute top priority so
                        # it always outranks the projection-head filler
                        with tc.high_priority():
                            finish_c0(p)
                            finish_c1_last(p)
                    else:
                        with tc.high_priority(offset=LOWPRI):
                            finish_c0(p)
                            if p == NPAIR - 2:
                                burst_c1(p)
                # pair-5 norm is emitted between the first two projection
                # heads: the PE instruction stream is static, so the bc
                # matmuls must sit AFTER ~4us of head matmuls to cover the
                # ScalarE reciprocal (+table load) latency without a stall

                # ---------------- output projection ----------------
                # PSUM ping-pongs the freed score tiles (tags stA/stB).
                # Depth-2 pipeline: each tile's pair-5 matmul (gated by the
                # last normalization) is deferred past the next tile's
                # early matmuls.  Output stores go out fp16 on the GpSimd
                # queue.
                def proj_head(tt):
                    ps = stps.tile([128, S], F32,
                                   tag=("stA", "stB")[tt % 2], name=f"prj{tt}",
                                   bufs=1)
                    for o0, ow in [(0, 512), (512, 256)]:
                        for p in range(NPAIR - 1):
                            nc.tensor.matmul(
                                ps[:, o0:o0 + ow],
                                attn_q(p, tt),
                                wprojT[:, p, o0:o0 + ow],
                                start=(p == 0),
                                stop=False,
                            )
                    return ps

                def proj_tail(tt, ps):
                    for o0, ow in [(0, 512), (512, 256)]:
                        nc.tensor.matmul(
                            ps[:, o0:o0 + ow],
                            attn_q(NPAIR - 1, tt),
                            wprojT[:, NPAIR - 1, o0:o0 + ow],
                            start=False,
                            stop=True,
                        )
                    ob = outp.tile([128, DIM], FP16, tag="ob", name="ob")
                    nc.vector.tensor_copy(ob[:, 0:512], ps[:, 0:512])
                    nc.scalar.copy(out=ob[:, 512:768], in_=ps[:, 512:768])
                    nc.gpsimd.dma_start(
                        out=out_ext[tt * 128:(tt + 1) * 128, :], in_=ob[:]
                    )

                # [head0, bc_c0, head1, bc_c1, tail0, head2, tail1, ...]:
                # each norm half sits behind a head's worth of PE work so
                # the split reciprocals are ready when the PE reaches the
                # bc matmuls, and tails 0-3 only need the c0 half
                pending = None
                for tt in range(NC_T):
                    ps = proj_head(tt)
                    if tt <= 1:
                        with tc.high_priority():
                            norm5_half(tt)
                    if pending is not None:
                        proj_tail(*pending)
                    pending = (tt, ps)
                proj_tail(*pending)

    nc.finalize()
    return nc


_NC_CACHE = None


def kernel(**inputs) -> np.ndarray:
    global _NC_CACHE
    x = np.asarray(inputs["x"], dtype=np.float32)
    w_qkv = np.asarray(inputs["w_qkv"], dtype=np.float32)
    w_proj = np.asarray(inputs["w_proj"], dtype=np.float32)
    b_proj = np.asarray(inputs["b_proj"], dtype=np.float32)
    B, H, W, C = x.shape
    assert (B, H * W, C) == (8, S, DIM)

    # host-side sharding + layout prep: channel-major fp16 operands
    wqkvT = np.ascontiguousarray(w_qkv.T).astype(np.float16)       # [768, 2304]
    wprojT = np.ascontiguousarray(w_proj.T).astype(np.float16)     # [768, 768]
    xTs = [
        np.ascontiguousarray(x[b].reshape(S, DIM).T).astype(np.float16)
        for b in range(B)
    ]

    if _NC_CACHE is None:
        _NC_CACHE = build_bass()
    nc = _NC_CACHE

    in_maps = [
        {"xT": xTs[b], "w_qkvT": wqkvT, "w_projT": wprojT}
        for b in range(B)
    ]
    res = run_bass_kernel_spmd(nc, in_maps, list(range(B)))
    out = np.stack(
        [
            np.asarray(res.results[b]["out"]).astype(np.float32).reshape(H, W, C)
            for b in range(B)
        ]
    )
    return (out + b_proj.reshape(1, 1, 1, C)).astype(np.float32)


if __name__ == "__main__":
    rng = np.random.default_rng(0)
    ins = {
        "x": rng.standard_normal((8, 32, 32, DIM), dtype=np.float32),
        "w_qkv": rng.standard_normal((3 * DIM, DIM), dtype=np.float32)
        * DIM ** -0.5,
        "w_proj": rng.standard_normal((DIM, DIM), dtype=np.float32) * DIM ** -0.5,
        "b_proj": np.zeros(DIM, dtype=np.float32),
    }
    o = kernel(**ins)
    print(o.shape, o.dtype)



# revision 64
# speedup vs baseline: 1.0294x; 1.0038x over previous
"""Trainium2 Bass kernel for nn_Attention_10917806866815.

Multi-head attention forward (B=8, S=32x32=1024, C=768, 12 heads, hd=64),
data-parallel across 8 NeuronCores: core b computes batch element b.
No collectives needed.

Host side (sharding-time prep in kernel()): inputs are pre-transposed to
channel-major and cast to fp16, so the device kernel is pure matmul work:
  xT [768,1024], w_qkvT [768,2304], w_projT [768,768] -- all fp16.

Pipeline (v2.1). The Tile scheduler is dependency-driven (emission order
is only a priority tie-break), and PSUM write-after-read hazards are
tracked per *tile*, so the slot structure is built around two separate
score tiles:

  st_A [128,1024] = c0 of both heads   (q columns 0-511)
  st_B [128,1024] = c1 of both heads   (q columns 512-1023)

Per slot: paired score matmuls (partition bases 0/64 -> disjoint PE row
groups, run concurrently) fill st_A then st_B; ScalarE exps the two
halves separately (exp_A, exp_B) into one merged pt tile [128, 2048].
The next slot's c0 matmuls only WAR-wait on exp_A and c1 only on exp_B,
so the slot cadence is the ScalarE back-to-back rate (~2.3us), not the
previous serialized scores->exp_h0->exp_h1->scores loop (~2.8us).

DMA: a dma_start costs ~1.3us of sequencer issue time, so the inputs
move as 8 large multi-dim-AP transfers (split between the SP and
Activation HWDGE queues), and the mid-kernel normalization bounce plus
the output stores issue from the otherwise-idle GpSimd software-DGE
queue.  Output is stored fp16 (halves traffic; fp16 rounding is far
inside the error budget).

QKV "extras" (projection chunks), PV chunk-0 (1-slot lag), the deferred
chunk-1 bursts, and evacuations are all emitted at low scheduler
priority so a ready score matmul always pops first.

Output projection in the epilogue ping-pongs the freed score tiles
(head = pairs 0-4, tail = pair 5 after the last normalization).

Precision: fp16 operands with fp32 PSUM accumulation.
"""

import numpy as np

import concourse.bass as bass
import concourse.mybir as mybir
import concourse.tile as tile
from concourse import bacc
from concourse.bass_utils import run_bass_kernel_spmd

DIM = 768
S = 1024
NH = 12
HD = 64
SCALE = HD ** -0.5

F32 = mybir.dt.float32
FP16 = mybir.dt.float16

NC_T = S // 128          # 8 token tiles
NC_C = DIM // 128        # 6 channel tiles
NPAIR = NH // 2          # 6 head pairs
VW = HD + 1              # 65: v columns per head incl. ones column

LOWPRI = -1_000_000      # deprioritize non-score work in the ready heap


def build_bass():
    nc = bacc.Bacc(None, target_bir_lowering=False)

    xT_ext = nc.declare_dram_parameter("xT", [DIM, S], FP16, isOutput=False)
    wqkvT_ext = nc.declare_dram_parameter(
        "w_qkvT", [DIM, 3 * DIM], FP16, isOutput=False
    )
    wprojT_ext = nc.declare_dram_parameter(
        "w_projT", [DIM, DIM], FP16, isOutput=False
    )
    out_ext = nc.declare_dram_parameter("out", [S, DIM], FP16, isOutput=True)

    with tile.TileContext(nc) as tc:
        from contextlib import ExitStack

        with ExitStack() as ctx:
            consts = ctx.enter_context(tc.tile_pool(name="consts", bufs=1))
            persist = ctx.enter_context(tc.tile_pool(name="persist", bufs=1))

            # c-major operands: [:, j, :] is channel-tile j.
            xT = persist.tile([128, NC_C, S], FP16, tag="xT", name="xT")
            wqkvT = persist.tile(
                [128, NC_C, 3 * DIM], FP16, tag="wqkvT", name="wqkvT"
            )
            wprojT = persist.tile([128, NC_C, DIM], FP16, tag="wprojT", name="wprojT")

            # ---- bulk input DMA: 8 large transfers, ordered by need ----
            # srcs as [p, k, ...] views of the DRAM tensors
            x_src = xT_ext[:].rearrange("(k p) s -> p k s", k=NC_C)
            w_src = wqkvT_ext[:].rearrange(
                "(k p) (g c) -> p k g c", k=NC_C, g=3
            )
            wp_src = wprojT_ext[:].rearrange("(k p) c -> p k c", k=NC_C)
            w_dst = wqkvT[:].rearrange("p k (g c) -> p k g c", g=3)

            # x split by k-tiles across the SP and Activation HWDGE queues
            # so both column halves land ~12us; q0/k0 ride the Vector
            # queue (small, land early); q1/k1 + wproj on the GpSimd
            # SWDGE which is otherwise idle until the first norm bounce.
            # x in quarters: both c0 quarters first so the first qk chunks
            # can start ~4us earlier; c1 quarters follow on the same queues
            nc.sync.dma_start(out=xT[:, 0:3, 0:512], in_=x_src[:, 0:3, 0:512])
            nc.scalar.dma_start(
                out=xT[:, 3:6, 0:512], in_=x_src[:, 3:6, 0:512]
            )
            nc.sync.dma_start(
                out=xT[:, 0:3, 512:1024], in_=x_src[:, 0:3, 512:1024]
            )
            nc.scalar.dma_start(
                out=xT[:, 3:6, 512:1024], in_=x_src[:, 3:6, 512:1024]
            )
            nc.gpsimd.dma_start(                                 # q0
                out=w_dst[:, :, 0, 0:128], in_=w_src[:, :, 0, 0:128]
            )
            nc.gpsimd.dma_start(                                 # k0
                out=w_dst[:, :, 1, 0:128], in_=w_src[:, :, 1, 0:128]
            )
            nc.gpsimd.dma_start(                                 # q1
                out=w_dst[:, :, 0, 128:256], in_=w_src[:, :, 0, 128:256]
            )
            nc.gpsimd.dma_start(                                 # k1
                out=w_dst[:, :, 1, 128:256], in_=w_src[:, :, 1, 128:256]
            )
            # v heads 0-7 (pair-0 extras need them early)
            nc.sync.dma_start(
                out=w_dst[:, :, 2, 0:512], in_=w_src[:, :, 2, 0:512]
            )
            # q2-5, k2-5
            nc.sync.dma_start(
                out=w_dst[:, :, 0, 256:768], in_=w_src[:, :, 0, 256:768]
            )
            nc.sync.dma_start(
                out=w_dst[:, :, 1, 256:768], in_=w_src[:, :, 1, 256:768]
            )
            # v heads 8-11
            nc.scalar.dma_start(
                out=w_dst[:, :, 2, 512:768], in_=w_src[:, :, 2, 512:768]
            )
            nc.gpsimd.dma_start(out=wprojT[:], in_=wp_src[:])

            qkT = [
                persist.tile([128, S], FP16, tag=f"qkT{ot}", name=f"qkT{ot}")
                for ot in range(2 * NPAIR)
            ]
            # v_ext rows padded to NH*VW+63 so every per-head stationary
            # slice can be 128 columns wide (NumWeights==128 -> the LDW
            # uses fast-weight-load and hides behind in-flight matmuls);
            # PV out rows 65-127 are garbage and never read.
            v_ext = [
                persist.tile([128, NH * VW + 63], FP16, tag=f"vext{tt}",
                             name=f"vext{tt}")
                for tt in range(NC_T)
            ]
            # attnT as column-half tiles: projection q-tiles 0-3 only
            # depend on the c0 half, so pair-5's epilogue normalize can
            # release them early
            attnTa = [
                persist.tile([128, 512], FP16, tag=f"attnTa{p}",
                             name=f"attnTa{p}")
                for p in range(NPAIR)
            ]
            attnTb = [
                persist.tile([128, 512], FP16, tag=f"attnTb{p}",
                             name=f"attnTb{p}")
                for p in range(NPAIR)
            ]

            def attn_q(p, tt):
                # [128, 128] slice of pair p's attnT at q-tile tt
                return (attnTa, attnTb)[tt // 4][p][
                    :, (tt % 4) * 128:(tt % 4 + 1) * 128
                ]
            ones64 = consts.tile([1, 64], FP16, tag="ones64", name="ones64")
            nc.vector.memset(ones64[:], 1.0)
            for tt in range(NC_T):
                nc.gpsimd.memset(v_ext[tt][:], 1.0)

            with (
                tc.tile_pool(name="stps", bufs=1, space="PSUM") as stps,
                tc.tile_pool(name="pvps", bufs=1, space="PSUM") as pvps,
                tc.tile_pool(name="bgps", bufs=1, space="PSUM") as bgps,
                tc.tile_pool(name="ptpool", bufs=1) as ptpool,
                tc.tile_pool(name="normp", bufs=2) as normp,
                tc.tile_pool(name="outp", bufs=3) as outp,
                tc.tile_pool(name="rdram", bufs=2, space="DRAM") as rdram,
            ):
                # 2 shared background PSUM banks: QKV-projection extras,
                # chunk-1 PV bursts, warm-up, norm broadcasts.  Each
                # logical use occupies its tag contiguously in emission
                # order.
                bg_flip = [0]

                def bg_tile(name, shape=(128, 512)):
                    t = bgps.tile(list(shape), F32, tag=f"bg{bg_flip[0]}",
                                  name=name, bufs=1)
                    bg_flip[0] ^= 1
                    return t

                # ---- QKV building blocks ----
                def emit_qk_chunk(ot, c):
                    ps = bg_tile("qkvp")
                    for k in range(NC_C):
                        nc.tensor.matmul(
                            ps[:],
                            wqkvT[:, k, ot * 128:(ot + 1) * 128],
                            xT[:, k, c * 512:(c + 1) * 512],
                            start=(k == 0),
                            stop=(k == NC_C - 1),
                        )
                    nc.vector.tensor_copy(qkT[ot][:, c * 512:(c + 1) * 512], ps[:])

                def emit_v_chunk(tt, c):
                    o0, ow, h0, nh = [
                        (2 * DIM, 512, 0, 8), (2 * DIM + 512, 256, 8, 4)
                    ][c]
                    ps = bg_tile("vp")
                    for k in range(NC_C):
                        nc.tensor.matmul(
                            ps[:, :ow],
                            xT[:, k, tt * 128:(tt + 1) * 128],
                            wqkvT[:, k, o0:o0 + ow],
                            start=(k == 0),
                            stop=(k == NC_C - 1),
                        )
                    dst = (
                        v_ext[tt][:, 0:NH * VW]
                        .rearrange("p (h e) -> p h e", e=VW)[:, h0:h0 + nh, 0:HD]
                    )
                    nc.vector.tensor_copy(
                        dst, ps[:, :ow].rearrange("p (h e) -> p h e", e=HD)
                    )

                # extras[p][T]: QKV work dependencies only require:
                #   v chunk-0 tile T ready before pair-0 PV consumes it at
                #   slot T+1; pair p+1's q/k ready before pair p+1.
                # The dep-driven scheduler fills PE idle time with these
                # (they run at low priority).
                extras = [[[] for _ in range(NC_T)] for _ in range(NPAIR)]

                def TH(f, *a):
                    return lambda: f(*a)

                for tt in range(NC_T):
                    extras[0][tt].append(TH(emit_v_chunk, tt, 0))
                for p in range(1, NPAIR - 1):
                    extras[p][2].append(TH(emit_qk_chunk, p + 1, 0))
                    extras[p][3].append(TH(emit_qk_chunk, NPAIR + p + 1, 0))
                    extras[p][5].append(TH(emit_qk_chunk, p + 1, 1))
                    extras[p][6].append(TH(emit_qk_chunk, NPAIR + p + 1, 1))
                for i in range(NC_T):  # v chunk-1 (needed by pair 4's PV)
                    extras[1 + i // 3][[1, 4, 7][i % 3]].append(
                        TH(emit_v_chunk, i, 1)
                    )

                # ---- HAM warm-up: keep the PE busy through the DMA
                # lead-in so the first real matmuls run at full clock ----
                # 28 matmuls (~7-8us): long enough to cover the input-DMA
                # wait so the PE never sees a >3.4us idle window (which
                # would re-throttle HAM and run the prologue at 1.2 GHz)
                wu = consts.tile([128, 512], FP16, tag="wu", name="wu")
                nc.vector.memset(wu[:], 0.0)
                wups = bg_tile("wups")
                NWU = 14
                for i in range(NWU):
                    nc.tensor.matmul(
                        wups[:], wu[:, 0:128], wu[:],
                        start=(i == 0), stop=(i == NWU - 1),
                    )

                # ---- prologue: q/k for pairs 0 and 1 up front (low
                # priority so pair-0 score matmuls preempt as soon as
                # their chunks land) ----
                with tc.high_priority(offset=LOWPRI):
                    emit_qk_chunk(0, 0)
                    emit_qk_chunk(NPAIR, 0)
                    emit_qk_chunk(1, 0)
                    emit_qk_chunk(NPAIR + 1, 0)
                    emit_qk_chunk(0, 1)
                    emit_qk_chunk(NPAIR, 1)
                    emit_qk_chunk(1, 1)
                    emit_qk_chunk(NPAIR + 1, 1)

                # ---- attention: software-pipelined slot stream ----
                pts_of = {}     # (p, T) -> pt tile [128, 2048] h-major
                pv0_of = {}     # p -> [pv0_h0, pv0_h1]  (chunk-0 accums)
                pv1_of = {}     # last pair only: incremental chunk-1 accums
                sums_sb_of = {}

                def sc_mm(st, p, T, c):
                    kT_t = qkT[NPAIR + p]
                    qT_t = qkT[p]
                    for h in range(2):
                        r0 = h * 64
                        nc.tensor.matmul(
                            st[:, h * 512:(h + 1) * 512],
                            kT_t[r0:r0 + 64, T * 128:(T + 1) * 128],
                            qT_t[r0:r0 + 64, c * 512:(c + 1) * 512],
                            start=True,
                            stop=True,
                        )

                def slot(p, T):
                    if T == 0:
                        sums_sb_of[p] = normp.tile(
                            [1, 2 * S], F32, tag="sums", name="sums", bufs=2
                        )
                        pv0_of[p] = [
                            pvps.tile([128, 512], F32, tag=f"pva{h}",
                                      name=f"pva{h}", bufs=1)
                            for h in range(2)
                        ]
                        if p == NPAIR - 1:
                            # last pair: chunk-1 accumulates incrementally
                            # in the (now extras-free) background banks so
                            # the epilogue isn't serialized behind a burst;
                            # sums go to per-half tiles so each half's
                            # reciprocal fires as soon as its rows land
                            pv1_of[p] = [bg_tile(f"pvL{h}") for h in range(2)]
                            sums5[0] = normp.tile([1, 2, 512], F32,
                                                  tag="s5c0", name="s5c0",
                                                  bufs=1)
                            sums5[1] = normp.tile([1, 2, 512], F32,
                                                  tag="s5c1", name="s5c1",
                                                  bufs=1)
                    st_a = stps.tile([128, S], F32, tag="stA", name="stA",
                                     bufs=1)
                    st_b = stps.tile([128, S], F32, tag="stB", name="stB",
                                     bufs=1)
                    # per-half pt tiles: chunk-0 consumers only RAW-wait on
                    # exp_A, chunk-1 only on exp_B
                    ptA = ptpool.tile([128, S], FP16, tag=f"ptA{T}",
                                      name=f"ptA{T}", bufs=2)
                    ptB = ptpool.tile([128, S], FP16, tag=f"ptB{T}",
                                      name=f"ptB{T}", bufs=2)
                    pts_of[(p, T)] = (ptA, ptB)

                    sc_mm(st_a, p, T, 0)
                    nc.scalar.activation(
                        out=ptA[:].rearrange("p (h q) -> p h q", h=2),
                        in_=st_a[:].rearrange("p (h q) -> p h q", h=2),
                        func=mybir.ActivationFunctionType.Exp,
                        scale=float(SCALE),
                    )
                    # the very last slot's PV feeds the epilogue critical
                    # path: normal priority so the finish chain isn't
                    # stuck behind projection-head filler
                    last_slot = p == NPAIR - 1 and T == NC_T - 1
                    with tc.high_priority(offset=0 if last_slot else LOWPRI):
                        if T > 0:
                            for h in range(2):
                                nc.tensor.matmul(
                                    pv0_of[p][h][:],
                                    v_ext[T - 1][
                                        :, (2 * p + h) * VW:(2 * p + h) * VW + 128
                                    ],
                                    pts_of[(p, T - 1)][0][:, h * 512:(h + 1) * 512],
                                    start=(T == 1),
                                    stop=(T == NC_T - 1),
                                )
                        if p == NPAIR - 1 and T > 0:
                            for h in range(2):
                                nc.tensor.matmul(
                                    pv1_of[p][h][:],
                                    v_ext[T - 1][
                                        :, (2 * p + h) * VW:(2 * p + h) * VW + 128
                                    ],
                                    pts_of[(p, T - 1)][1][:, h * 512:(h + 1) * 512],
                                    start=(T == 1),
                                    stop=(T == NC_T - 1),
                                )
                        for th in extras[p][T]:
                            th()
                    sc_mm(st_b, p, T, 1)
                    nc.scalar.activation(
                        out=ptB[:].rearrange("p (h q) -> p h q", h=2),
                        in_=st_b[:].rearrange("p (h q) -> p h q", h=2),
                        func=mybir.ActivationFunctionType.Exp,
                        scale=float(SCALE),
                    )

                def scalar_recip(dst, src):
                    nc.scalar.add_instruction(
                        mybir.InstActivation(
                            name=nc.get_next_instruction_name(),
                            ins=[
                                nc.scalar.lower_ap(src),
                                mybir.ImmediateValue(
                                    dtype=mybir.dt.float32, value=0.0
                                ),
                                mybir.ImmediateValue(
                                    dtype=mybir.dt.float32, value=1.0
                                ),
                                mybir.ImmediateValue(
                                    dtype=mybir.dt.float32, value=0.0
                                ),
                            ],
                            outs=[nc.scalar.lower_ap(dst)],
                            func=mybir.ActivationFunctionType.Reciprocal,
                        )
                    )

                recip5 = {}
                sums5 = {}

                def finish_c0(p):
                    for h in range(2):
                        nc.tensor.matmul(
                            pv0_of[p][h][:],
                            v_ext[NC_T - 1][
                                :, (2 * p + h) * VW:(2 * p + h) * VW + 128
                            ],
                            pts_of[(p, NC_T - 1)][0][:, h * 512:(h + 1) * 512],
                            start=False,
                            stop=True,
                        )
                    for h in range(2):
                        nc.vector.tensor_copy(
                            sums5[0][0:1, h, :] if p == NPAIR - 1
                            else sums_sb_of[p][0:1, h * S:h * S + 512],
                            pv0_of[p][h][HD:HD + 1, :],
                        )
                    if p == NPAIR - 1:
                        # preload the reciprocal ACT table set (the real
                        # reciprocals would otherwise pay the ~2.7us table
                        # switch on the critical tail), then the c0-half
                        # reciprocal as soon as its sums rows land
                        scalar_recip(
                            normp.tile([1, 1], F32, tag="rscr", name="rscr",
                                       bufs=1)[:],
                            ones64[0:1, 0:1],
                        )
                        recip5[0] = normp.tile([1, 2, 512], FP16, tag="rc0",
                                               name="rc0", bufs=1)
                        scalar_recip(recip5[0][:], sums5[0][:])
                    for h in range(2):
                        nc.vector.tensor_copy(
                            attnTa[p][h * 64:(h + 1) * 64, :],
                            pv0_of[p][h][0:HD, :],
                        )
                    del pv0_of[p]

                def finish_c1_last(p):
                    for h in range(2):
                        nc.tensor.matmul(
                            pv1_of[p][h][:],
                            v_ext[NC_T - 1][
                                :, (2 * p + h) * VW:(2 * p + h) * VW + 128
                            ],
                            pts_of[(p, NC_T - 1)][1][:, h * 512:(h + 1) * 512],
                            start=False,
                            stop=True,
                        )
                    for h in range(2):
                        nc.vector.tensor_copy(
                            sums5[1][0:1, h, :],
                            pv1_of[p][h][HD:HD + 1, :],
                        )
                    recip5[1] = normp.tile([1, 2, 512], FP16, tag="rc1",
                                           name="rc1", bufs=1)
                    scalar_recip(recip5[1][:], sums5[1][:])
                    # attnTb copies are deferred to norm5_half(1) so the
                    # c0-half normalize multiplies run first on the DVE
                    for Tq in range(NC_T):
                        del pts_of[(p, Tq)]

                def burst_c1(p):
                    pv1 = [bg_tile(f"pvb{h}") for h in range(2)]
                    for Tq in range(NC_T):
                        for h in range(2):
                            nc.tensor.matmul(
                                pv1[h][:],
                                v_ext[Tq][
                                    :, (2 * p + h) * VW:(2 * p + h) * VW + 128
                                ],
                                pts_of[(p, Tq)][1][:, h * 512:(h + 1) * 512],
                                start=(Tq == 0),
                                stop=(Tq == NC_T - 1),
                            )
                    for h in range(2):
                        nc.vector.tensor_copy(
                            sums_sb_of[p][0:1, h * S + 512:h * S + 1024],
                            pv1[h][HD:HD + 1, :],
                        )
                        nc.vector.tensor_copy(
                            attnTb[p][h * 64:(h + 1) * 64, :],
                            pv1[h][0:HD, :],
                        )
                    for Tq in range(NC_T):
                        del pts_of[(p, Tq)]
                    norm(p)

                def norm(p):
                    # reciprocal of the 2048 sums: repartition [1,2048] ->
                    # [128,16] via a DRAM bounce (issued on the idle GpSimd
                    # SWDGE queue) so the 8-cycle/element DVE divide runs
                    # on 128 lanes (pairs 0-4; the last pair is handled by
                    # norm5_half on the epilogue path)
                    sums_sb = sums_sb_of[p]
                    rd = rdram.tile([1, 2 * S], F32, tag="rd", name="rd")
                    sd = rdram.tile([1, 2 * S], F32, tag="sd", name="sd")
                    nc.gpsimd.dma_start(out=sd[:], in_=sums_sb[:])
                    sr = normp.tile([128, 16], F32, tag="sr", name="sr")
                    nc.gpsimd.dma_start(
                        out=sr[:],
                        in_=bass.AP(
                            tensor=sd.tensor,
                            offset=sd.offset,
                            ap=[[16, 128], [1, 16]],
                        ),
                    )
                    rr = normp.tile([128, 16], F32, tag="rr", name="rr")
                    nc.vector.reciprocal(rr[:], sr[:])
                    nc.gpsimd.dma_start(
                        out=bass.AP(
                            tensor=rd.tensor,
                            offset=rd.offset,
                            ap=[[16, 128], [1, 16]],
                        ),
                        in_=rr[:],
                    )
                    rb = normp.tile([128, S], F32, tag="rb", name="rb")
                    for h in range(2):
                        row = rd[0:1, h * S:(h + 1) * S]
                        row_bc = bass.AP(
                            tensor=row.tensor,
                            offset=row.offset,
                            ap=[[0, 64]] + list(row.ap[1:]),
                        )
                        nc.gpsimd.dma_start(
                            out=rb[h * 64:(h + 1) * 64, :], in_=row_bc
                        )
                    rb_r = rb[:].rearrange("d (c q) -> d c q", q=512)
                    nc.vector.tensor_mul(
                        attnTa[p][:], attnTa[p][:], rb_r[:, 0, :]
                    )
                    nc.vector.tensor_mul(
                        attnTb[p][:], attnTb[p][:], rb_r[:, 1, :]
                    )

                def norm5_half(c):
                    # last pair, one column half: broadcast 1/sums via two
                    # matmuls into a freed pva bank (bg banks still hold
                    # the unread chunk-1 accumulators), then normalize
                    p = NPAIR - 1
                    at = (attnTa, attnTb)[c][p]
                    bc = pvps.tile([128, 512], F32, tag=f"pva{c}",
                                   name=f"bc{c}", bufs=1)
                    for h in range(2):
                        nc.tensor.matmul(
                            bc[h * 64:(h + 1) * 64, :],
                            ones64[0:1, :],
                            recip5[c][0:1, h, :],
                            start=True,
                            stop=True,
                        )
                    if c == 1:
                        for h in range(2):
                            nc.vector.tensor_copy(
                                at[h * 64:(h + 1) * 64, :],
                                pv1_of[p][h][0:HD, :],
                            )
                        del pv1_of[p]
                    for h in range(2):
                        nc.vector.tensor_mul(
                            at[h * 64:(h + 1) * 64, :],
                            at[h * 64:(h + 1) * 64, :],
                            bc[h * 64:(h + 1) * 64, :],
                        )

                # emission order: chunk-1 burst of pair p-1 deferred past
                # the next pair's first two slots (low priority keeps it
                # out of the scores' way).  Pair 4's burst is un-deferred
                # (the bg banks belong to pair 5's incremental chunk-1
                # during pair 5), and pair 5 finishes both chunks inline.
                for p in range(NPAIR):
                    slot(p, 0)
                    slot(p, 1)
                    if 0 < p < NPAIR - 1:
                        with tc.high_priority(offset=LOWPRI):
                            burst_c1(p - 1)
                    for T in range(2, NC_T):
                        slot(p, T)
                    if p == NPAIR - 1:
                        # the last pair's finish/normalize chain is the
                        # epilogue critical path: absolute top priority so
                        # it always outranks the projection-head filler
                        with tc.high_priority():
                            finish_c0(p)
                            finish_c1_last(p)
                    else:
                        with tc.high_priority(offset=LOWPRI):
                            finish_c0(p)
                            if p == NPAIR - 2:
                                burst_c1(p)
                # pair-5 norm is emitted between the first two projection
                # heads: the PE instruction stream is static, so the bc
                # matmuls must sit AFTER ~4us of head matmuls to cover the
                # ScalarE reciprocal (+table load) latency without a stall

                # ---------------- output projection ----------------
                # PSUM ping-pongs the freed score tiles (tags stA/stB).
                # Depth-2 pipeline: each tile's pair-5 matmul (gated by the
                # last normalization) is deferred past the next tile's
                # early matmuls.  Output stores go out fp16 on the GpSimd
                # queue.
                def proj_head(tt):
                    ps = stps.tile([128, S], F32,
                                   tag=("stA", "stB")[tt % 2], name=f"prj{tt}",
                                   bufs=1)
                    for o0, ow in [(0, 512), (512, 256)]:
                        for p in range(NPAIR - 1):
                            nc.tensor.matmul(
                                ps[:, o0:o0 + ow],
                                attn_q(p, tt),
                                wprojT[:, p, o0:o0 + ow],
                                start=(p == 0),
                                stop=False,
                            )
                    return ps

                def proj_tail(tt, ps):
                    for o0, ow in [(0, 512), (512, 256)]:
                        nc.tensor.matmul(
                            ps[:, o0:o0 + ow],
                            attn_q(NPAIR - 1, tt),
                            wprojT[:, NPAIR - 1, o0:o0 + ow],
                            start=False,
                            stop=True,
                        )
                    ob = outp.tile([128, DIM], FP16, tag="ob", name="ob")
                    nc.vector.tensor_copy(ob[:, 0:512], ps[:, 0:512])
                    nc.scalar.copy(out=ob[:, 512:768], in_=ps[:, 512:768])
                    nc.gpsimd.dma_start(
                        out=out_ext[tt * 128:(tt + 1) * 128, :], in_=ob[:]
                    )

                # [head0, bc_c0, head1, bc_c1, tail0, head2, tail1, ...]:
                # each norm half sits behind a head's worth of PE work so
                # the split reciprocals are ready when the PE reaches the
                # bc matmuls, and tails 0-3 only need the c0 half
                pending = None
                for tt in range(NC_T):
                    ps = proj_head(tt)
                    if tt <= 1:
                        with tc.high_priority():
                            norm5_half(tt)
                    if pending is not None:
                        proj_tail(*pending)
                    pending = (tt, ps)
                proj_tail(*pending)

    nc.finalize()
    return nc


_NC_CACHE = None


def kernel(**inputs) -> np.ndarray:
    global _NC_CACHE
    x = np.asarray(inputs["x"], dtype=np.float32)
    w_qkv = np.asarray(inputs["w_qkv"], dtype=np.float32)
    w_proj = np.asarray(inputs["w_proj"], dtype=np.float32)
    b_proj = np.asarray(inputs["b_proj"], dtype=np.float32)
    B, H, W, C = x.shape
    assert (B, H * W, C) == (8, S, DIM)

    # host-side sharding + layout prep: channel-major fp16 operands
    wqkvT = np.ascontiguousarray(w_qkv.T).astype(np.float16)       # [768, 2304]
    wprojT = np.ascontiguousarray(w_proj.T).astype(np.float16)     # [768, 768]
    xTs = [
        np.ascontiguousarray(x[b].reshape(S, DIM).T).astype(np.float16)
        for b in range(B)
    ]

    if _NC_CACHE is None:
        _NC_CACHE = build_bass()
    nc = _NC_CACHE

    in_maps = [
        {"xT": xTs[b], "w_qkvT": wqkvT, "w_projT": wprojT}
        for b in range(B)
    ]
    res = run_bass_kernel_spmd(nc, in_maps, list(range(B)))
    out = np.stack(
        [
            np.asarray(res.results[b]["out"]).astype(np.float32).reshape(H, W, C)
            for b in range(B)
        ]
    )
    return (out + b_proj.reshape(1, 1, 1, C)).astype(np.float32)


if __name__ == "__main__":
    rng = np.random.default_rng(0)
    ins = {
        "x": rng.standard_normal((8, 32, 32, DIM), dtype=np.float32),
        "w_qkv": rng.standard_normal((3 * DIM, DIM), dtype=np.float32)
        * DIM ** -0.5,
        "w_proj": rng.standard_normal((DIM, DIM), dtype=np.float32) * DIM ** -0.5,
        "b_proj": np.zeros(DIM, dtype=np.float32),
    }
    o = kernel(**ins)
    print(o.shape, o.dtype)



# revision 70
# speedup vs baseline: 1.0324x; 1.0029x over previous
"""Trainium2 Bass kernel for nn_Attention_10917806866815.

Multi-head attention forward (B=8, S=32x32=1024, C=768, 12 heads, hd=64),
data-parallel across 8 NeuronCores: core b computes batch element b.
No collectives needed.

Host side (sharding-time prep in kernel()): inputs are pre-transposed to
channel-major and cast to fp16, so the device kernel is pure matmul work:
  xT [768,1024], w_qkvT [768,2304], w_projT [768,768] -- all fp16.

Pipeline (v2.1). The Tile scheduler is dependency-driven (emission order
is only a priority tie-break), and PSUM write-after-read hazards are
tracked per *tile*, so the slot structure is built around two separate
score tiles:

  st_A [128,1024] = c0 of both heads   (q columns 0-511)
  st_B [128,1024] = c1 of both heads   (q columns 512-1023)

Per slot: paired score matmuls (partition bases 0/64 -> disjoint PE row
groups, run concurrently) fill st_A then st_B; ScalarE exps the two
halves separately (exp_A, exp_B) into one merged pt tile [128, 2048].
The next slot's c0 matmuls only WAR-wait on exp_A and c1 only on exp_B,
so the slot cadence is the ScalarE back-to-back rate (~2.3us), not the
previous serialized scores->exp_h0->exp_h1->scores loop (~2.8us).

DMA: a dma_start costs ~1.3us of sequencer issue time, so the inputs
move as 8 large multi-dim-AP transfers (split between the SP and
Activation HWDGE queues), and the mid-kernel normalization bounce plus
the output stores issue from the otherwise-idle GpSimd software-DGE
queue.  Output is stored fp16 (halves traffic; fp16 rounding is far
inside the error budget).

QKV "extras" (projection chunks), PV chunk-0 (1-slot lag), the deferred
chunk-1 bursts, and evacuations are all emitted at low scheduler
priority so a ready score matmul always pops first.

Output projection in the epilogue ping-pongs the freed score tiles
(head = pairs 0-4, tail = pair 5 after the last normalization).

Precision: fp16 operands with fp32 PSUM accumulation.
"""

import numpy as np

import concourse.bass as bass
import concourse.mybir as mybir
import concourse.tile as tile
from concourse import bacc
from concourse.bass_utils import run_bass_kernel_spmd

DIM = 768
S = 1024
NH = 12
HD = 64
SCALE = HD ** -0.5

F32 = mybir.dt.float32
FP16 = mybir.dt.float16

NC_T = S // 128          # 8 token tiles
NC_C = DIM // 128        # 6 channel tiles
NPAIR = NH // 2          # 6 head pairs
VW = HD + 1              # 65: v columns per head incl. ones column

LOWPRI = -1_000_000      # deprioritize non-score work in the ready heap


def build_bass():
    nc = bacc.Bacc(None, target_bir_lowering=False)

    xT_ext = nc.declare_dram_parameter("xT", [DIM, S], FP16, isOutput=False)
    wqkvT_ext = nc.declare_dram_parameter(
        "w_qkvT", [DIM, 3 * DIM], FP16, isOutput=False
    )
    wprojT_ext = nc.declare_dram_parameter(
        "w_projT", [DIM, DIM], FP16, isOutput=False
    )
    out_ext = nc.declare_dram_parameter("out", [S, DIM], FP16, isOutput=True)

    with tile.TileContext(nc) as tc:
        from contextlib import ExitStack

        with ExitStack() as ctx:
            consts = ctx.enter_context(tc.tile_pool(name="consts", bufs=1))
            persist = ctx.enter_context(tc.tile_pool(name="persist", bufs=1))

            # c-major operands: [:, j, :] is channel-tile j.
            xT = persist.tile([128, NC_C, S], FP16, tag="xT", name="xT")
            wqkvT = persist.tile(
                [128, NC_C, 3 * DIM], FP16, tag="wqkvT", name="wqkvT"
            )
            wprojT = persist.tile([128, NC_C, DIM], FP16, tag="wprojT", name="wprojT")

            # ---- bulk input DMA: 8 large transfers, ordered by need ----
            # srcs as [p, k, ...] views of the DRAM tensors
            x_src = xT_ext[:].rearrange("(k p) s -> p k s", k=NC_C)
            w_src = wqkvT_ext[:].rearrange(
                "(k p) (g c) -> p k g c", k=NC_C, g=3
            )
            wp_src = wprojT_ext[:].rearrange("(k p) c -> p k c", k=NC_C)
            w_dst = wqkvT[:].rearrange("p k (g c) -> p k g c", g=3)

            # x split by k-tiles across the SP and Activation HWDGE queues
            # so both column halves land ~12us; q0/k0 ride the Vector
            # queue (small, land early); q1/k1 + wproj on the GpSimd
            # SWDGE which is otherwise idle until the first norm bounce.
            # x in quarters: both c0 quarters first so the first qk chunks
            # can start ~4us earlier; c1 quarters follow on the same queues
            nc.sync.dma_start(out=xT[:, 0:3, 0:512], in_=x_src[:, 0:3, 0:512])
            nc.scalar.dma_start(
                out=xT[:, 3:6, 0:512], in_=x_src[:, 3:6, 0:512]
            )
            nc.sync.dma_start(
                out=xT[:, 0:3, 512:1024], in_=x_src[:, 0:3, 512:1024]
            )
            nc.scalar.dma_start(
                out=xT[:, 3:6, 512:1024], in_=x_src[:, 3:6, 512:1024]
            )
            nc.gpsimd.dma_start(                                 # q0
                out=w_dst[:, :, 0, 0:128], in_=w_src[:, :, 0, 0:128]
            )
            nc.gpsimd.dma_start(                                 # k0
                out=w_dst[:, :, 1, 0:128], in_=w_src[:, :, 1, 0:128]
            )
            nc.gpsimd.dma_start(                                 # q1
                out=w_dst[:, :, 0, 128:256], in_=w_src[:, :, 0, 128:256]
            )
            nc.gpsimd.dma_start(                                 # k1
                out=w_dst[:, :, 1, 128:256], in_=w_src[:, :, 1, 128:256]
            )
            # v heads 0-7 (pair-0 extras need them early)
            nc.sync.dma_start(
                out=w_dst[:, :, 2, 0:512], in_=w_src[:, :, 2, 0:512]
            )
            # q2-5, k2-5
            nc.sync.dma_start(
                out=w_dst[:, :, 0, 256:768], in_=w_src[:, :, 0, 256:768]
            )
            nc.sync.dma_start(
                out=w_dst[:, :, 1, 256:768], in_=w_src[:, :, 1, 256:768]
            )
            # v heads 8-11
            nc.scalar.dma_start(
                out=w_dst[:, :, 2, 512:768], in_=w_src[:, :, 2, 512:768]
            )
            nc.gpsimd.dma_start(out=wprojT[:], in_=wp_src[:])

            qkT = [
                persist.tile([128, S], FP16, tag=f"qkT{ot}", name=f"qkT{ot}")
                for ot in range(2 * NPAIR)
            ]
            # v_ext rows padded to NH*VW+63 so every per-head stationary
            # slice can be 128 columns wide (NumWeights==128 -> the LDW
            # uses fast-weight-load and hides behind in-flight matmuls);
            # PV out rows 65-127 are garbage and never read.
            v_ext = [
                persist.tile([128, NH * VW + 63], FP16, tag=f"vext{tt}",
                             name=f"vext{tt}")
                for tt in range(NC_T)
            ]
            # attnT as column-half tiles: projection q-tiles 0-3 only
            # depend on the c0 half, so pair-5's epilogue normalize can
            # release them early
            attnTa = [
                persist.tile([128, 512], FP16, tag=f"attnTa{p}",
                             name=f"attnTa{p}")
                for p in range(NPAIR)
            ]
            attnTb = [
                persist.tile([128, 512], FP16, tag=f"attnTb{p}",
                             name=f"attnTb{p}")
                for p in range(NPAIR)
            ]

            def attn_q(p, tt):
                # [128, 128] slice of pair p's attnT at q-tile tt
                return (attnTa, attnTb)[tt // 4][p][
                    :, (tt % 4) * 128:(tt % 4 + 1) * 128
                ]
            ones64 = consts.tile([1, 64], FP16, tag="ones64", name="ones64")
            nc.vector.memset(ones64[:], 1.0)
            for tt in range(NC_T):
                nc.gpsimd.memset(v_ext[tt][:], 1.0)

            with (
                tc.tile_pool(name="stps", bufs=1, space="PSUM") as stps,
                tc.tile_pool(name="pvps", bufs=1, space="PSUM") as pvps,
                tc.tile_pool(name="bgps", bufs=1, space="PSUM") as bgps,
                tc.tile_pool(name="ptpool", bufs=1) as ptpool,
                tc.tile_pool(name="normp", bufs=2) as normp,
                tc.tile_pool(name="outp", bufs=3) as outp,
                tc.tile_pool(name="rdram", bufs=2, space="DRAM") as rdram,
            ):
                # 2 shared background PSUM banks: QKV-projection extras,
                # chunk-1 PV bursts, warm-up, norm broadcasts.  Each
                # logical use occupies its tag contiguously in emission
                # order.
                bg_flip = [0]

                def bg_tile(name, shape=(128, 512)):
                    t = bgps.tile(list(shape), F32, tag=f"bg{bg_flip[0]}",
                                  name=name, bufs=1)
                    bg_flip[0] ^= 1
                    return t

                # ---- QKV building blocks ----
                def emit_qk_chunk(ot, c):
                    ps = bg_tile("qkvp")
                    for k in range(NC_C):
                        nc.tensor.matmul(
                            ps[:],
                            wqkvT[:, k, ot * 128:(ot + 1) * 128],
                            xT[:, k, c * 512:(c + 1) * 512],
                            start=(k == 0),
                            stop=(k == NC_C - 1),
                        )
                    nc.vector.tensor_copy(qkT[ot][:, c * 512:(c + 1) * 512], ps[:])

                def emit_v_chunk(tt, c):
                    o0, ow, h0, nh = [
                        (2 * DIM, 512, 0, 8), (2 * DIM + 512, 256, 8, 4)
                    ][c]
                    ps = bg_tile("vp")
                    for k in range(NC_C):
                        nc.tensor.matmul(
                            ps[:, :ow],
                            xT[:, k, tt * 128:(tt + 1) * 128],
                            wqkvT[:, k, o0:o0 + ow],
                            start=(k == 0),
                            stop=(k == NC_C - 1),
                        )
                    dst = (
                        v_ext[tt][:, 0:NH * VW]
                        .rearrange("p (h e) -> p h e", e=VW)[:, h0:h0 + nh, 0:HD]
                    )
                    nc.vector.tensor_copy(
                        dst, ps[:, :ow].rearrange("p (h e) -> p h e", e=HD)
                    )

                # extras[p][T]: QKV work dependencies only require:
                #   v chunk-0 tile T ready before pair-0 PV consumes it at
                #   slot T+1; pair p+1's q/k ready before pair p+1.
                # The dep-driven scheduler fills PE idle time with these
                # (they run at low priority).
                extras = [[[] for _ in range(NC_T)] for _ in range(NPAIR)]

                def TH(f, *a):
                    return lambda: f(*a)

                for tt in range(NC_T):
                    extras[0][tt].append(TH(emit_v_chunk, tt, 0))
                for p in range(1, NPAIR - 1):
                    extras[p][2].append(TH(emit_qk_chunk, p + 1, 0))
                    extras[p][3].append(TH(emit_qk_chunk, NPAIR + p + 1, 0))
                    extras[p][5].append(TH(emit_qk_chunk, p + 1, 1))
                    extras[p][6].append(TH(emit_qk_chunk, NPAIR + p + 1, 1))
                for i in range(NC_T):  # v chunk-1 (needed by pair 4's PV)
                    extras[1 + i // 3][[1, 4, 7][i % 3]].append(
                        TH(emit_v_chunk, i, 1)
                    )

                # ---- HAM warm-up: keep the PE busy through the DMA
                # lead-in so the first real matmuls run at full clock ----
                # 28 matmuls (~7-8us): long enough to cover the input-DMA
                # wait so the PE never sees a >3.4us idle window (which
                # would re-throttle HAM and run the prologue at 1.2 GHz)
                wu = consts.tile([128, 512], FP16, tag="wu", name="wu")
                nc.vector.memset(wu[:], 0.0)
                wups = bg_tile("wups")
                NWU = 16
                for i in range(NWU):
                    nc.tensor.matmul(
                        wups[:], wu[:, 0:128], wu[:],
                        start=(i == 0), stop=(i == NWU - 1),
                    )

                # ---- prologue: q/k for pairs 0 and 1 up front (low
                # priority so pair-0 score matmuls preempt as soon as
                # their chunks land) ----
                with tc.high_priority(offset=LOWPRI):
                    emit_qk_chunk(0, 0)
                    emit_qk_chunk(NPAIR, 0)
                    emit_qk_chunk(1, 0)
                    emit_qk_chunk(NPAIR + 1, 0)
                    emit_qk_chunk(0, 1)
                    emit_qk_chunk(NPAIR, 1)
                    emit_qk_chunk(1, 1)
                    emit_qk_chunk(NPAIR + 1, 1)

                # ---- attention: software-pipelined slot stream ----
                pts_of = {}     # (p, T) -> pt tile [128, 2048] h-major
                pv0_of = {}     # p -> [pv0_h0, pv0_h1]  (chunk-0 accums)
                pv1_of = {}     # last pair only: incremental chunk-1 accums
                sums_sb_of = {}

                def sc_mm(st, p, T, c):
                    kT_t = qkT[NPAIR + p]
                    qT_t = qkT[p]
                    for h in range(2):
                        r0 = h * 64
                        nc.tensor.matmul(
                            st[:, h * 512:(h + 1) * 512],
                            kT_t[r0:r0 + 64, T * 128:(T + 1) * 128],
                            qT_t[r0:r0 + 64, c * 512:(c + 1) * 512],
                            start=True,
                            stop=True,
                        )

                def slot(p, T):
                    if T == 0:
                        if p < NPAIR - 1:
                            sums_sb_of[p] = normp.tile(
                                [1, 2 * S], F32, tag="sums", name="sums",
                                bufs=2
                            )
                        pv0_of[p] = [
                            pvps.tile([128, 512], F32, tag=f"pva{h}",
                                      name=f"pva{h}", bufs=1)
                            for h in range(2)
                        ]
                        if p == NPAIR - 1:
                            # last pair: chunk-1 accumulates incrementally
                            # in the (now extras-free) background banks so
                            # the epilogue isn't serialized behind a burst;
                            # sums go to per-half tiles so each half's
                            # reciprocal fires as soon as its rows land
                            pv1_of[p] = [bg_tile(f"pvL{h}") for h in range(2)]
                    st_a = stps.tile([128, S], F32, tag="stA", name="stA",
                                     bufs=1)
                    st_b = stps.tile([128, S], F32, tag="stB", name="stB",
                                     bufs=1)
                    # per-half pt tiles: chunk-0 consumers only RAW-wait on
                    # exp_A, chunk-1 only on exp_B
                    ptA = ptpool.tile([128, S], FP16, tag=f"ptA{T}",
                                      name=f"ptA{T}", bufs=2)
                    ptB = ptpool.tile([128, S], FP16, tag=f"ptB{T}",
                                      name=f"ptB{T}", bufs=2)
                    pts_of[(p, T)] = (ptA, ptB)

                    sc_mm(st_a, p, T, 0)
                    nc.scalar.activation(
                        out=ptA[:].rearrange("p (h q) -> p h q", h=2),
                        in_=st_a[:].rearrange("p (h q) -> p h q", h=2),
                        func=mybir.ActivationFunctionType.Exp,
                        scale=float(SCALE),
                    )
                    # the very last slot's PV feeds the epilogue critical
                    # path: normal priority so the finish chain isn't
                    # stuck behind projection-head filler
                    last_slot = p == NPAIR - 1 and T == NC_T - 1
                    with tc.high_priority(offset=0 if last_slot else LOWPRI):
                        if T > 0:
                            for h in range(2):
                                nc.tensor.matmul(
                                    pv0_of[p][h][:],
                                    v_ext[T - 1][
                                        :, (2 * p + h) * VW:(2 * p + h) * VW + 128
                                    ],
                                    pts_of[(p, T - 1)][0][:, h * 512:(h + 1) * 512],
                                    start=(T == 1),
                                    stop=(T == NC_T - 1),
                                )
                        if p == NPAIR - 1 and T > 0:
                            for h in range(2):
                                nc.tensor.matmul(
                                    pv1_of[p][h][:],
                                    v_ext[T - 1][
                                        :, (2 * p + h) * VW:(2 * p + h) * VW + 128
                                    ],
                                    pts_of[(p, T - 1)][1][:, h * 512:(h + 1) * 512],
                                    start=(T == 1),
                                    stop=(T == NC_T - 1),
                                )
                        for th in extras[p][T]:
                            th()
                    sc_mm(st_b, p, T, 1)
                    nc.scalar.activation(
                        out=ptB[:].rearrange("p (h q) -> p h q", h=2),
                        in_=st_b[:].rearrange("p (h q) -> p h q", h=2),
                        func=mybir.ActivationFunctionType.Exp,
                        scale=float(SCALE),
                    )

                def scalar_recip(dst, src):
                    nc.scalar.add_instruction(
                        mybir.InstActivation(
                            name=nc.get_next_instruction_name(),
                            ins=[
                                nc.scalar.lower_ap(src),
                                mybir.ImmediateValue(
                                    dtype=mybir.dt.float32, value=0.0
                                ),
                                mybir.ImmediateValue(
                                    dtype=mybir.dt.float32, value=1.0
                                ),
                                mybir.ImmediateValue(
                                    dtype=mybir.dt.float32, value=0.0
                                ),
                            ],
                            outs=[nc.scalar.lower_ap(dst)],
                            func=mybir.ActivationFunctionType.Reciprocal,
                        )
                    )

                recip5 = {}

                def finish_c0(p):
                    for h in range(2):
                        nc.tensor.matmul(
                            pv0_of[p][h][:],
                            v_ext[NC_T - 1][
                                :, (2 * p + h) * VW:(2 * p + h) * VW + 128
                            ],
                            pts_of[(p, NC_T - 1)][0][:, h * 512:(h + 1) * 512],
                            start=False,
                            stop=True,
                        )
                    if p == NPAIR - 1:
                        # preload the reciprocal ACT table set (the real
                        # reciprocals would otherwise pay the ~2.7us table
                        # switch on the critical tail), then the c0-half
                        # reciprocals straight off the PSUM sums rows --
                        # no DVE copy in the chain
                        scalar_recip(
                            normp.tile([1, 1], F32, tag="rscr", name="rscr",
                                       bufs=1)[:],
                            ones64[0:1, 0:1],
                        )
                        recip5[0] = normp.tile([1, 2, 512], FP16, tag="rc0",
                                               name="rc0", bufs=1)
                        for h in range(2):
                            scalar_recip(
                                recip5[0][0:1, h, :],
                                pv0_of[p][h][HD:HD + 1, :],
                            )
                    else:
                        for h in range(2):
                            nc.vector.tensor_copy(
                                sums_sb_of[p][0:1, h * S:h * S + 512],
                                pv0_of[p][h][HD:HD + 1, :],
                            )
                    for h in range(2):
                        nc.vector.tensor_copy(
                            attnTa[p][h * 64:(h + 1) * 64, :],
                            pv0_of[p][h][0:HD, :],
                        )
                    del pv0_of[p]

                def finish_c1_last(p):
                    for h in range(2):
                        nc.tensor.matmul(
                            pv1_of[p][h][:],
                            v_ext[NC_T - 1][
                                :, (2 * p + h) * VW:(2 * p + h) * VW + 128
                            ],
                            pts_of[(p, NC_T - 1)][1][:, h * 512:(h + 1) * 512],
                            start=False,
                            stop=True,
                        )
                    recip5[1] = normp.tile([1, 2, 512], FP16, tag="rc1",
                                           name="rc1", bufs=1)
                    for h in range(2):
                        scalar_recip(
                            recip5[1][0:1, h, :],
                            pv1_of[p][h][HD:HD + 1, :],
                        )
                    # attnTb copies are deferred to norm5_half(1) so the
                    # c0-half normalize multiplies run first on the DVE
                    for Tq in range(NC_T):
                        del pts_of[(p, Tq)]

                def burst_c1(p):
                    pv1 = [bg_tile(f"pvb{h}") for h in range(2)]
                    for Tq in range(NC_T):
                        for h in range(2):
                            nc.tensor.matmul(
                                pv1[h][:],
                                v_ext[Tq][
                                    :, (2 * p + h) * VW:(2 * p + h) * VW + 128
                                ],
                                pts_of[(p, Tq)][1][:, h * 512:(h + 1) * 512],
                                start=(Tq == 0),
                                stop=(Tq == NC_T - 1),
                            )
                    for h in range(2):
                        nc.vector.tensor_copy(
                            sums_sb_of[p][0:1, h * S + 512:h * S + 1024],
                            pv1[h][HD:HD + 1, :],
                        )
                        nc.vector.tensor_copy(
                            attnTb[p][h * 64:(h + 1) * 64, :],
                            pv1[h][0:HD, :],
                        )
                    for Tq in range(NC_T):
                        del pts_of[(p, Tq)]
                    norm(p)

                def norm(p):
                    # reciprocal of the 2048 sums: repartition [1,2048] ->
                    # [128,16] via a DRAM bounce (issued on the idle GpSimd
                    # SWDGE queue) so the 8-cycle/element DVE divide runs
                    # on 128 lanes (pairs 0-4; the last pair is handled by
                    # norm5_half on the epilogue path)
                    sums_sb = sums_sb_of[p]
                    rd = rdram.tile([1, 2 * S], F32, tag="rd", name="rd")
                    sd = rdram.tile([1, 2 * S], F32, tag="sd", name="sd")
                    nc.gpsimd.dma_start(out=sd[:], in_=sums_sb[:])
                    sr = normp.tile([128, 16], F32, tag="sr", name="sr")
                    nc.gpsimd.dma_start(
                        out=sr[:],
                        in_=bass.AP(
                            tensor=sd.tensor,
                            offset=sd.offset,
                            ap=[[16, 128], [1, 16]],
                        ),
                    )
                    rr = normp.tile([128, 16], F32, tag="rr", name="rr")
                    nc.vector.reciprocal(rr[:], sr[:])
                    nc.gpsimd.dma_start(
                        out=bass.AP(
                            tensor=rd.tensor,
                            offset=rd.offset,
                            ap=[[16, 128], [1, 16]],
                        ),
                        in_=rr[:],
                    )
                    rb = normp.tile([128, S], F32, tag="rb", name="rb")
                    for h in range(2):
                        row = rd[0:1, h * S:(h + 1) * S]
                        row_bc = bass.AP(
                            tensor=row.tensor,
                            offset=row.offset,
                            ap=[[0, 64]] + list(row.ap[1:]),
                        )
                        nc.gpsimd.dma_start(
                            out=rb[h * 64:(h + 1) * 64, :], in_=row_bc
                        )
                    rb_r = rb[:].rearrange("d (c q) -> d c q", q=512)
                    nc.vector.tensor_mul(
                        attnTa[p][:], attnTa[p][:], rb_r[:, 0, :]
                    )
                    nc.vector.tensor_mul(
                        attnTb[p][:], attnTb[p][:], rb_r[:, 1, :]
                    )

                def norm5_half(c):
                    # last pair, one column half: broadcast 1/sums via two
                    # matmuls into a freed pva bank (bg banks still hold
                    # the unread chunk-1 accumulators), then normalize
                    p = NPAIR - 1
                    at = (attnTa, attnTb)[c][p]
                    bc = pvps.tile([128, 512], F32, tag=f"pva{c}",
                                   name=f"bc{c}", bufs=1)
                    for h in range(2):
                        nc.tensor.matmul(
                            bc[h * 64:(h + 1) * 64, :],
                            ones64[0:1, :],
                            recip5[c][0:1, h, :],
                            start=True,
                            stop=True,
                        )
                    if c == 1:
                        for h in range(2):
                            nc.vector.tensor_copy(
                                at[h * 64:(h + 1) * 64, :],
                                pv1_of[p][h][0:HD, :],
                            )
                        del pv1_of[p]
                    for h in range(2):
                        nc.vector.tensor_mul(
                            at[h * 64:(h + 1) * 64, :],
                            at[h * 64:(h + 1) * 64, :],
                            bc[h * 64:(h + 1) * 64, :],
                        )

                # emission order: chunk-1 burst of pair p-1 deferred past
                # the next pair's first two slots (low priority keeps it
                # out of the scores' way).  Pair 4's burst is un-deferred
                # (the bg banks belong to pair 5's incremental chunk-1
                # during pair 5), and pair 5 finishes both chunks inline.
                for p in range(NPAIR):
                    slot(p, 0)
                    slot(p, 1)
                    if 0 < p < NPAIR - 1:
                        with tc.high_priority(offset=LOWPRI):
                            burst_c1(p - 1)
                    for T in range(2, NC_T):
                        slot(p, T)
                    if p == NPAIR - 1:
                        # the last pair's finish/normalize chain is the
                        # epilogue critical path: absolute top priority so
                        # it always outranks the projection-head filler
                        with tc.high_priority():
                            finish_c0(p)
                            finish_c1_last(p)
                    else:
                        with tc.high_priority(offset=LOWPRI):
                            finish_c0(p)
                            if p == NPAIR - 2:
                                burst_c1(p)
                # pair-5 norm is emitted between the first two projection
                # heads: the PE instruction stream is static, so the bc
                # matmuls must sit AFTER ~4us of head matmuls to cover the
                # ScalarE reciprocal (+table load) latency without a stall

                # ---------------- output projection ----------------
                # PSUM ping-pongs the freed score tiles (tags stA/stB).
                # Depth-2 pipeline: each tile's pair-5 matmul (gated by the
                # last normalization) is deferred past the next tile's
                # early matmuls.  Output stores go out fp16 on the GpSimd
                # queue.
                def proj_head(tt):
                    ps = stps.tile([128, S], F32,
                                   tag=("stA", "stB")[tt % 2], name=f"prj{tt}",
                                   bufs=1)
                    for o0, ow in [(0, 512), (512, 256)]:
                        for p in range(NPAIR - 1):
                            nc.tensor.matmul(
                                ps[:, o0:o0 + ow],
                                attn_q(p, tt),
                                wprojT[:, p, o0:o0 + ow],
                                start=(p == 0),
                                stop=False,
                            )
                    return ps

                def proj_tail(tt, ps):
                    for o0, ow in [(0, 512), (512, 256)]:
                        nc.tensor.matmul(
                            ps[:, o0:o0 + ow],
                            attn_q(NPAIR - 1, tt),
                            wprojT[:, NPAIR - 1, o0:o0 + ow],
                            start=False,
                            stop=True,
                        )
                    ob = outp.tile([128, DIM], FP16, tag="ob", name="ob")
                    nc.vector.tensor_copy(ob[:, 0:512], ps[:, 0:512])
                    nc.scalar.copy(out=ob[:, 512:768], in_=ps[:, 512:768])
                    nc.gpsimd.dma_start(
                        out=out_ext[tt * 128:(tt + 1) * 128, :], in_=ob[:]
                    )

                # [head0, bc_c0, head1, bc_c1, tail0, head2, tail1, ...]:
                # each norm half sits behind a head's worth of PE work so
                # the split reciprocals are ready when the PE reaches the
                # bc matmuls, and tails 0-3 only need the c0 half
                pending = None
                for tt in range(NC_T):
                    ps = proj_head(tt)
                    if tt <= 1:
                        with tc.high_priority():
                            norm5_half(tt)
                    if pending is not None:
                        proj_tail(*pending)
                    pending = (tt, ps)
                proj_tail(*pending)

    nc.finalize()
    return nc


_NC_CACHE = None


def kernel(**inputs) -> np.ndarray:
    global _NC_CACHE
    x = np.asarray(inputs["x"], dtype=np.float32)
    w_qkv = np.asarray(inputs["w_qkv"], dtype=np.float32)
    w_proj = np.asarray(inputs["w_proj"], dtype=np.float32)
    b_proj = np.asarray(inputs["b_proj"], dtype=np.float32)
    B, H, W, C = x.shape
    assert (B, H * W, C) == (8, S, DIM)

    # host-side sharding + layout prep: channel-major fp16 operands
    wqkvT = np.ascontiguousarray(w_qkv.T).astype(np.float16)       # [768, 2304]
    wprojT = np.ascontiguousarray(w_proj.T).astype(np.float16)     # [768, 768]
    xTs = [
        np.ascontiguousarray(x[b].reshape(S, DIM).T).astype(np.float16)
        for b in range(B)
    ]

    if _NC_CACHE is None:
        _NC_CACHE = build_bass()
    nc = _NC_CACHE

    in_maps = [
        {"xT": xTs[b], "w_qkvT": wqkvT, "w_projT": wprojT}
        for b in range(B)
    ]
    res = run_bass_kernel_spmd(nc, in_maps, list(range(B)))
    out = np.stack(
        [
            np.asarray(res.results[b]["out"]).astype(np.float32).reshape(H, W, C)
            for b in range(B)
        ]
    )
    return (out + b_proj.reshape(1, 1, 1, C)).astype(np.float32)


if __name__ == "__main__":
    rng = np.random.default_rng(0)
    ins = {
        "x": rng.standard_normal((8, 32, 32, DIM), dtype=np.float32),
        "w_qkv": rng.standard_normal((3 * DIM, DIM), dtype=np.float32)
        * DIM ** -0.5,
        "w_proj": rng.standard_normal((DIM, DIM), dtype=np.float32) * DIM ** -0.5,
        "b_proj": np.zeros(DIM, dtype=np.float32),
    }
    o = kernel(**ins)
    print(o.shape, o.dtype)



# revision 75
# speedup vs baseline: 1.0438x; 1.0110x over previous
"""Trainium2 Bass kernel for nn_Attention_10917806866815.

Multi-head attention forward (B=8, S=32x32=1024, C=768, 12 heads, hd=64),
data-parallel across 8 NeuronCores: core b computes batch element b.
No collectives needed.

Host side (sharding-time prep in kernel()): inputs are pre-transposed to
channel-major and cast to fp16, so the device kernel is pure matmul work:
  xT [768,1024], w_qkvT [768,2304], w_projT [768,768] -- all fp16.

Pipeline (v2.1). The Tile scheduler is dependency-driven (emission order
is only a priority tie-break), and PSUM write-after-read hazards are
tracked per *tile*, so the slot structure is built around two separate
score tiles:

  st_A [128,1024] = c0 of both heads   (q columns 0-511)
  st_B [128,1024] = c1 of both heads   (q columns 512-1023)

Per slot: paired score matmuls (partition bases 0/64 -> disjoint PE row
groups, run concurrently) fill st_A then st_B; ScalarE exps the two
halves separately (exp_A, exp_B) into one merged pt tile [128, 2048].
The next slot's c0 matmuls only WAR-wait on exp_A and c1 only on exp_B,
so the slot cadence is the ScalarE back-to-back rate (~2.3us), not the
previous serialized scores->exp_h0->exp_h1->scores loop (~2.8us).

DMA: a dma_start costs ~1.3us of sequencer issue time, so the inputs
move as 8 large multi-dim-AP transfers (split between the SP and
Activation HWDGE queues), and the mid-kernel normalization bounce plus
the output stores issue from the otherwise-idle GpSimd software-DGE
queue.  Output is stored fp16 (halves traffic; fp16 rounding is far
inside the error budget).

QKV "extras" (projection chunks), PV chunk-0 (1-slot lag), the deferred
chunk-1 bursts, and evacuations are all emitted at low scheduler
priority so a ready score matmul always pops first.

Output projection in the epilogue ping-pongs the freed score tiles
(head = pairs 0-4, tail = pair 5 after the last normalization).

Precision: fp16 operands with fp32 PSUM accumulation.
"""

import numpy as np

import concourse.bass as bass
import concourse.mybir as mybir
import concourse.tile as tile
from concourse import bacc
from concourse.bass_utils import run_bass_kernel_spmd

DIM = 768
S = 1024
NH = 12
HD = 64
SCALE = HD ** -0.5

F32 = mybir.dt.float32
FP16 = mybir.dt.float16

NC_T = S // 128          # 8 token tiles
NC_C = DIM // 128        # 6 channel tiles
NPAIR = NH // 2          # 6 head pairs
VW = HD + 1              # 65: v columns per head incl. ones column

LOWPRI = -1_000_000      # deprioritize non-score work in the ready heap


def build_bass():
    nc = bacc.Bacc(None, target_bir_lowering=False)

    xT_ext = nc.declare_dram_parameter("xT", [DIM, S], FP16, isOutput=False)
    wqkvT_ext = nc.declare_dram_parameter(
        "w_qkvT", [DIM, 3 * DIM], FP16, isOutput=False
    )
    wprojT_ext = nc.declare_dram_parameter(
        "w_projT", [DIM, DIM], FP16, isOutput=False
    )
    out_ext = nc.declare_dram_parameter("out", [S, DIM], FP16, isOutput=True)

    with tile.TileContext(nc) as tc:
        from contextlib import ExitStack

        with ExitStack() as ctx:
            consts = ctx.enter_context(tc.tile_pool(name="consts", bufs=1))
            persist = ctx.enter_context(tc.tile_pool(name="persist", bufs=1))

            # c-major operands: [:, j, :] is channel-tile j.
            xT = persist.tile([128, NC_C, S], FP16, tag="xT", name="xT")
            wqkvT = persist.tile(
                [128, NC_C, 3 * DIM], FP16, tag="wqkvT", name="wqkvT"
            )
            wprojT = persist.tile([128, NC_C, DIM], FP16, tag="wprojT", name="wprojT")

            # ---- bulk input DMA: 8 large transfers, ordered by need ----
            # srcs as [p, k, ...] views of the DRAM tensors
            x_src = xT_ext[:].rearrange("(k p) s -> p k s", k=NC_C)
            w_src = wqkvT_ext[:].rearrange(
                "(k p) (g c) -> p k g c", k=NC_C, g=3
            )
            wp_src = wprojT_ext[:].rearrange("(k p) c -> p k c", k=NC_C)
            w_dst = wqkvT[:].rearrange("p k (g c) -> p k g c", g=3)

            # x split by k-tiles across the SP and Activation HWDGE queues
            # so both column halves land ~12us; q0/k0 ride the Vector
            # queue (small, land early); q1/k1 + wproj on the GpSimd
            # SWDGE which is otherwise idle until the first norm bounce.
            # x in quarters: both c0 quarters first so the first qk chunks
            # can start ~4us earlier; c1 quarters follow on the same queues
            nc.sync.dma_start(out=xT[:, 0:3, 0:512], in_=x_src[:, 0:3, 0:512])
            nc.scalar.dma_start(
                out=xT[:, 3:6, 0:512], in_=x_src[:, 3:6, 0:512]
            )
            nc.sync.dma_start(
                out=xT[:, 0:3, 512:1024], in_=x_src[:, 0:3, 512:1024]
            )
            nc.scalar.dma_start(
                out=xT[:, 3:6, 512:1024], in_=x_src[:, 3:6, 512:1024]
            )
            nc.gpsimd.dma_start(                                 # q0
                out=w_dst[:, :, 0, 0:128], in_=w_src[:, :, 0, 0:128]
            )
            nc.gpsimd.dma_start(                                 # k0
                out=w_dst[:, :, 1, 0:128], in_=w_src[:, :, 1, 0:128]
            )
            nc.gpsimd.dma_start(                                 # q1
                out=w_dst[:, :, 0, 128:256], in_=w_src[:, :, 0, 128:256]
            )
            nc.gpsimd.dma_start(                                 # k1
                out=w_dst[:, :, 1, 128:256], in_=w_src[:, :, 1, 128:256]
            )
            # v heads 0-7 (pair-0 extras need them early)
            nc.sync.dma_start(
                out=w_dst[:, :, 2, 0:512], in_=w_src[:, :, 2, 0:512]
            )
            # q2-5, k2-5
            nc.sync.dma_start(
                out=w_dst[:, :, 0, 256:768], in_=w_src[:, :, 0, 256:768]
            )
            nc.sync.dma_start(
                out=w_dst[:, :, 1, 256:768], in_=w_src[:, :, 1, 256:768]
            )
            # v heads 8-11
            nc.scalar.dma_start(
                out=w_dst[:, :, 2, 512:768], in_=w_src[:, :, 2, 512:768]
            )
            nc.gpsimd.dma_start(out=wprojT[:], in_=wp_src[:])

            qkT = [
                persist.tile([128, S], FP16, tag=f"qkT{ot}", name=f"qkT{ot}")
                for ot in range(2 * NPAIR)
            ]
            # v_ext rows padded to NH*VW+63 so every per-head stationary
            # slice can be 128 columns wide (NumWeights==128 -> the LDW
            # uses fast-weight-load and hides behind in-flight matmuls);
            # PV out rows 65-127 are garbage and never read.
            v_ext = [
                persist.tile([128, NH * VW + 63], FP16, tag=f"vext{tt}",
                             name=f"vext{tt}")
                for tt in range(NC_T)
            ]
            # attnT as column-half tiles: projection q-tiles 0-3 only
            # depend on the c0 half, so pair-5's epilogue normalize can
            # release them early
            attnTa = [
                persist.tile([128, 512], FP16, tag=f"attnTa{p}",
                             name=f"attnTa{p}")
                for p in range(NPAIR)
            ]
            attnTb = [
                persist.tile([128, 512], FP16, tag=f"attnTb{p}",
                             name=f"attnTb{p}")
                for p in range(NPAIR)
            ]

            def attn_q(p, tt):
                # [128, 128] slice of pair p's attnT at q-tile tt
                return (attnTa, attnTb)[tt // 4][p][
                    :, (tt % 4) * 128:(tt % 4 + 1) * 128
                ]
            ones64 = consts.tile([1, 64], FP16, tag="ones64", name="ones64")
            nc.vector.memset(ones64[:], 1.0)
            for tt in range(NC_T):
                nc.gpsimd.memset(v_ext[tt][:], 1.0)

            with (
                tc.tile_pool(name="stps", bufs=1, space="PSUM") as stps,
                tc.tile_pool(name="pvps", bufs=1, space="PSUM") as pvps,
                tc.tile_pool(name="bgps", bufs=1, space="PSUM") as bgps,
                tc.tile_pool(name="ptpool", bufs=1) as ptpool,
                tc.tile_pool(name="normp", bufs=2) as normp,
                tc.tile_pool(name="outp", bufs=3) as outp,
                tc.tile_pool(name="rdram", bufs=2, space="DRAM") as rdram,
            ):
                # 2 shared background PSUM banks: QKV-projection extras,
                # chunk-1 PV bursts, warm-up, norm broadcasts.  Each
                # logical use occupies its tag contiguously in emission
                # order.
                bg_flip = [0]

                def bg_tile(name, shape=(128, 512)):
                    t = bgps.tile(list(shape), F32, tag=f"bg{bg_flip[0]}",
                                  name=name, bufs=1)
                    bg_flip[0] ^= 1
                    return t

                # ---- QKV building blocks ----
                def emit_qk_chunk(ot, c):
                    ps = bg_tile("qkvp")
                    for k in range(NC_C):
                        nc.tensor.matmul(
                            ps[:],
                            wqkvT[:, k, ot * 128:(ot + 1) * 128],
                            xT[:, k, c * 512:(c + 1) * 512],
                            start=(k == 0),
                            stop=(k == NC_C - 1),
                        )
                    nc.vector.tensor_copy(qkT[ot][:, c * 512:(c + 1) * 512], ps[:])

                def emit_v_chunk(tt, c):
                    o0, ow, h0, nh = [
                        (2 * DIM, 512, 0, 8), (2 * DIM + 512, 256, 8, 4)
                    ][c]
                    ps = bg_tile("vp")
                    for k in range(NC_C):
                        nc.tensor.matmul(
                            ps[:, :ow],
                            xT[:, k, tt * 128:(tt + 1) * 128],
                            wqkvT[:, k, o0:o0 + ow],
                            start=(k == 0),
                            stop=(k == NC_C - 1),
                        )
                    dst = (
                        v_ext[tt][:, 0:NH * VW]
                        .rearrange("p (h e) -> p h e", e=VW)[:, h0:h0 + nh, 0:HD]
                    )
                    nc.vector.tensor_copy(
                        dst, ps[:, :ow].rearrange("p (h e) -> p h e", e=HD)
                    )

                # extras[p][T]: QKV work dependencies only require:
                #   v chunk-0 tile T ready before pair-0 PV consumes it at
                #   slot T+1; pair p+1's q/k ready before pair p+1.
                # The dep-driven scheduler fills PE idle time with these
                # (they run at low priority).
                extras = [[[] for _ in range(NC_T)] for _ in range(NPAIR)]

                def TH(f, *a):
                    return lambda: f(*a)

                for tt in range(NC_T):
                    extras[0][tt].append(TH(emit_v_chunk, tt, 0))
                for p in range(1, NPAIR - 1):
                    extras[p][2].append(TH(emit_qk_chunk, p + 1, 0))
                    extras[p][3].append(TH(emit_qk_chunk, NPAIR + p + 1, 0))
                    extras[p][5].append(TH(emit_qk_chunk, p + 1, 1))
                    extras[p][6].append(TH(emit_qk_chunk, NPAIR + p + 1, 1))
                for i in range(NC_T):  # v chunk-1 (needed by pair 4's PV)
                    extras[1 + i // 3][[1, 4, 7][i % 3]].append(
                        TH(emit_v_chunk, i, 1)
                    )

                # ---- HAM warm-up: keep the PE busy through the DMA
                # lead-in so the first real matmuls run at full clock ----
                # 28 matmuls (~7-8us): long enough to cover the input-DMA
                # wait so the PE never sees a >3.4us idle window (which
                # would re-throttle HAM and run the prologue at 1.2 GHz)
                wu = consts.tile([128, 512], FP16, tag="wu", name="wu")
                nc.vector.memset(wu[:], 0.0)
                wups = bg_tile("wups")
                NWU = 16
                for i in range(NWU):
                    nc.tensor.matmul(
                        wups[:], wu[:, 0:128], wu[:],
                        start=(i == 0), stop=(i == NWU - 1),
                    )

                # ---- prologue: q/k for pairs 0 and 1 up front (low
                # priority so pair-0 score matmuls preempt as soon as
                # their chunks land) ----
                with tc.high_priority(offset=LOWPRI):
                    emit_qk_chunk(0, 0)
                    emit_qk_chunk(NPAIR, 0)
                    emit_qk_chunk(1, 0)
                    emit_qk_chunk(NPAIR + 1, 0)
                    emit_qk_chunk(0, 1)
                    emit_qk_chunk(NPAIR, 1)
                    emit_qk_chunk(1, 1)
                    emit_qk_chunk(NPAIR + 1, 1)

                # ---- attention: software-pipelined slot stream ----
                pts_of = {}     # (p, T) -> pt tile [128, 2048] h-major
                pv0_of = {}     # p -> [pv0_h0, pv0_h1]  (chunk-0 accums)
                pv1_of = {}     # last pair only: incremental chunk-1 accums
                sums_sb_of = {}

                def sc_mm(st, p, T, c):
                    kT_t = qkT[NPAIR + p]
                    qT_t = qkT[p]
                    for h in range(2):
                        r0 = h * 64
                        nc.tensor.matmul(
                            st[:, h * 512:(h + 1) * 512],
                            kT_t[r0:r0 + 64, T * 128:(T + 1) * 128],
                            qT_t[r0:r0 + 64, c * 512:(c + 1) * 512],
                            start=True,
                            stop=True,
                        )

                def slot(p, T):
                    if T == 0:
                        if p < NPAIR - 1:
                            sums_sb_of[p] = normp.tile(
                                [1, 2 * S], F32, tag="sums", name="sums",
                                bufs=2
                            )
                        pv0_of[p] = [
                            pvps.tile([128, 512], F32, tag=f"pva{h}",
                                      name=f"pva{h}", bufs=1)
                            for h in range(2)
                        ]
                        if p == NPAIR - 1:
                            # last pair: chunk-1 accumulates incrementally
                            # in the (now extras-free) background banks so
                            # the epilogue isn't serialized behind a burst;
                            # sums go to per-half tiles so each half's
                            # reciprocal fires as soon as its rows land
                            pv1_of[p] = [bg_tile(f"pvL{h}") for h in range(2)]
                    st_a = stps.tile([128, S], F32, tag="stA", name="stA",
                                     bufs=1)
                    st_b = stps.tile([128, S], F32, tag="stB", name="stB",
                                     bufs=1)
                    # per-half pt tiles: chunk-0 consumers only RAW-wait on
                    # exp_A, chunk-1 only on exp_B
                    ptA = ptpool.tile([128, S], FP16, tag=f"ptA{T}",
                                      name=f"ptA{T}", bufs=2)
                    ptB = ptpool.tile([128, S], FP16, tag=f"ptB{T}",
                                      name=f"ptB{T}", bufs=2)
                    pts_of[(p, T)] = (ptA, ptB)

                    sc_mm(st_a, p, T, 0)
                    nc.scalar.activation(
                        out=ptA[:].rearrange("p (h q) -> p h q", h=2),
                        in_=st_a[:].rearrange("p (h q) -> p h q", h=2),
                        func=mybir.ActivationFunctionType.Exp,
                        scale=float(SCALE),
                    )
                    # the very last slot's PV feeds the epilogue critical
                    # path: normal priority so the finish chain isn't
                    # stuck behind projection-head filler
                    last_slot = p == NPAIR - 1 and T == NC_T - 1
                    with tc.high_priority(offset=0 if last_slot else LOWPRI):
                        if T > 0:
                            for h in range(2):
                                nc.tensor.matmul(
                                    pv0_of[p][h][:],
                                    v_ext[T - 1][
                                        :, (2 * p + h) * VW:(2 * p + h) * VW + 128
                                    ],
                                    pts_of[(p, T - 1)][0][:, h * 512:(h + 1) * 512],
                                    start=(T == 1),
                                    stop=(T == NC_T - 1),
                                )
                        if p == NPAIR - 1 and T > 0:
                            for h in range(2):
                                nc.tensor.matmul(
                                    pv1_of[p][h][:],
                                    v_ext[T - 1][
                                        :, (2 * p + h) * VW:(2 * p + h) * VW + 128
                                    ],
                                    pts_of[(p, T - 1)][1][:, h * 512:(h + 1) * 512],
                                    start=(T == 1),
                                    stop=(T == NC_T - 1),
                                )
                        for th in extras[p][T]:
                            th()
                    sc_mm(st_b, p, T, 1)
                    nc.scalar.activation(
                        out=ptB[:].rearrange("p (h q) -> p h q", h=2),
                        in_=st_b[:].rearrange("p (h q) -> p h q", h=2),
                        func=mybir.ActivationFunctionType.Exp,
                        scale=float(SCALE),
                    )

                def scalar_recip(dst, src):
                    nc.scalar.add_instruction(
                        mybir.InstActivation(
                            name=nc.get_next_instruction_name(),
                            ins=[
                                nc.scalar.lower_ap(src),
                                mybir.ImmediateValue(
                                    dtype=mybir.dt.float32, value=0.0
                                ),
                                mybir.ImmediateValue(
                                    dtype=mybir.dt.float32, value=1.0
                                ),
                                mybir.ImmediateValue(
                                    dtype=mybir.dt.float32, value=0.0
                                ),
                            ],
                            outs=[nc.scalar.lower_ap(dst)],
                            func=mybir.ActivationFunctionType.Reciprocal,
                        )
                    )

                recip5 = {}

                def finish_c0(p, hot=False):
                    from contextlib import nullcontext

                    for h in range(2):
                        nc.tensor.matmul(
                            pv0_of[p][h][:],
                            v_ext[NC_T - 1][
                                :, (2 * p + h) * VW:(2 * p + h) * VW + 128
                            ],
                            pts_of[(p, NC_T - 1)][0][:, h * 512:(h + 1) * 512],
                            start=False,
                            stop=True,
                        )
                    if hot:
                        with tc.high_priority():
                            for h in range(2):
                                nc.vector.tensor_copy(
                                    sums_sb_of[p][0:1, h * S:h * S + 512],
                                    pv0_of[p][h][HD:HD + 1, :],
                                )
                            for h in range(2):
                                nc.vector.tensor_copy(
                                    attnTa[p][h * 64:(h + 1) * 64, :],
                                    pv0_of[p][h][0:HD, :],
                                )
                        del pv0_of[p]
                        return
                    if p == NPAIR - 1:
                        # preload the reciprocal ACT table set (the real
                        # reciprocals would otherwise pay the ~2.7us table
                        # switch on the critical tail), then the c0-half
                        # reciprocals straight off the PSUM sums rows --
                        # no DVE copy in the chain
                        scalar_recip(
                            normp.tile([1, 1], F32, tag="rscr", name="rscr",
                                       bufs=1)[:],
                            ones64[0:1, 0:1],
                        )
                        recip5[0] = normp.tile([1, 2, 512], FP16, tag="rc0",
                                               name="rc0", bufs=1)
                        for h in range(2):
                            scalar_recip(
                                recip5[0][0:1, h, :],
                                pv0_of[p][h][HD:HD + 1, :],
                            )
                    else:
                        for h in range(2):
                            nc.vector.tensor_copy(
                                sums_sb_of[p][0:1, h * S:h * S + 512],
                                pv0_of[p][h][HD:HD + 1, :],
                            )
                    for h in range(2):
                        nc.vector.tensor_copy(
                            attnTa[p][h * 64:(h + 1) * 64, :],
                            pv0_of[p][h][0:HD, :],
                        )
                    del pv0_of[p]

                def finish_c1_last(p):
                    for h in range(2):
                        nc.tensor.matmul(
                            pv1_of[p][h][:],
                            v_ext[NC_T - 1][
                                :, (2 * p + h) * VW:(2 * p + h) * VW + 128
                            ],
                            pts_of[(p, NC_T - 1)][1][:, h * 512:(h + 1) * 512],
                            start=False,
                            stop=True,
                        )
                    recip5[1] = normp.tile([1, 2, 512], FP16, tag="rc1",
                                           name="rc1", bufs=1)
                    for h in range(2):
                        scalar_recip(
                            recip5[1][0:1, h, :],
                            pv1_of[p][h][HD:HD + 1, :],
                        )
                    # attnTb copies are deferred to norm5_half(1) so the
                    # c0-half normalize multiplies run first on the DVE
                    for Tq in range(NC_T):
                        del pts_of[(p, Tq)]

                def burst_c1(p, hot=False):
                    # hot: the evac/norm chain (NOT the matmuls) goes at
                    # top priority so the DVE queue doesn't defer it past
                    # the epilogue -- the projection heads RAW-depend on
                    # this pair's normalize multiplies
                    from contextlib import nullcontext

                    pv1 = [bg_tile(f"pvb{h}") for h in range(2)]
                    for Tq in range(NC_T):
                        for h in range(2):
                            nc.tensor.matmul(
                                pv1[h][:],
                                v_ext[Tq][
                                    :, (2 * p + h) * VW:(2 * p + h) * VW + 128
                                ],
                                pts_of[(p, Tq)][1][:, h * 512:(h + 1) * 512],
                                start=(Tq == 0),
                                stop=(Tq == NC_T - 1),
                            )
                    with tc.high_priority() if hot else nullcontext():
                        for h in range(2):
                            nc.vector.tensor_copy(
                                sums_sb_of[p][0:1, h * S + 512:h * S + 1024],
                                pv1[h][HD:HD + 1, :],
                            )
                            nc.vector.tensor_copy(
                                attnTb[p][h * 64:(h + 1) * 64, :],
                                pv1[h][0:HD, :],
                            )
                        for Tq in range(NC_T):
                            del pts_of[(p, Tq)]
                        norm(p)

                def norm(p):
                    # reciprocal of the 2048 sums: repartition [1,2048] ->
                    # [128,16] via a DRAM bounce (issued on the idle GpSimd
                    # SWDGE queue) so the 8-cycle/element DVE divide runs
                    # on 128 lanes (pairs 0-4; the last pair is handled by
                    # norm5_half on the epilogue path)
                    sums_sb = sums_sb_of[p]
                    rd = rdram.tile([1, 2 * S], F32, tag="rd", name="rd")
                    sd = rdram.tile([1, 2 * S], F32, tag="sd", name="sd")
                    nc.gpsimd.dma_start(out=sd[:], in_=sums_sb[:])
                    sr = normp.tile([128, 16], F32, tag="sr", name="sr")
                    nc.gpsimd.dma_start(
                        out=sr[:],
                        in_=bass.AP(
                            tensor=sd.tensor,
                            offset=sd.offset,
                            ap=[[16, 128], [1, 16]],
                        ),
                    )
                    rr = normp.tile([128, 16], F32, tag="rr", name="rr")
                    nc.vector.reciprocal(rr[:], sr[:])
                    nc.gpsimd.dma_start(
                        out=bass.AP(
                            tensor=rd.tensor,
                            offset=rd.offset,
                            ap=[[16, 128], [1, 16]],
                        ),
                        in_=rr[:],
                    )
                    rb = normp.tile([128, S], F32, tag="rb", name="rb")
                    for h in range(2):
                        row = rd[0:1, h * S:(h + 1) * S]
                        row_bc = bass.AP(
                            tensor=row.tensor,
                            offset=row.offset,
                            ap=[[0, 64]] + list(row.ap[1:]),
                        )
                        nc.gpsimd.dma_start(
                            out=rb[h * 64:(h + 1) * 64, :], in_=row_bc
                        )
                    rb_r = rb[:].rearrange("d (c q) -> d c q", q=512)
                    nc.vector.tensor_mul(
                        attnTa[p][:], attnTa[p][:], rb_r[:, 0, :]
                    )
                    nc.vector.tensor_mul(
                        attnTb[p][:], attnTb[p][:], rb_r[:, 1, :]
                    )

                def norm5_half(c):
                    # last pair, one column half: broadcast 1/sums via two
                    # matmuls into a freed pva bank (bg banks still hold
                    # the unread chunk-1 accumulators), then normalize
                    p = NPAIR - 1
                    at = (attnTa, attnTb)[c][p]
                    bc = pvps.tile([128, 512], F32, tag=f"pva{c}",
                                   name=f"bc{c}", bufs=1)
                    for h in range(2):
                        nc.tensor.matmul(
                            bc[h * 64:(h + 1) * 64, :],
                            ones64[0:1, :],
                            recip5[c][0:1, h, :],
                            start=True,
                            stop=True,
                        )
                    if c == 1:
                        for h in range(2):
                            nc.vector.tensor_copy(
                                at[h * 64:(h + 1) * 64, :],
                                pv1_of[p][h][0:HD, :],
                            )
                        del pv1_of[p]
                    for h in range(2):
                        nc.vector.tensor_mul(
                            at[h * 64:(h + 1) * 64, :],
                            at[h * 64:(h + 1) * 64, :],
                            bc[h * 64:(h + 1) * 64, :],
                        )

                # emission order: chunk-1 burst of pair p-1 deferred past
                # the next pair's first two slots (low priority keeps it
                # out of the scores' way).  Pair 4's burst is un-deferred
                # (the bg banks belong to pair 5's incremental chunk-1
                # during pair 5), and pair 5 finishes both chunks inline.
                for p in range(NPAIR):
                    slot(p, 0)
                    slot(p, 1)
                    if 0 < p < NPAIR - 1:
                        with tc.high_priority(offset=LOWPRI):
                            burst_c1(p - 1)
                    for T in range(2, NC_T):
                        slot(p, T)
                    if p == NPAIR - 1:
                        # the last pair's finish/normalize chain is the
                        # epilogue critical path: absolute top priority so
                        # it always outranks the projection-head filler
                        with tc.high_priority():
                            finish_c0(p)
                            finish_c1_last(p)
                    else:
                        with tc.high_priority(offset=LOWPRI):
                            finish_c0(p, hot=(p == NPAIR - 2))
                            if p == NPAIR - 2:
                                burst_c1(p, hot=True)
                # pair-5 norm is emitted between the first two projection
                # heads: the PE instruction stream is static, so the bc
                # matmuls must sit AFTER ~4us of head matmuls to cover the
                # ScalarE reciprocal (+table load) latency without a stall

                # ---------------- output projection ----------------
                # PSUM ping-pongs the freed score tiles (tags stA/stB).
                # Depth-2 pipeline: each tile's pair-5 matmul (gated by the
                # last normalization) is deferred past the next tile's
                # early matmuls.  Output stores go out fp16 on the GpSimd
                # queue.
                def proj_head(tt):
                    ps = stps.tile([128, S], F32,
                                   tag=("stA", "stB")[tt % 2], name=f"prj{tt}",
                                   bufs=1)
                    for o0, ow in [(0, 512), (512, 256)]:
                        for p in range(NPAIR - 1):
                            nc.tensor.matmul(
                                ps[:, o0:o0 + ow],
                                attn_q(p, tt),
                                wprojT[:, p, o0:o0 + ow],
                                start=(p == 0),
                                stop=False,
                            )
                    return ps

                def proj_tail(tt, ps):
                    for o0, ow in [(0, 512), (512, 256)]:
                        nc.tensor.matmul(
                            ps[:, o0:o0 + ow],
                            attn_q(NPAIR - 1, tt),
                            wprojT[:, NPAIR - 1, o0:o0 + ow],
                            start=False,
                            stop=True,
                        )
                    ob = outp.tile([128, DIM], FP16, tag="ob", name="ob")
                    nc.vector.tensor_copy(ob[:, 0:512], ps[:, 0:512])
                    nc.scalar.copy(out=ob[:, 512:768], in_=ps[:, 512:768])
                    nc.gpsimd.dma_start(
                        out=out_ext[tt * 128:(tt + 1) * 128, :], in_=ob[:]
                    )

                # [head0, bc_c0, head1, bc_c1, tail0, head2, tail1, ...]:
                # each norm half sits behind a head's worth of PE work so
                # the split reciprocals are ready when the PE reaches the
                # bc matmuls, and tails 0-3 only need the c0 half
                pending = None
                for tt in range(NC_T):
                    ps = proj_head(tt)
                    if tt <= 1:
                        with tc.high_priority():
                            norm5_half(tt)
                    if pending is not None:
                        proj_tail(*pending)
                    pending = (tt, ps)
                proj_tail(*pending)

    nc.finalize()
    return nc


_NC_CACHE = None


def kernel(**inputs) -> np.ndarray:
    global _NC_CACHE
    x = np.asarray(inputs["x"], dtype=np.float32)
    w_qkv = np.asarray(inputs["w_qkv"], dtype=np.float32)
    w_proj = np.asarray(inputs["w_proj"], dtype=np.float32)
    b_proj = np.asarray(inputs["b_proj"], dtype=np.float32)
    B, H, W, C = x.shape
    assert (B, H * W, C) == (8, S, DIM)

    # host-side sharding + layout prep: channel-major fp16 operands
    wqkvT = np.ascontiguousarray(w_qkv.T).astype(np.float16)       # [768, 2304]
    wprojT = np.ascontiguousarray(w_proj.T).astype(np.float16)     # [768, 768]
    xTs = [
        np.ascontiguousarray(x[b].reshape(S, DIM).T).astype(np.float16)
        for b in range(B)
    ]

    if _NC_CACHE is None:
        _NC_CACHE = build_bass()
    nc = _NC_CACHE

    in_maps = [
        {"xT": xTs[b], "w_qkvT": wqkvT, "w_projT": wprojT}
        for b in range(B)
    ]
    res = run_bass_kernel_spmd(nc, in_maps, list(range(B)))
    out = np.stack(
        [
            np.asarray(res.results[b]["out"]).astype(np.float32).reshape(H, W, C)
            for b in range(B)
        ]
    )
    return (out + b_proj.reshape(1, 1, 1, C)).astype(np.float32)


if __name__ == "__main__":
    rng = np.random.default_rng(0)
    ins = {
        "x": rng.standard_normal((8, 32, 32, DIM), dtype=np.float32),
        "w_qkv": rng.standard_normal((3 * DIM, DIM), dtype=np.float32)
        * DIM ** -0.5,
        "w_proj": rng.standard_normal((DIM, DIM), dtype=np.float32) * DIM ** -0.5,
        "b_proj": np.zeros(DIM, dtype=np.float32),
    }
    o = kernel(**ins)
    print(o.shape, o.dtype)



# revision 79
# speedup vs baseline: 1.0690x; 1.0242x over previous
"""Trainium2 Bass kernel for nn_Attention_10917806866815.

Multi-head attention forward (B=8, S=32x32=1024, C=768, 12 heads, hd=64),
data-parallel across 8 NeuronCores: core b computes batch element b.
No collectives needed.

Host side (sharding-time prep in kernel()): inputs are pre-transposed to
channel-major and cast to fp16, so the device kernel is pure matmul work:
  xT [768,1024], w_qkvT [768,2304], w_projT [768,768] -- all fp16.

Pipeline (v2.1). The Tile scheduler is dependency-driven (emission order
is only a priority tie-break), and PSUM write-after-read hazards are
tracked per *tile*, so the slot structure is built around two separate
score tiles:

  st_A [128,1024] = c0 of both heads   (q columns 0-511)
  st_B [128,1024] = c1 of both heads   (q columns 512-1023)

Per slot: paired score matmuls (partition bases 0/64 -> disjoint PE row
groups, run concurrently) fill st_A then st_B; ScalarE exps the two
halves separately (exp_A, exp_B) into one merged pt tile [128, 2048].
The next slot's c0 matmuls only WAR-wait on exp_A and c1 only on exp_B,
so the slot cadence is the ScalarE back-to-back rate (~2.3us), not the
previous serialized scores->exp_h0->exp_h1->scores loop (~2.8us).

DMA: a dma_start costs ~1.3us of sequencer issue time, so the inputs
move as 8 large multi-dim-AP transfers (split between the SP and
Activation HWDGE queues), and the mid-kernel normalization bounce plus
the output stores issue from the otherwise-idle GpSimd software-DGE
queue.  Output is stored fp16 (halves traffic; fp16 rounding is far
inside the error budget).

QKV "extras" (projection chunks), PV chunk-0 (1-slot lag), the deferred
chunk-1 bursts, and evacuations are all emitted at low scheduler
priority so a ready score matmul always pops first.

Output projection in the epilogue ping-pongs the freed score tiles
(head = pairs 0-4, tail = pair 5 after the last normalization).

Precision: fp16 operands with fp32 PSUM accumulation.
"""

import numpy as np

import concourse.bass as bass
import concourse.mybir as mybir
import concourse.tile as tile
from concourse import bacc
from concourse.bass_utils import run_bass_kernel_spmd

DIM = 768
S = 1024
NH = 12
HD = 64
SCALE = HD ** -0.5

F32 = mybir.dt.float32
FP16 = mybir.dt.float16

NC_T = S // 128          # 8 token tiles
NC_C = DIM // 128        # 6 channel tiles
NPAIR = NH // 2          # 6 head pairs
VW = HD + 1              # 65: v columns per head incl. ones column

LOWPRI = -1_000_000      # deprioritize non-score work in the ready heap


def build_bass():
    nc = bacc.Bacc(None, target_bir_lowering=False)

    xT_ext = nc.declare_dram_parameter("xT", [DIM, S], FP16, isOutput=False)
    wqkvT_ext = nc.declare_dram_parameter(
        "w_qkvT", [DIM, 3 * DIM], FP16, isOutput=False
    )
    wprojT_ext = nc.declare_dram_parameter(
        "w_projT", [DIM, DIM], FP16, isOutput=False
    )
    out_ext = nc.declare_dram_parameter("out", [S, DIM], FP16, isOutput=True)

    with tile.TileContext(nc) as tc:
        from contextlib import ExitStack

        with ExitStack() as ctx:
            consts = ctx.enter_context(tc.tile_pool(name="consts", bufs=1))
            persist = ctx.enter_context(tc.tile_pool(name="persist", bufs=1))

            # c-major operands: [:, j, :] is channel-tile j.
            xT = persist.tile([128, NC_C, S], FP16, tag="xT", name="xT")
            wqkvT = persist.tile(
                [128, NC_C, 3 * DIM], FP16, tag="wqkvT", name="wqkvT"
            )
            wprojT = persist.tile([128, NC_C, DIM], FP16, tag="wprojT", name="wprojT")

            # ---- bulk input DMA: 8 large transfers, ordered by need ----
            # srcs as [p, k, ...] views of the DRAM tensors
            x_src = xT_ext[:].rearrange("(k p) s -> p k s", k=NC_C)
            w_src = wqkvT_ext[:].rearrange(
                "(k p) (g c) -> p k g c", k=NC_C, g=3
            )
            wp_src = wprojT_ext[:].rearrange("(k p) c -> p k c", k=NC_C)
            w_dst = wqkvT[:].rearrange("p k (g c) -> p k g c", g=3)

            # x split by k-tiles across the SP and Activation HWDGE queues
            # so both column halves land ~12us; q0/k0 ride the Vector
            # queue (small, land early); q1/k1 + wproj on the GpSimd
            # SWDGE which is otherwise idle until the first norm bounce.
            # x in quarters: both c0 quarters first so the first qk chunks
            # can start ~4us earlier; c1 quarters follow on the same queues
            nc.sync.dma_start(out=xT[:, 0:3, 0:512], in_=x_src[:, 0:3, 0:512])
            nc.scalar.dma_start(
                out=xT[:, 3:6, 0:512], in_=x_src[:, 3:6, 0:512]
            )
            nc.sync.dma_start(
                out=xT[:, 0:3, 512:1024], in_=x_src[:, 0:3, 512:1024]
            )
            nc.scalar.dma_start(
                out=xT[:, 3:6, 512:1024], in_=x_src[:, 3:6, 512:1024]
            )
            nc.gpsimd.dma_start(                                 # q0
                out=w_dst[:, :, 0, 0:128], in_=w_src[:, :, 0, 0:128]
            )
            nc.gpsimd.dma_start(                                 # k0
                out=w_dst[:, :, 1, 0:128], in_=w_src[:, :, 1, 0:128]
            )
            nc.gpsimd.dma_start(                                 # q1
                out=w_dst[:, :, 0, 128:256], in_=w_src[:, :, 0, 128:256]
            )
            nc.gpsimd.dma_start(                                 # k1
                out=w_dst[:, :, 1, 128:256], in_=w_src[:, :, 1, 128:256]
            )
            # v heads 0-7 (pair-0 extras need them early)
            nc.sync.dma_start(
                out=w_dst[:, :, 2, 0:512], in_=w_src[:, :, 2, 0:512]
            )
            # q2-5, k2-5
            nc.sync.dma_start(
                out=w_dst[:, :, 0, 256:768], in_=w_src[:, :, 0, 256:768]
            )
            nc.sync.dma_start(
                out=w_dst[:, :, 1, 256:768], in_=w_src[:, :, 1, 256:768]
            )
            # v heads 8-11
            nc.scalar.dma_start(
                out=w_dst[:, :, 2, 512:768], in_=w_src[:, :, 2, 512:768]
            )
            nc.gpsimd.dma_start(out=wprojT[:], in_=wp_src[:])

            qkT = [
                persist.tile([128, S], FP16, tag=f"qkT{ot}", name=f"qkT{ot}")
                for ot in range(2 * NPAIR)
            ]
            # v_ext rows padded to NH*VW+63 so every per-head stationary
            # slice can be 128 columns wide (NumWeights==128 -> the LDW
            # uses fast-weight-load and hides behind in-flight matmuls);
            # PV out rows 65-127 are garbage and never read.
            v_ext = [
                persist.tile([128, NH * VW + 63], FP16, tag=f"vext{tt}",
                             name=f"vext{tt}")
                for tt in range(NC_T)
            ]
            # attnT as column-half tiles: projection q-tiles 0-3 only
            # depend on the c0 half, so pair-5's epilogue normalize can
            # release them early
            attnTa = [
                persist.tile([128, 512], FP16, tag=f"attnTa{p}",
                             name=f"attnTa{p}")
                for p in range(NPAIR)
            ]
            attnTb = [
                persist.tile([128, 512], FP16, tag=f"attnTb{p}",
                             name=f"attnTb{p}")
                for p in range(NPAIR)
            ]

            def attn_q(p, tt):
                # [128, 128] slice of pair p's attnT at q-tile tt
                return (attnTa, attnTb)[tt // 4][p][
                    :, (tt % 4) * 128:(tt % 4 + 1) * 128
                ]
            ones64 = consts.tile([1, 64], FP16, tag="ones64", name="ones64")
            nc.vector.memset(ones64[:], 1.0)
            for tt in range(NC_T):
                nc.gpsimd.memset(v_ext[tt][:], 1.0)

            with (
                tc.tile_pool(name="stps", bufs=1, space="PSUM") as stps,
                tc.tile_pool(name="pvps", bufs=1, space="PSUM") as pvps,
                tc.tile_pool(name="bgps", bufs=1, space="PSUM") as bgps,
                tc.tile_pool(name="ptpool", bufs=1) as ptpool,
                tc.tile_pool(name="normp", bufs=2) as normp,
                tc.tile_pool(name="outp", bufs=3) as outp,
                tc.tile_pool(name="rdram", bufs=2, space="DRAM") as rdram,
            ):
                # 2 shared background PSUM banks: QKV-projection extras,
                # chunk-1 PV bursts, warm-up, norm broadcasts.  Each
                # logical use occupies its tag contiguously in emission
                # order.
                bg_flip = [0]

                def bg_tile(name, shape=(128, 512)):
                    t = bgps.tile(list(shape), F32, tag=f"bg{bg_flip[0]}",
                                  name=name, bufs=1)
                    bg_flip[0] ^= 1
                    return t

                # ---- QKV building blocks ----
                def emit_qk_chunk(ot, c):
                    ps = bg_tile("qkvp")
                    for k in range(NC_C):
                        nc.tensor.matmul(
                            ps[:],
                            wqkvT[:, k, ot * 128:(ot + 1) * 128],
                            xT[:, k, c * 512:(c + 1) * 512],
                            start=(k == 0),
                            stop=(k == NC_C - 1),
                        )
                    nc.vector.tensor_copy(qkT[ot][:, c * 512:(c + 1) * 512], ps[:])

                def emit_v_chunk(tt, c):
                    o0, ow, h0, nh = [
                        (2 * DIM, 512, 0, 8), (2 * DIM + 512, 256, 8, 4)
                    ][c]
                    ps = bg_tile("vp")
                    for k in range(NC_C):
                        nc.tensor.matmul(
                            ps[:, :ow],
                            xT[:, k, tt * 128:(tt + 1) * 128],
                            wqkvT[:, k, o0:o0 + ow],
                            start=(k == 0),
                            stop=(k == NC_C - 1),
                        )
                    dst = (
                        v_ext[tt][:, 0:NH * VW]
                        .rearrange("p (h e) -> p h e", e=VW)[:, h0:h0 + nh, 0:HD]
                    )
                    nc.vector.tensor_copy(
                        dst, ps[:, :ow].rearrange("p (h e) -> p h e", e=HD)
                    )

                # extras[p][T]: QKV work dependencies only require:
                #   v chunk-0 tile T ready before pair-0 PV consumes it at
                #   slot T+1; pair p+1's q/k ready before pair p+1.
                # The dep-driven scheduler fills PE idle time with these
                # (they run at low priority).
                extras = [[[] for _ in range(NC_T)] for _ in range(NPAIR)]

                def TH(f, *a):
                    return lambda: f(*a)

                for tt in range(NC_T):
                    extras[0][tt].append(TH(emit_v_chunk, tt, 0))
                for p in range(1, NPAIR - 1):
                    extras[p][2].append(TH(emit_qk_chunk, p + 1, 0))
                    extras[p][3].append(TH(emit_qk_chunk, NPAIR + p + 1, 0))
                    extras[p][5].append(TH(emit_qk_chunk, p + 1, 1))
                    extras[p][6].append(TH(emit_qk_chunk, NPAIR + p + 1, 1))
                for i in range(NC_T):  # v chunk-1 (needed by pair 4's PV)
                    extras[1 + i // 3][[1, 4, 7][i % 3]].append(
                        TH(emit_v_chunk, i, 1)
                    )

                # ---- HAM warm-up: keep the PE busy through the DMA
                # lead-in so the first real matmuls run at full clock ----
                # 28 matmuls (~7-8us): long enough to cover the input-DMA
                # wait so the PE never sees a >3.4us idle window (which
                # would re-throttle HAM and run the prologue at 1.2 GHz)
                wu = consts.tile([128, 512], FP16, tag="wu", name="wu")
                nc.vector.memset(wu[:], 0.0)
                wups = bg_tile("wups")
                NWU = 16
                for i in range(NWU):
                    nc.tensor.matmul(
                        wups[:], wu[:, 0:128], wu[:],
                        start=(i == 0), stop=(i == NWU - 1),
                    )

                # ---- prologue: q/k for pairs 0 and 1 up front (low
                # priority so pair-0 score matmuls preempt as soon as
                # their chunks land) ----
                with tc.high_priority(offset=LOWPRI):
                    emit_qk_chunk(0, 0)
                    emit_qk_chunk(NPAIR, 0)
                    emit_qk_chunk(1, 0)
                    emit_qk_chunk(NPAIR + 1, 0)
                    emit_qk_chunk(0, 1)
                    emit_qk_chunk(NPAIR, 1)
                    emit_qk_chunk(1, 1)
                    emit_qk_chunk(NPAIR + 1, 1)

                # ---- attention: software-pipelined slot stream ----
                pts_of = {}     # (p, T) -> pt tile [128, 2048] h-major
                pv0_of = {}     # p -> [pv0_h0, pv0_h1]  (chunk-0 accums)
                pv1_of = {}     # last pair only: incremental chunk-1 accums
                sums_sb_of = {}

                def sc_mm(st, p, T, c):
                    kT_t = qkT[NPAIR + p]
                    qT_t = qkT[p]
                    for h in range(2):
                        r0 = h * 64
                        nc.tensor.matmul(
                            st[:, h * 512:(h + 1) * 512],
                            kT_t[r0:r0 + 64, T * 128:(T + 1) * 128],
                            qT_t[r0:r0 + 64, c * 512:(c + 1) * 512],
                            start=True,
                            stop=True,
                        )

                def slot(p, T):
                    if T == 0:
                        if p < NPAIR - 1:
                            sums_sb_of[p] = normp.tile(
                                [1, 2 * S], F32, tag="sums", name="sums",
                                bufs=2
                            )
                        pv0_of[p] = [
                            pvps.tile([128, 512], F32, tag=f"pva{h}",
                                      name=f"pva{h}", bufs=1)
                            for h in range(2)
                        ]
                        if p == NPAIR - 1:
                            # last pair: chunk-1 accumulates incrementally
                            # in the (now extras-free) background banks so
                            # the epilogue isn't serialized behind a burst;
                            # sums go to per-half tiles so each half's
                            # reciprocal fires as soon as its rows land
                            pv1_of[p] = [bg_tile(f"pvL{h}") for h in range(2)]
                    st_a = stps.tile([128, S], F32, tag="stA", name="stA",
                                     bufs=1)
                    st_b = stps.tile([128, S], F32, tag="stB", name="stB",
                                     bufs=1)
                    # per-half pt tiles: chunk-0 consumers only RAW-wait on
                    # exp_A, chunk-1 only on exp_B
                    ptA = ptpool.tile([128, S], FP16, tag=f"ptA{T}",
                                      name=f"ptA{T}", bufs=2)
                    ptB = ptpool.tile([128, S], FP16, tag=f"ptB{T}",
                                      name=f"ptB{T}", bufs=2)
                    pts_of[(p, T)] = (ptA, ptB)

                    sc_mm(st_a, p, T, 0)
                    nc.scalar.activation(
                        out=ptA[:].rearrange("p (h q) -> p h q", h=2),
                        in_=st_a[:].rearrange("p (h q) -> p h q", h=2),
                        func=mybir.ActivationFunctionType.Exp,
                        scale=float(SCALE),
                    )
                    # the very last slot's PV feeds the epilogue critical
                    # path: normal priority so the finish chain isn't
                    # stuck behind projection-head filler
                    last_slot = p == NPAIR - 1 and T == NC_T - 1
                    with tc.high_priority(offset=0 if last_slot else LOWPRI):
                        if T > 0:
                            for h in range(2):
                                nc.tensor.matmul(
                                    pv0_of[p][h][:],
                                    v_ext[T - 1][
                                        :, (2 * p + h) * VW:(2 * p + h) * VW + 128
                                    ],
                                    pts_of[(p, T - 1)][0][:, h * 512:(h + 1) * 512],
                                    start=(T == 1),
                                    stop=(T == NC_T - 1),
                                )
                        if p == NPAIR - 1 and T > 0:
                            for h in range(2):
                                nc.tensor.matmul(
                                    pv1_of[p][h][:],
                                    v_ext[T - 1][
                                        :, (2 * p + h) * VW:(2 * p + h) * VW + 128
                                    ],
                                    pts_of[(p, T - 1)][1][:, h * 512:(h + 1) * 512],
                                    start=(T == 1),
                                    stop=(T == NC_T - 1),
                                )
                        for th in extras[p][T]:
                            th()
                    sc_mm(st_b, p, T, 1)
                    nc.scalar.activation(
                        out=ptB[:].rearrange("p (h q) -> p h q", h=2),
                        in_=st_b[:].rearrange("p (h q) -> p h q", h=2),
                        func=mybir.ActivationFunctionType.Exp,
                        scale=float(SCALE),
                    )

                def scalar_recip(dst, src):
                    nc.scalar.add_instruction(
                        mybir.InstActivation(
                            name=nc.get_next_instruction_name(),
                            ins=[
                                nc.scalar.lower_ap(src),
                                mybir.ImmediateValue(
                                    dtype=mybir.dt.float32, value=0.0
                                ),
                                mybir.ImmediateValue(
                                    dtype=mybir.dt.float32, value=1.0
                                ),
                                mybir.ImmediateValue(
                                    dtype=mybir.dt.float32, value=0.0
                                ),
                            ],
                            outs=[nc.scalar.lower_ap(dst)],
                            func=mybir.ActivationFunctionType.Reciprocal,
                        )
                    )

                recip5 = {}

                def finish_c0(p, hot=False):
                    from contextlib import nullcontext

                    for h in range(2):
                        nc.tensor.matmul(
                            pv0_of[p][h][:],
                            v_ext[NC_T - 1][
                                :, (2 * p + h) * VW:(2 * p + h) * VW + 128
                            ],
                            pts_of[(p, NC_T - 1)][0][:, h * 512:(h + 1) * 512],
                            start=False,
                            stop=True,
                        )
                    if hot:
                        with tc.high_priority():
                            for h in range(2):
                                nc.vector.tensor_copy(
                                    sums_sb_of[p][0:1, h * S:h * S + 512],
                                    pv0_of[p][h][HD:HD + 1, :],
                                )
                            for h in range(2):
                                nc.vector.tensor_copy(
                                    attnTa[p][h * 64:(h + 1) * 64, :],
                                    pv0_of[p][h][0:HD, :],
                                )
                            norm_half_bounce(p, 0)
                        del pv0_of[p]
                        return
                    if p == NPAIR - 1:
                        # preload the reciprocal ACT table set (the real
                        # reciprocals would otherwise pay the ~2.7us table
                        # switch on the critical tail), then the c0-half
                        # reciprocals straight off the PSUM sums rows --
                        # no DVE copy in the chain
                        scalar_recip(
                            normp.tile([1, 1], F32, tag="rscr", name="rscr",
                                       bufs=1)[:],
                            ones64[0:1, 0:1],
                        )
                        recip5[0] = normp.tile([1, 2, 512], FP16, tag="rc0",
                                               name="rc0", bufs=1)
                        for h in range(2):
                            scalar_recip(
                                recip5[0][0:1, h, :],
                                pv0_of[p][h][HD:HD + 1, :],
                            )
                    else:
                        for h in range(2):
                            nc.vector.tensor_copy(
                                sums_sb_of[p][0:1, h * S:h * S + 512],
                                pv0_of[p][h][HD:HD + 1, :],
                            )
                    for h in range(2):
                        nc.vector.tensor_copy(
                            attnTa[p][h * 64:(h + 1) * 64, :],
                            pv0_of[p][h][0:HD, :],
                        )
                    del pv0_of[p]

                def finish_c1_last(p):
                    for h in range(2):
                        nc.tensor.matmul(
                            pv1_of[p][h][:],
                            v_ext[NC_T - 1][
                                :, (2 * p + h) * VW:(2 * p + h) * VW + 128
                            ],
                            pts_of[(p, NC_T - 1)][1][:, h * 512:(h + 1) * 512],
                            start=False,
                            stop=True,
                        )
                    recip5[1] = normp.tile([1, 2, 512], FP16, tag="rc1",
                                           name="rc1", bufs=1)
                    for h in range(2):
                        scalar_recip(
                            recip5[1][0:1, h, :],
                            pv1_of[p][h][HD:HD + 1, :],
                        )
                    # attnTb copies are deferred to norm5_half(1) so the
                    # c0-half normalize multiplies run first on the DVE
                    for Tq in range(NC_T):
                        del pts_of[(p, Tq)]

                def burst_c1(p, hot=False):
                    # hot: the evac/norm chain (NOT the matmuls) goes at
                    # top priority so the DVE queue doesn't defer it past
                    # the epilogue -- the projection heads RAW-depend on
                    # this pair's normalize multiplies
                    from contextlib import nullcontext

                    pv1 = [bg_tile(f"pvb{h}") for h in range(2)]
                    for Tq in range(NC_T):
                        for h in range(2):
                            nc.tensor.matmul(
                                pv1[h][:],
                                v_ext[Tq][
                                    :, (2 * p + h) * VW:(2 * p + h) * VW + 128
                                ],
                                pts_of[(p, Tq)][1][:, h * 512:(h + 1) * 512],
                                start=(Tq == 0),
                                stop=(Tq == NC_T - 1),
                            )
                    with tc.high_priority() if hot else nullcontext():
                        for h in range(2):
                            nc.vector.tensor_copy(
                                sums_sb_of[p][0:1, h * S + 512:h * S + 1024],
                                pv1[h][HD:HD + 1, :],
                            )
                            nc.vector.tensor_copy(
                                attnTb[p][h * 64:(h + 1) * 64, :],
                                pv1[h][0:HD, :],
                            )
                        for Tq in range(NC_T):
                            del pts_of[(p, Tq)]
                        if hot:
                            norm_half_bounce(p, 1)
                        else:
                            norm(p)

                def norm_half_bounce(p, c):
                    # one q-column half of a pair's normalization via the
                    # DRAM bounce; used for pair 4 so the c0 half is done
                    # before the projection heads need attnTa[4]
                    at = (attnTa, attnTb)[c][p]
                    sums_sb = sums_sb_of[p]
                    src = bass.AP(
                        tensor=sums_sb.tensor,
                        offset=sums_sb.offset + c * 512,
                        ap=[list(sums_sb[:].ap[0])] + [[1024, 2], [1, 512]],
                    )
                    sd = rdram.tile([1, S], F32, tag=f"hsd{c}", name="hsd")
                    nc.gpsimd.dma_start(out=sd[:], in_=src)
                    sr = normp.tile([128, 8], F32, tag="hsr", name="hsr")
                    nc.gpsimd.dma_start(
                        out=sr[:],
                        in_=bass.AP(
                            tensor=sd.tensor,
                            offset=sd.offset,
                            ap=[[8, 128], [1, 8]],
                        ),
                    )
                    rr = normp.tile([128, 8], F32, tag="hrr", name="hrr")
                    nc.vector.reciprocal(rr[:], sr[:])
                    rd = rdram.tile([1, S], F32, tag=f"hrd{c}", name="hrd")
                    nc.gpsimd.dma_start(
                        out=bass.AP(
                            tensor=rd.tensor,
                            offset=rd.offset,
                            ap=[[8, 128], [1, 8]],
                        ),
                        in_=rr[:],
                    )
                    rb = normp.tile([128, 512], F32, tag="hrb", name="hrb")
                    for h in range(2):
                        row = rd[0:1, h * 512:(h + 1) * 512]
                        row_bc = bass.AP(
                            tensor=row.tensor,
                            offset=row.offset,
                            ap=[[0, 64]] + list(row.ap[1:]),
                        )
                        nc.gpsimd.dma_start(
                            out=rb[h * 64:(h + 1) * 64, :], in_=row_bc
                        )
                    nc.vector.tensor_mul(at[:], at[:], rb[:])

                def norm(p):
                    # reciprocal of the 2048 sums: repartition [1,2048] ->
                    # [128,16] via a DRAM bounce (issued on the idle GpSimd
                    # SWDGE queue) so the 8-cycle/element DVE divide runs
                    # on 128 lanes (pairs 0-4; the last pair is handled by
                    # norm5_half on the epilogue path)
                    sums_sb = sums_sb_of[p]
                    rd = rdram.tile([1, 2 * S], F32, tag="rd", name="rd")
                    sd = rdram.tile([1, 2 * S], F32, tag="sd", name="sd")
                    nc.gpsimd.dma_start(out=sd[:], in_=sums_sb[:])
                    sr = normp.tile([128, 16], F32, tag="sr", name="sr")
                    nc.gpsimd.dma_start(
                        out=sr[:],
                        in_=bass.AP(
                            tensor=sd.tensor,
                            offset=sd.offset,
                            ap=[[16, 128], [1, 16]],
                        ),
                    )
                    rr = normp.tile([128, 16], F32, tag="rr", name="rr")
                    nc.vector.reciprocal(rr[:], sr[:])
                    nc.gpsimd.dma_start(
                        out=bass.AP(
                            tensor=rd.tensor,
                            offset=rd.offset,
                            ap=[[16, 128], [1, 16]],
                        ),
                        in_=rr[:],
                    )
                    rb = normp.tile([128, S], F32, tag="rb", name="rb")
                    for h in range(2):
                        row = rd[0:1, h * S:(h + 1) * S]
                        row_bc = bass.AP(
                            tensor=row.tensor,
                            offset=row.offset,
                            ap=[[0, 64]] + list(row.ap[1:]),
                        )
                        nc.gpsimd.dma_start(
                            out=rb[h * 64:(h + 1) * 64, :], in_=row_bc
                        )
                    rb_r = rb[:].rearrange("d (c q) -> d c q", q=512)
                    nc.vector.tensor_mul(
                        attnTa[p][:], attnTa[p][:], rb_r[:, 0, :]
                    )
                    nc.vector.tensor_mul(
                        attnTb[p][:], attnTb[p][:], rb_r[:, 1, :]
                    )

                def norm5_half(c):
                    # last pair, one column half: broadcast 1/sums via two
                    # matmuls into a freed pva bank (bg banks still hold
                    # the unread chunk-1 accumulators), then normalize
                    p = NPAIR - 1
                    at = (attnTa, attnTb)[c][p]
                    bc = pvps.tile([128, 512], F32, tag=f"pva{c}",
                                   name=f"bc{c}", bufs=1)
                    for h in range(2):
                        nc.tensor.matmul(
                            bc[h * 64:(h + 1) * 64, :],
                            ones64[0:1, :],
                            recip5[c][0:1, h, :],
                            start=True,
                            stop=True,
                        )
                    if c == 1:
                        for h in range(2):
                            nc.vector.tensor_copy(
                                at[h * 64:(h + 1) * 64, :],
                                pv1_of[p][h][0:HD, :],
                            )
                        del pv1_of[p]
                    for h in range(2):
                        nc.vector.tensor_mul(
                            at[h * 64:(h + 1) * 64, :],
                            at[h * 64:(h + 1) * 64, :],
                            bc[h * 64:(h + 1) * 64, :],
                        )

                # emission order: chunk-1 burst of pair p-1 deferred past
                # the next pair's first two slots (low priority keeps it
                # out of the scores' way).  Pair 4's burst is un-deferred
                # (the bg banks belong to pair 5's incremental chunk-1
                # during pair 5), and pair 5 finishes both chunks inline.
                for p in range(NPAIR):
                    slot(p, 0)
                    slot(p, 1)
                    if 0 < p < NPAIR - 1:
                        with tc.high_priority(offset=LOWPRI):
                            burst_c1(p - 1)
                    for T in range(2, NC_T):
                        slot(p, T)
                    if p == NPAIR - 1:
                        # the last pair's finish/normalize chain is the
                        # epilogue critical path: absolute top priority so
                        # it always outranks the projection-head filler
                        with tc.high_priority():
                            finish_c0(p)
                            finish_c1_last(p)
                    else:
                        with tc.high_priority(offset=LOWPRI):
                            finish_c0(p, hot=(p == NPAIR - 2))
                            if p == NPAIR - 2:
                                burst_c1(p, hot=True)
                # pair-5 norm is emitted between the first two projection
                # heads: the PE instruction stream is static, so the bc
                # matmuls must sit AFTER ~4us of head matmuls to cover the
                # ScalarE reciprocal (+table load) latency without a stall

                # ---------------- output projection ----------------
                # PSUM ping-pongs the freed score tiles (tags stA/stB).
                # Depth-2 pipeline: each tile's pair-5 matmul (gated by the
                # last normalization) is deferred past the next tile's
                # early matmuls.  Output stores go out fp16 on the GpSimd
                # queue.
                def proj_head(tt):
                    ps = stps.tile([128, S], F32,
                                   tag=("stA", "stB")[tt % 2], name=f"prj{tt}",
                                   bufs=1)
                    for o0, ow in [(0, 512), (512, 256)]:
                        for p in range(NPAIR - 1):
                            nc.tensor.matmul(
                                ps[:, o0:o0 + ow],
                                attn_q(p, tt),
                                wprojT[:, p, o0:o0 + ow],
                                start=(p == 0),
                                stop=False,
                            )
                    return ps

                def proj_tail(tt, ps):
                    for o0, ow in [(0, 512), (512, 256)]:
                        nc.tensor.matmul(
                            ps[:, o0:o0 + ow],
                            attn_q(NPAIR - 1, tt),
                            wprojT[:, NPAIR - 1, o0:o0 + ow],
                            start=False,
                            stop=True,
                        )
                    ob = outp.tile([128, DIM], FP16, tag="ob", name="ob")
                    nc.vector.tensor_copy(ob[:, 0:512], ps[:, 0:512])
                    nc.scalar.copy(out=ob[:, 512:768], in_=ps[:, 512:768])
                    nc.gpsimd.dma_start(
                        out=out_ext[tt * 128:(tt + 1) * 128, :], in_=ob[:]
                    )

                # [head0, bc_c0, head1, bc_c1, tail0, head2, tail1, ...]:
                # each norm half sits behind a head's worth of PE work so
                # the split reciprocals are ready when the PE reaches the
                # bc matmuls, and tails 0-3 only need the c0 half
                pending = None
                for tt in range(NC_T):
                    ps = proj_head(tt)
                    if tt <= 1:
                        with tc.high_priority():
                            norm5_half(tt)
                    if pending is not None:
                        proj_tail(*pending)
                    pending = (tt, ps)
                proj_tail(*pending)

    nc.finalize()
    return nc


_NC_CACHE = None


def kernel(**inputs) -> np.ndarray:
    global _NC_CACHE
    x = np.asarray(inputs["x"], dtype=np.float32)
    w_qkv = np.asarray(inputs["w_qkv"], dtype=np.float32)
    w_proj = np.asarray(inputs["w_proj"], dtype=np.float32)
    b_proj = np.asarray(inputs["b_proj"], dtype=np.float32)
    B, H, W, C = x.shape
    assert (B, H * W, C) == (8, S, DIM)

    # host-side sharding + layout prep: channel-major fp16 operands
    wqkvT = np.ascontiguousarray(w_qkv.T).astype(np.float16)       # [768, 2304]
    wprojT = np.ascontiguousarray(w_proj.T).astype(np.float16)     # [768, 768]
    xTs = [
        np.ascontiguousarray(x[b].reshape(S, DIM).T).astype(np.float16)
        for b in range(B)
    ]

    if _NC_CACHE is None:
        _NC_CACHE = build_bass()
    nc = _NC_CACHE

    in_maps = [
        {"xT": xTs[b], "w_qkvT": wqkvT, "w_projT": wprojT}
        for b in range(B)
    ]
    res = run_bass_kernel_spmd(nc, in_maps, list(range(B)))
    out = np.stack(
        [
            np.asarray(res.results[b]["out"]).astype(np.float32).reshape(H, W, C)
            for b in range(B)
        ]
    )
    return (out + b_proj.reshape(1, 1, 1, C)).astype(np.float32)


if __name__ == "__main__":
    rng = np.random.default_rng(0)
    ins = {
        "x": rng.standard_normal((8, 32, 32, DIM), dtype=np.float32),
        "w_qkv": rng.standard_normal((3 * DIM, DIM), dtype=np.float32)
        * DIM ** -0.5,
        "w_proj": rng.standard_normal((DIM, DIM), dtype=np.float32) * DIM ** -0.5,
        "b_proj": np.zeros(DIM, dtype=np.float32),
    }
    o = kernel(**ins)
    print(o.shape, o.dtype)

